# revision 47
# baseline (speedup 1.0000x reference)
"""Trainium2 Bass kernel for single-head causal attention.

x:[4,4096,1024] f32, W_q/W_k/W_v:[1024,64], W_o:[64,1024].

Sharding: 8 cores = 4 batches x 2 query-stripe roles. Role r of a batch
owns query blocks {2j+r : j=0..3} (512 queries each). Program slot j has
key extent E[j] = (8j+8) 128-key chunks, which exactly covers role 1's
block 2j+1 and over-covers role 0's block 2j by 4 chunks (dead).

All per-core differences (which batch, which stripe, dead chunks) are
carried in the input data; one SPMD program runs on all 8 cores:
- x is shipped as [128, 4096, 8] (d_model-chunk partition, seq, chunk)
  so any column range is a fully contiguous DMA (no sub-512B-element
  descriptor penalty), streamed in arrival-ordered bites.
- The diagonal causal band masks (4 x [128,512]) are generated on the
  otherwise-idle Pool engine with affine_select, not DMAed.
- Partner-block chunks (band >= 4) take no mask op at all: the exp runs
  with a per-partition bias from the w tensor (-30 for role 0 => p ~
  1e-10, 0 for role 1), so role-0's dead chunks vanish from both the PV
  numerator and the denominator row.

bf16 everywhere on the matmul paths (end-to-end rel err ~5e-3 vs the
2e-2 gate). Engines: PE does all matmuls (62.3us busy, the roofline of
this schedule), ACT does exp + the slot-3 ot copies + half its output
scales, DVE does diag masks + kvt/qt copies + finalize + the other
scales. Attention chunks are emitted in waves matched to DMA arrival,
next wave's KV projection interleaved between chunks as PE filler, PV
matmuls lagged 10 chunks behind their scores. The slot-3 tail runs a
per-128-query finalize pipeline (ot copy and 1/den read the PV PSUM
bank directly) with per-tile y DMAs so the final HWDGE descriptor-gen
chain (625ns each, serialized) stays short.
"""

import sys

for _p in ("/opt/trn_rl_repo",):
    if _p not in sys.path:
        sys.path.insert(0, _p)

import numpy as np

D_MODEL = 1024
D_HEAD = 64
SEQ = 4096
BATCH = 4
NCORES = 8
NQ = 2048          # queries per core
P = 128
DCH = D_MODEL // P  # 8 contraction chunks
NSLOT = 4           # query slots of 512
E = [8, 16, 24, 32]  # key chunks per slot
NWAVE = 8           # key superchunks of 512

# Attention chunks per wave: matched to DMA arrival order (early waves
# light), per-slot ascending kc, every chunk (j,kc) in wave >= kc//4.
WAVES = [
    [(0, k) for k in range(4)],
    [(0, k) for k in range(4, 8)] + [(1, k) for k in range(4)],
    [(1, k) for k in range(4, 8)] + [(2, k) for k in range(4)],
    [(1, k) for k in range(8, 12)] + [(2, k) for k in range(4, 8)]
    + [(3, k) for k in range(4)],
    [(1, k) for k in range(12, 16)] + [(2, k) for k in range(8, 12)]
    + [(3, k) for k in range(4, 8)],
    [(2, k) for k in range(12, 24)],
    [(3, k) for k in range(8, 22)],
    [(3, k) for k in range(22, 32)],
]

_prog = None


def _check_waves():
    seen = {}
    total = 0
    for w, wv in enumerate(WAVES):
        for j, kc in wv:
            assert kc // 4 <= w, (w, j, kc)
            assert seen.get(j, -1) == kc - 1, (j, kc)
            seen[j] = kc
            total += 1
    assert total == sum(E) == 80
    return {j: max(w for w, wv in enumerate(WAVES) if (j, E[j] - 1) in wv)
            for j in range(NSLOT)}


def _build_program():
    import concourse.bacc as bacc
    import concourse.mybir as mybir
    import concourse.tile as tile
    from concourse.masks import make_identity

    fp32 = mybir.dt.float32
    f32r = mybir.dt.float32r
    bf16 = mybir.dt.bfloat16
    nc = bacc.Bacc("TRN2", target_bir_lowering=False, debug=False)

    # w layout: [wq 0:512 | wkv 512:1536 | rbias 1536:1540]
    # rbias: -30 for role 0, 0 for role 1. Partner-band chunks (band >= 4)
    # run exp with this per-partition bias instead of a 0/1 mask multiply:
    # each slot's band region only ever covers its own partner block, which
    # is entirely dead for role 0 (exp(s - 30) ~ 1e-10) and entirely alive
    # for role 1 (bias 0). Replaces the 16 explicit partner-mask DVE ops.
    xt = nc.dram_tensor("xt", [P, SEQ, DCH], bf16, kind="ExternalInput")
    w = nc.dram_tensor("w", [P, DCH * 192 + 4], bf16, kind="ExternalInput")
    wo = nc.dram_tensor("wo", [D_HEAD, D_MODEL], bf16, kind="ExternalInput")
    y = nc.dram_tensor("y", [NSLOT, P, 4, D_MODEL], bf16, kind="ExternalOutput")

    last_wave = _check_waves()
    # output-projection ops (j, i): slot0 -> waves 2,3; slot1 -> 5,6;
    # slot2 -> wave 6 (where DVE is light: slot-3 chunks kc<24 need no
    # masks); slot3 -> tail (wave index NWAVE)
    out_sched = {wi: [] for wi in range(NWAVE + 1)}
    for j, tgt in ((0, (2, 3)), (1, (5, 6)), (2, (6, 6)), (3, (8, 8))):
        for i in range(8):
            out_sched[tgt[i // 4]].append((j, i))

    with tile.TileContext(nc) as tc:
        with (
            tc.tile_pool(name="singles", bufs=1) as singles,
            tc.tile_pool(name="work", bufs=10) as work,
            tc.tile_pool(name="ypool", bufs=2) as ypool,
            tc.tile_pool(name="mm_ps", bufs=1, space="PSUM") as mm_ps,
            tc.tile_pool(name="s_ps", bufs=3, space="PSUM") as s_ps_pool,
            tc.tile_pool(name="pv_ps", bufs=1, space="PSUM") as pv_pool,
        ):
            # ---- persistent SBUF ----
            w_sb = singles.tile([P, DCH * 192 + 4], bf16, tag="w_sb")
            xt_sb = singles.tile([P, SEQ, DCH], bf16, tag="xt_sb")
            msk_sb = singles.tile([P, 4, 512], bf16, tag="msk_sb")
            wo_sb = singles.tile([D_HEAD, D_MODEL], bf16, tag="wo_sb")
            kvt = singles.tile([P, SEQ], bf16, tag="kvt")  # 0:64 K^T, 64:128 V^T
            qt_sb = singles.tile([D_HEAD, NQ], bf16, tag="qt_sb")
            vaug = singles.tile([P, 32, D_HEAD + 1], bf16, tag="vaug")
            ot = singles.tile([D_HEAD + 1, NQ], bf16, tag="ot")
            rden = singles.tile([1, NQ], fp32, tag="rden")
            rbias = singles.tile([P, 1], fp32, tag="rbias")
            rdent = singles.tile([P, 16], fp32, tag="rdent")
            ident = singles.tile([P, D_HEAD], bf16, tag="ident")
            one_sb = singles.tile([1, 1], fp32, tag="one_sb")

            # ---- input DMAs (SP queue, ordered by first use) ----
            def ld_x(dst, src, s0):
                nc.sync.dma_start(
                    out=dst[:, s0 : s0 + 512, :], in_=src[:, s0 : s0 + 512, :]
                )

            def ld_xh(dst, src, s0, n=512):
                nc.sync.dma_start(
                    out=dst[:, s0 : s0 + n, :], in_=src[:, s0 : s0 + n, :]
                )

            nc.sync.dma_start(out=w_sb[:, 0:512], in_=w[:, 0:512])
            ld_xh(xt_sb, xt, 0, 128)
            nc.sync.dma_start(out=w_sb[:, 512:1536], in_=w[:, 512:1536])
            ld_xh(xt_sb, xt, 128, 256)
            ld_xh(xt_sb, xt, 384, 128)
            nc.sync.dma_start(out=w_sb[:, 1536:1540], in_=w[:, 1536:1540])
            ld_x(xt_sb, xt, 512)       # chunk 1: wave-1 fresh keys
            ld_x(xt_sb, xt, 1024)      # chunk 2: slot-1 queries
            ld_x(xt_sb, xt, 2048)      # chunk 4: slot-2 queries
            nc.sync.dma_start(out=wo_sb, in_=wo[:, :])
            ld_x(xt_sb, xt, 1536)      # chunk 3
            ld_x(xt_sb, xt, 3072)      # chunk 6: slot-3 queries
            ld_x(xt_sb, xt, 2560)      # chunk 5
            ld_x(xt_sb, xt, 3584)      # chunk 7

            nc.vector.memset(one_sb, 1.0)
            nc.vector.memset(vaug[:, :, D_HEAD : D_HEAD + 1], 1.0)
            make_identity(nc, ident[D_HEAD:P, :])
            # fp32 per-partition exp bias (see w layout note)
            nc.vector.tensor_copy(out=rbias, in_=w_sb[:, 1536:1537])
            # causal band masks generated on the idle Pool engine:
            # msk_sb[p, c, jq] = 1 if 128c + p <= jq else 0, per band c
            for c in range(4):
                nc.gpsimd.memset(msk_sb[:, c, :], 1.0)
                nc.gpsimd.affine_select(
                    out=msk_sb[:, c, :],
                    in_=msk_sb[:, c, :],
                    compare_op=mybir.AluOpType.is_ge,
                    fill=0.0,
                    base=-128 * c,
                    channel_multiplier=-1,
                    pattern=[[1, 512]],
                )

            def kv_proj_mms(sc):
                """Generator: one KV-projection matmul per next() call."""
                kp = mm_ps.tile([P, 512], fp32, tag="mm")
                for dc in range(DCH):
                    nc.tensor.matmul(
                        kp,
                        lhsT=w_sb[:, 512 + dc * 128 : 512 + dc * 128 + 128],
                        rhs=xt_sb[:, sc * 512 : (sc + 1) * 512, dc],
                        start=(dc == 0),
                        stop=(dc == DCH - 1),
                    )
                    yield
                nc.vector.tensor_copy(
                    out=kvt[:, sc * 512 : (sc + 1) * 512], in_=kp
                )
                yield

            def transposes(sc):
                for t in range(4):  # V^T 128-col blocks -> natural V chunks
                    kc = sc * 4 + t
                    tp = s_ps_pool.tile([P, D_HEAD], bf16, tag="s_ps")
                    nc.tensor.transpose(
                        tp,
                        kvt[D_HEAD:P, kc * P : (kc + 1) * P],
                        ident[D_HEAD:P, :],
                    )
                    nc.vector.tensor_copy(out=vaug[:, kc, :D_HEAD], in_=tp)

            # PV accumulators: full-bank tiles, PV uses rows 0:65
            pv = [
                pv_pool.tile([P, 512], fp32, tag=f"pv{g}", name=f"pv{g}")
                for g in range(NSLOT)
            ]
            freed = []  # pv banks released by finalized slots
            y_tiles = {}
            ncopy = [0]
            pending_pv = []

            def emit_pv(j, kc):
                nc.tensor.matmul(
                    pv[j][0 : D_HEAD + 1, :],
                    lhsT=vaug[:, kc, :],
                    rhs=pending_pv_pt.pop((j, kc)),
                    start=(kc == 0),
                    stop=(kc == E[j] - 1),
                    skip_group_check=True,
                )

            pending_pv_pt = {}
            tail_pt = {}

            def emit_chunk(j, kc, wv=0):
                sps = s_ps_pool.tile([P, 512], fp32, tag="s_ps")
                nc.tensor.matmul(
                    sps,
                    lhsT=kvt[0:D_HEAD, kc * P : (kc + 1) * P],
                    rhs=qt_sb[:, j * 512 : (j + 1) * 512],
                    start=True,
                    stop=True,
                )
                p_t = work.tile([P, 512], bf16, tag="p_t")
                band = kc - (E[j] - 8)
                if band >= 4:
                    # partner block: role-0 kills the whole chunk via the
                    # exp bias (exp(s - 30) ~ 1e-10); role 1 keeps it whole
                    nc.scalar.activation(
                        p_t, sps, mybir.ActivationFunctionType.Exp,
                        bias=rbias[:, 0:1],
                    )
                else:
                    nc.scalar.activation(
                        p_t, sps, mybir.ActivationFunctionType.Exp
                    )
                if 0 <= band < 4:
                    # diagonal band: per-query causal step mask
                    nc.vector.tensor_tensor(
                        p_t, p_t, msk_sb[:, band, :], mybir.AluOpType.mult
                    )
                pending_pv_pt[(j, kc)] = p_t
                pending_pv.append((j, kc))
                if len(pending_pv) > 10:
                    emit_pv(*pending_pv.pop(0))

            def emit_out_op(j, i, bank=None):
                """One output-projection matmul + scaled PSUM->SBUF copy."""
                t, no = i // 2, i % 2
                if bank is None:
                    bank = freed[ncopy[0] % len(freed)]
                    ncopy[0] += 1
                q0 = j * 512 + t * P
                nc.tensor.matmul(
                    bank,
                    lhsT=ot[0:D_HEAD, q0 : q0 + P],
                    rhs=wo_sb[:, no * 512 : (no + 1) * 512],
                    start=True,
                    stop=True,
                )
                if j == 3 and i % 2 == 1:
                    nc.scalar.mul(
                        y_tiles[j][:, t, no * 512 : (no + 1) * 512],
                        bank,
                        rdent[:, 4 * j + t : 4 * j + t + 1],
                    )
                else:
                    nc.vector.tensor_scalar_mul(
                        y_tiles[j][:, t, no * 512 : (no + 1) * 512],
                        bank,
                        rdent[:, 4 * j + t : 4 * j + t + 1],
                    )
                if j == 3:
                    if i % 2 == 1:
                        # per-tile DMA: few enough that HWDGE desc-gen
                        # (625ns each, serialized) stays off the tail path
                        nc.sync.dma_start(
                            out=y[j][:, t : t + 1, :],
                            in_=y_tiles[j][:, t : t + 1, :],
                        )
                elif i == 3:
                    nc.sync.dma_start(
                        out=y[j][:, 0:2, :], in_=y_tiles[j][:, 0:2, :]
                    )
                elif i == 7:
                    nc.sync.dma_start(
                        out=y[j][:, 2:4, :], in_=y_tiles[j][:, 2:4, :]
                    )

            def finalize_half(j, h):
                """Half of slot-j finalize: O^T + den, 1/den, rdent cols."""
                c0 = j * 512 + h * 256
                c1 = c0 + 256
                nc.vector.tensor_copy(
                    out=ot[:, c0:c1], in_=pv[j][0 : D_HEAD + 1, h * 256 : h * 256 + 256]
                )
                nc.vector.reciprocal(
                    rden[:, c0:c1], ot[D_HEAD : D_HEAD + 1, c0:c1]
                )
                for t in (2 * h, 2 * h + 1):
                    nc.tensor.matmul(
                        pv[j][:, t : t + 1],
                        lhsT=rden[:, j * 512 + t * P : j * 512 + (t + 1) * P],
                        rhs=one_sb,
                        start=True,
                        stop=True,
                    )
                nc.vector.tensor_copy(
                    out=rdent[:, 4 * j + 2 * h : 4 * j + 2 * h + 2],
                    in_=pv[j][:, 2 * h : 2 * h + 2],
                )

            def finalize(j):
                finalize_half(j, 0)
                finalize_half(j, 1)
                freed.append(pv[j])
                y_tiles[j] = ypool.tile(
                    [P, 4, D_MODEL], bf16, tag="y_sb", name=f"y{j}"
                )

            def q_piece(c0, n):
                """Prologue Q projection over columns [c0, c0+n) of slot 0.
                Uses the s_ps pool so pieces rotate PSUM banks instead of
                serializing on the single mm bank."""
                qp = s_ps_pool.tile([D_HEAD, n], fp32, tag="s_ps", name=f"qp{c0}")
                for dc in range(DCH):
                    nc.tensor.matmul(
                        qp,
                        lhsT=w_sb[:, dc * 64 : dc * 64 + 64],
                        rhs=xt_sb[:, c0 : c0 + n, dc],
                        start=(dc == 0),
                        stop=(dc == DCH - 1),
                    )
                nc.vector.tensor_copy(out=qt_sb[:, c0 : c0 + n], in_=qp)

            def kv_piece(c0, n):
                kp = s_ps_pool.tile([P, n], fp32, tag="s_ps", name=f"kp{c0}")
                for dc in range(DCH):
                    nc.tensor.matmul(
                        kp,
                        lhsT=w_sb[:, 512 + dc * 128 : 512 + dc * 128 + 128],
                        rhs=xt_sb[:, c0 : c0 + n, dc],
                        start=(dc == 0),
                        stop=(dc == DCH - 1),
                    )
                nc.vector.tensor_copy(out=kvt[:, c0 : c0 + n], in_=kp)

            def q_proj_steps(j):
                """Generator version of q_proj: one matmul per next()."""
                qp = mm_ps.tile([D_HEAD, 512], fp32, tag="mm")
                for dc in range(DCH):
                    nc.tensor.matmul(
                        qp,
                        lhsT=w_sb[:, dc * 64 : dc * 64 + 64],
                        rhs=xt_sb[:, j * 1024 : j * 1024 + 512, dc],
                        start=(dc == 0),
                        stop=(dc == DCH - 1),
                    )
                    yield
                nc.vector.tensor_copy(
                    out=qt_sb[:, j * 512 : (j + 1) * 512], in_=qp
                )
                yield

            def transpose_steps(sc):
                tp = s_ps_pool.tile([P, 4, D_HEAD], bf16, tag="s_ps", name="tp4")
                for t in range(4):
                    kc = sc * 4 + t
                    nc.tensor.transpose(
                        tp[:, t, :],
                        kvt[D_HEAD:P, kc * P : (kc + 1) * P],
                        ident[D_HEAD:P, :],
                    )
                    yield
                nc.vector.tensor_copy(
                    out=vaug[:, sc * 4 : sc * 4 + 4, :D_HEAD], in_=tp
                )
                yield

            # Q projection for slot j runs as filler inside wave j (its
            # first consumers are that wave's chunks)
            qproj_wave = {1: 1, 2: 2, 3: 3}

            # ---- prologue + wave 0: piece-width projections matched to the
            # DMA bite arrival order (x cols 0:128, 128:384, 384:512). The
            # V-transposes for kc 0..1 must be emitted before chunk (0,3)
            # pops PV(0,0), else the vaug dependency is never recorded. ----
            tg0 = transpose_steps(0)
            q_piece(0, 128)
            kv_piece(0, 128)
            q_piece(128, 256)
            kv_piece(128, 256)
            q_piece(384, 128)
            kv_piece(384, 128)
            next(tg0)
            next(tg0)
            emit_chunk(0, 0, 0)
            emit_chunk(0, 1, 0)
            next(tg0)
            next(tg0)
            emit_chunk(0, 2, 0)
            emit_chunk(0, 3, 0)
            for _ in tg0:  # drain: emits the packed vaug copy for kc 0..3
                pass

            # ---- streamed waves ----
            # Wave sc fillers: [qproj (if due), KV proj for sc, spacer,
            # V transposes for sc], popped two per chunk. Wave 0's proj and
            # transposes ran in the prologue.
            for sc in range(1, NWAVE):
                chunks = WAVES[sc]
                oo = list(out_sched[sc])
                nq = 9 if sc in qproj_wave else 0
                filler = []
                if nq:
                    filler.append(q_proj_steps(qproj_wave[sc]))
                # superchunk 7's projection and transposes run in wave 6
                # (its x arrives ~27us, far earlier) so wave 7 starts with
                # kvt/vaug complete and no filler gating
                kv_scs = (sc,)
                for s in kv_scs:
                    filler.append(kv_proj_mms(s))
                    filler.append(iter([None, None]))  # spacer: kvt drains
                    filler.append(transpose_steps(s))
                total_fill = nq + 16 * len(kv_scs)
                fill_iter = (x for g in filler for x in g)
                pops = [0]

                def pop_fill(upto=None, k=None):
                    tgt = upto if upto is not None else pops[0] + k
                    while pops[0] < min(tgt, total_fill):
                        if next(fill_iter, -1) == -1:
                            pops[0] = total_fill
                            break
                        pops[0] += 1

                # emission-order safety points: a fresh chunk's scores need
                # this wave's kvt copy emitted; a fresh chunk's PV needs its
                # V-transpose emitted
                proj_safe = nq + 9
                n = len(chunks)
                for ci, (j, kc) in enumerate(chunks):
                    if kc // 4 == sc:
                        pop_fill(upto=proj_safe)
                    if len(pending_pv) >= 4:
                        j2, kc2 = pending_pv[0]
                        if kc2 // 4 == sc:
                            pop_fill(upto=total_fill)
                    emit_chunk(j, kc, sc)
                    if oo and ci % 2 == 1:
                        emit_out_op(*oo.pop(0))
                    k = -(-(total_fill - pops[0]) // (n - ci))  # ceil
                    pop_fill(k=min(k, 4))
                pop_fill(upto=total_fill)
                for op in oo:
                    emit_out_op(*op)
                fin = [j for j in range(NSLOT) if last_wave[j] == sc and j != 3]
                if fin:
                    while pending_pv:
                        emit_pv(*pending_pv.pop(0))
                    for j in fin:
                        finalize(j)

            # ---- slot-3 tail: per-128-query-tile pipeline. The ot copy and
            # the reciprocal both read the pv PSUM bank directly (no serial
            # copy->recip dependency), then rdent -> out-proj x2 -> scaled
            # copy (DVE half, ACT half) -> per-tile y DMA, so successive
            # tiles overlap across engines. ----
            while pending_pv:
                emit_pv(*pending_pv.pop(0))
            y_tiles[3] = ypool.tile([P, 4, D_MODEL], bf16, tag="y_sb", name="y3")

            def finalize_q(j, t):
                """Per-128-query finalize: the ot copy and the reciprocal
                both read the pv PSUM bank directly."""
                c0 = j * 512 + t * P
                nc.scalar.activation(
                    ot[0:D_HEAD, c0 : c0 + P],
                    pv[j][0:D_HEAD, t * P : (t + 1) * P],
                    mybir.ActivationFunctionType.Copy,
                )
                nc.vector.reciprocal(
                    rden[:, c0 : c0 + P],
                    pv[j][D_HEAD : D_HEAD + 1, t * P : (t + 1) * P],
                )
                nc.tensor.matmul(
                    pv[j][:, t : t + 1],
                    lhsT=rden[:, c0 : c0 + P],
                    rhs=one_sb,
                    start=True,
                    stop=True,
                )
                nc.vector.tensor_copy(
                    out=rdent[:, 4 * j + t : 4 * j + t + 1],
                    in_=pv[j][:, t : t + 1],
                )

            # emission order keeps each engine FIFO unblocked: the recip
            # chains for tiles 0-2 all precede the first scale op. Each
            # out-op gets its own PSUM bank (score + mm banks are idle by
            # now) so no matmul waits on a previous scale's drain.
            tail_banks = [
                s_ps_pool.tile([P, 512], fp32, tag="s_ps", name=f"tb{k}")
                for k in range(3)
            ] + [mm_ps.tile([P, 512], fp32, tag="mm", name="tb3")]
            finalize_q(3, 0)
            finalize_q(3, 1)
            finalize_q(3, 2)
            finalize_q(3, 3)
            emit_out_op(3, 0, bank=tail_banks[0])
            emit_out_op(3, 1, bank=tail_banks[1])
            emit_out_op(3, 2, bank=tail_banks[2])
            emit_out_op(3, 3, bank=tail_banks[3])
            for i in range(4, 8):
                emit_out_op(3, i, bank=pv[i - 4])

    nc.finalize()
    return nc


def _get_program():
    global _prog
    if _prog is None:
        _prog = _build_program()
    return _prog


def kernel(x, W_q, W_k, W_v, W_o):
    import ml_dtypes
    from concourse.bass_utils import run_bass_kernel_spmd

    bf = ml_dtypes.bfloat16
    nc = _get_program()

    x = np.asarray(x, dtype=np.float32)
    scale = np.float32(1.0 / np.sqrt(D_HEAD))
    wq_s = np.asarray(W_q, dtype=np.float32) * scale
    wkv = np.concatenate(
        [np.asarray(W_k, dtype=np.float32), np.asarray(W_v, dtype=np.float32)],
        axis=1,
    )  # [1024, 128]
    wq_part = wq_s.reshape(DCH, P, 64).transpose(1, 0, 2).reshape(P, 512)
    wkv_part = wkv.reshape(DCH, P, 128).transpose(1, 0, 2).reshape(P, 1024)
    # per-role exp bias for partner-band chunks: -30 kills the whole chunk
    # for role 0 (exp(s - 30) ~ 1e-10), 0 keeps it whole for role 1
    w_hosts = []
    for r in range(2):
        rb = np.full((P, 4), 0.0 if r == 1 else -30.0, dtype=np.float32)
        w_hosts.append(
            np.ascontiguousarray(
                np.concatenate([wq_part, wkv_part, rb], axis=1)
            ).astype(bf)
        )  # [128, 1540]
    wo_host = np.ascontiguousarray(np.asarray(W_o, dtype=np.float32)).astype(bf)

    in_maps = []
    for c in range(NCORES):
        b, r = c // 2, c % 2
        xt_b = x[b].T  # [1024, 4096]
        # permuted key order: position 2k holds this core's query block
        # 2k+r, position 2k+1 holds the partner block 2k+(1-r)
        cols = np.concatenate(
            [
                np.arange(512 * g, 512 * g + 512)
                for k in range(NSLOT)
                for g in (2 * k + r, 2 * k + 1 - r)
            ]
        )
        xt_host = np.ascontiguousarray(
            xt_b[:, cols].reshape(DCH, P, SEQ).transpose(1, 2, 0)
        ).astype(bf)  # [128, 4096, 8]
        in_maps.append(
            {
                "xt": xt_host,
                "w": w_hosts[r],
                "wo": wo_host,
            }
        )

    res = run_bass_kernel_spmd(nc, in_maps, core_ids=list(range(NCORES)))
    out = np.empty((BATCH, SEQ, D_MODEL), dtype=np.float32)
    for c in range(NCORES):
        b, r = c // 2, c % 2
        yv = np.asarray(res.results[c]["y"]).astype(np.float32)
        # y[j, p, t, :] -> query 512*(2j+r) + 128t + p
        yv = yv.transpose(0, 2, 1, 3)  # [j, t, p, m]
        for j in range(NSLOT):
            q0 = 512 * (2 * j + r)
            out[b, q0 : q0 + 512, :] = yv[j].reshape(512, D_MODEL)
    return out



# revision 67
# speedup vs baseline: 1.0034x; 1.0034x over previous
"""Trainium2 Bass kernel for single-head causal attention.

x:[4,4096,1024] f32, W_q/W_k/W_v:[1024,64], W_o:[64,1024].

Sharding: 8 cores = 4 batches x 2 query-stripe roles. Role r of a batch
owns query blocks {2j+r : j=0..3} (512 queries each). Program slot j has
key extent E[j] = (8j+8) 128-key chunks, which exactly covers role 1's
block 2j+1 and over-covers role 0's block 2j by 4 chunks (dead).

All per-core differences (which batch, which stripe, dead chunks) are
carried in the input data; one SPMD program runs on all 8 cores:
- x is shipped as [128, 4096, 8] (d_model-chunk partition, seq, chunk)
  so any column range is a fully contiguous DMA (no sub-512B-element
  descriptor penalty), streamed in arrival-ordered bites.
- The diagonal causal band masks (4 x [128,512]) are generated on the
  otherwise-idle Pool engine with affine_select, not DMAed.
- Partner-block chunks (band >= 4) take no mask op at all: the exp runs
  with a per-partition bias from the w tensor (-30 for role 0 => p ~
  1e-10, 0 for role 1), so role-0's dead chunks vanish from both the PV
  numerator and the denominator row.

bf16 everywhere on the matmul paths (end-to-end rel err ~5e-3 vs the
2e-2 gate). Engines: PE does all matmuls (62.3us busy, the roofline of
this schedule), ACT does exp + the slot-3 ot copies + half its output
scales, DVE does diag masks + kvt/qt copies + finalize + the other
scales. Attention chunks are emitted in waves matched to DMA arrival,
next wave's KV projection interleaved between chunks as PE filler, PV
matmuls lagged 10 chunks behind their scores. The slot-3 tail runs a
per-128-query finalize pipeline (ot copy and 1/den read the PV PSUM
bank directly) with per-tile y DMAs so the final HWDGE descriptor-gen
chain (625ns each, serialized) stays short.
"""

import sys

for _p in ("/opt/trn_rl_repo",):
    if _p not in sys.path:
        sys.path.insert(0, _p)

import numpy as np

D_MODEL = 1024
D_HEAD = 64
SEQ = 4096
BATCH = 4
NCORES = 8
NQ = 2048          # queries per core
P = 128
DCH = D_MODEL // P  # 8 contraction chunks
NSLOT = 4           # query slots of 512
E = [8, 16, 24, 32]  # key chunks per slot
NWAVE = 8           # key superchunks of 512

# Attention chunks per wave: matched to DMA arrival order (early waves
# light), per-slot ascending kc, every chunk (j,kc) in wave >= kc//4.
WAVES = [
    [(0, k) for k in range(4)],
    [(0, k) for k in range(4, 8)] + [(1, k) for k in range(4)],
    [(1, k) for k in range(4, 8)] + [(2, k) for k in range(4)],
    [(1, k) for k in range(8, 12)] + [(2, k) for k in range(4, 8)]
    + [(3, k) for k in range(4)],
    [(1, k) for k in range(12, 16)] + [(2, k) for k in range(8, 12)]
    + [(3, k) for k in range(4, 8)],
    [(2, k) for k in range(12, 24)],
    [(3, k) for k in range(8, 22)],
    [(3, k) for k in range(22, 32)],
]

_prog = None


def _check_waves():
    seen = {}
    total = 0
    for w, wv in enumerate(WAVES):
        for j, kc in wv:
            assert kc // 4 <= w, (w, j, kc)
            assert seen.get(j, -1) == kc - 1, (j, kc)
            seen[j] = kc
            total += 1
    assert total == sum(E) == 80
    return {j: max(w for w, wv in enumerate(WAVES) if (j, E[j] - 1) in wv)
            for j in range(NSLOT)}


def _build_program():
    import concourse.bacc as bacc
    import concourse.mybir as mybir
    import concourse.tile as tile
    from concourse.masks import make_identity

    fp32 = mybir.dt.float32
    f32r = mybir.dt.float32r
    bf16 = mybir.dt.bfloat16
    nc = bacc.Bacc("TRN2", target_bir_lowering=False, debug=False)

    # w layout: [wq 0:512 | wkv 512:1536 | rbias 1536:1540]
    # rbias: -30 for role 0, 0 for role 1. Partner-band chunks (band >= 4)
    # run exp with this per-partition bias instead of a 0/1 mask multiply:
    # each slot's band region only ever covers its own partner block, which
    # is entirely dead for role 0 (exp(s - 30) ~ 1e-10) and entirely alive
    # for role 1 (bias 0). Replaces the 16 explicit partner-mask DVE ops.
    xt = nc.dram_tensor("xt", [P, SEQ, DCH], bf16, kind="ExternalInput")
    w = nc.dram_tensor("w", [P, DCH * 192 + 4], bf16, kind="ExternalInput")
    wo = nc.dram_tensor("wo", [D_HEAD, D_MODEL], bf16, kind="ExternalInput")
    y = nc.dram_tensor("y", [NSLOT, P, 4, D_MODEL], bf16, kind="ExternalOutput")

    last_wave = _check_waves()
    # output-projection ops (j, i): slot0 -> waves 2,3; slot1 -> 5,6;
    # slot2 -> wave 6 (where DVE is light: slot-3 chunks kc<24 need no
    # masks); slot3 -> tail (wave index NWAVE)
    out_sched = {wi: [] for wi in range(NWAVE + 1)}
    for j, tgt in ((0, (2, 3)), (1, (5, 6)), (2, (6, 6)), (3, (8, 8))):
        for i in range(8):
            out_sched[tgt[i // 4]].append((j, i))

    with tile.TileContext(nc) as tc:
        with (
            tc.tile_pool(name="singles", bufs=1) as singles,
            tc.tile_pool(name="work", bufs=10) as work,
            tc.tile_pool(name="ypool", bufs=2) as ypool,
            tc.tile_pool(name="mm_ps", bufs=1, space="PSUM") as mm_ps,
            tc.tile_pool(name="s_ps", bufs=3, space="PSUM") as s_ps_pool,
            tc.tile_pool(name="pv_ps", bufs=1, space="PSUM") as pv_pool,
        ):
            # ---- persistent SBUF ----
            w_sb = singles.tile([P, DCH * 192 + 4], bf16, tag="w_sb")
            xt_sb = singles.tile([P, SEQ, DCH], bf16, tag="xt_sb")
            msk_sb = singles.tile([P, 4, 512], bf16, tag="msk_sb")
            wo_sb = singles.tile([D_HEAD, D_MODEL], bf16, tag="wo_sb")
            kvt = singles.tile([P, SEQ], bf16, tag="kvt")  # 0:64 K^T, 64:128 V^T
            qt_sb = singles.tile([D_HEAD, NQ], bf16, tag="qt_sb")
            vaug = singles.tile([P, 32, D_HEAD + 1], bf16, tag="vaug")
            ot = singles.tile([D_HEAD + 1, NQ], bf16, tag="ot")
            rden = singles.tile([1, NQ], fp32, tag="rden")
            rbias = singles.tile([P, 1], fp32, tag="rbias")
            rdent = singles.tile([P, 16], fp32, tag="rdent")
            ident = singles.tile([P, D_HEAD], bf16, tag="ident")
            one_sb = singles.tile([1, 1], fp32, tag="one_sb")

            # ---- input DMAs (SP queue, ordered by first use) ----
            def ld_x(dst, src, s0):
                nc.sync.dma_start(
                    out=dst[:, s0 : s0 + 512, :], in_=src[:, s0 : s0 + 512, :]
                )

            def ld_xh(dst, src, s0, n=512):
                nc.sync.dma_start(
                    out=dst[:, s0 : s0 + n, :], in_=src[:, s0 : s0 + n, :]
                )

            nc.sync.dma_start(out=w_sb[:, 0:512], in_=w[:, 0:512])
            ld_xh(xt_sb, xt, 0, 128)
            nc.sync.dma_start(out=w_sb[:, 512:1536], in_=w[:, 512:1536])
            ld_xh(xt_sb, xt, 128, 256)
            ld_xh(xt_sb, xt, 384, 128)
            nc.sync.dma_start(out=w_sb[:, 1536:1540], in_=w[:, 1536:1540])
            ld_x(xt_sb, xt, 512)       # chunk 1: wave-1 fresh keys
            ld_x(xt_sb, xt, 1024)      # chunk 2: slot-1 queries
            ld_x(xt_sb, xt, 2048)      # chunk 4: slot-2 queries
            nc.sync.dma_start(out=wo_sb, in_=wo[:, :])
            ld_x(xt_sb, xt, 1536)      # chunk 3
            ld_x(xt_sb, xt, 3072)      # chunk 6: slot-3 queries
            ld_x(xt_sb, xt, 2560)      # chunk 5
            ld_x(xt_sb, xt, 3584)      # chunk 7

            nc.vector.memset(one_sb, 1.0)
            nc.vector.memset(vaug[:, :, D_HEAD : D_HEAD + 1], 1.0)
            make_identity(nc, ident[D_HEAD:P, :])
            # fp32 per-partition exp bias (see w layout note)
            nc.vector.tensor_copy(out=rbias, in_=w_sb[:, 1536:1537])
            # causal band masks generated on the idle Pool engine:
            # msk_sb[p, c, jq] = 1 if 128c + p <= jq else 0, per band c
            for c in range(4):
                nc.gpsimd.memset(msk_sb[:, c, :], 1.0)
                nc.gpsimd.affine_select(
                    out=msk_sb[:, c, :],
                    in_=msk_sb[:, c, :],
                    compare_op=mybir.AluOpType.is_ge,
                    fill=0.0,
                    base=-128 * c,
                    channel_multiplier=-1,
                    pattern=[[1, 512]],
                )

            def kv_proj_mms(sc):
                """Generator: one KV-projection matmul per next() call."""
                kp = mm_ps.tile([P, 512], fp32, tag="mm")
                for dc in range(DCH):
                    nc.tensor.matmul(
                        kp,
                        lhsT=w_sb[:, 512 + dc * 128 : 512 + dc * 128 + 128],
                        rhs=xt_sb[:, sc * 512 : (sc + 1) * 512, dc],
                        start=(dc == 0),
                        stop=(dc == DCH - 1),
                    )
                    yield
                nc.vector.tensor_copy(
                    out=kvt[:, sc * 512 : (sc + 1) * 512], in_=kp
                )
                yield

            def transposes(sc):
                for t in range(4):  # V^T 128-col blocks -> natural V chunks
                    kc = sc * 4 + t
                    tp = s_ps_pool.tile([P, D_HEAD], bf16, tag="s_ps")
                    nc.tensor.transpose(
                        tp,
                        kvt[D_HEAD:P, kc * P : (kc + 1) * P],
                        ident[D_HEAD:P, :],
                    )
                    nc.vector.tensor_copy(out=vaug[:, kc, :D_HEAD], in_=tp)

            # PV accumulators: full-bank tiles, PV uses rows 0:65
            pv = [
                pv_pool.tile([P, 512], fp32, tag=f"pv{g}", name=f"pv{g}")
                for g in range(NSLOT)
            ]
            freed = []  # pv banks released by finalized slots
            y_tiles = {}
            ncopy = [0]
            pending_pv = []

            def emit_pv(j, kc):
                nc.tensor.matmul(
                    pv[j][0 : D_HEAD + 1, :],
                    lhsT=vaug[:, kc, :],
                    rhs=pending_pv_pt.pop((j, kc)),
                    start=(kc == 0),
                    stop=(kc == E[j] - 1),
                    skip_group_check=True,
                )

            pending_pv_pt = {}

            def emit_chunk(j, kc, wv=0):
                sps = s_ps_pool.tile([P, 512], fp32, tag="s_ps")
                nc.tensor.matmul(
                    sps,
                    lhsT=kvt[0:D_HEAD, kc * P : (kc + 1) * P],
                    rhs=qt_sb[:, j * 512 : (j + 1) * 512],
                    start=True,
                    stop=True,
                )
                p_t = work.tile([P, 512], bf16, tag="p_t")
                band = kc - (E[j] - 8)
                if band >= 4:
                    # partner block: role-0 kills the whole chunk via the
                    # exp bias (exp(s - 30) ~ 1e-10); role 1 keeps it whole
                    nc.scalar.activation(
                        p_t, sps, mybir.ActivationFunctionType.Exp,
                        bias=rbias[:, 0:1],
                    )
                else:
                    nc.scalar.activation(
                        p_t, sps, mybir.ActivationFunctionType.Exp
                    )
                if 0 <= band < 4:
                    # diagonal band: per-query causal step mask
                    nc.vector.tensor_tensor(
                        p_t, p_t, msk_sb[:, band, :], mybir.AluOpType.mult
                    )
                pending_pv_pt[(j, kc)] = p_t
                pending_pv.append((j, kc))
                if len(pending_pv) > 10:
                    emit_pv(*pending_pv.pop(0))

            def emit_out_op(j, i, bank=None):
                """One output-projection matmul + scaled PSUM->SBUF copy."""
                t, no = i // 2, i % 2
                if bank is None:
                    bank = freed[ncopy[0] % len(freed)]
                    ncopy[0] += 1
                q0 = j * 512 + t * P
                nc.tensor.matmul(
                    bank,
                    lhsT=ot[0:D_HEAD, q0 : q0 + P],
                    rhs=wo_sb[:, no * 512 : (no + 1) * 512],
                    start=True,
                    stop=True,
                )
                if j == 3 and i % 2 == 0:
                    nc.scalar.mul(
                        y_tiles[j][:, t, no * 512 : (no + 1) * 512],
                        bank,
                        rdent[:, 4 * j + t : 4 * j + t + 1],
                    )
                else:
                    nc.vector.tensor_scalar_mul(
                        y_tiles[j][:, t, no * 512 : (no + 1) * 512],
                        bank,
                        rdent[:, 4 * j + t : 4 * j + t + 1],
                    )
                if j == 3:
                    if i % 2 == 1:
                        # per-tile DMA: few enough that HWDGE desc-gen
                        # (625ns each, serialized) stays off the tail path
                        nc.sync.dma_start(
                            out=y[j][:, t : t + 1, :],
                            in_=y_tiles[j][:, t : t + 1, :],
                        )
                elif i == 3:
                    nc.sync.dma_start(
                        out=y[j][:, 0:2, :], in_=y_tiles[j][:, 0:2, :]
                    )
                elif i == 7:
                    nc.sync.dma_start(
                        out=y[j][:, 2:4, :], in_=y_tiles[j][:, 2:4, :]
                    )

            def finalize_half(j, h):
                """Half of slot-j finalize: O^T + den, 1/den, rdent cols."""
                c0 = j * 512 + h * 256
                c1 = c0 + 256
                nc.vector.tensor_copy(
                    out=ot[:, c0:c1], in_=pv[j][0 : D_HEAD + 1, h * 256 : h * 256 + 256]
                )
                nc.vector.reciprocal(
                    rden[:, c0:c1], ot[D_HEAD : D_HEAD + 1, c0:c1]
                )
                for t in (2 * h, 2 * h + 1):
                    nc.tensor.matmul(
                        pv[j][:, t : t + 1],
                        lhsT=rden[:, j * 512 + t * P : j * 512 + (t + 1) * P],
                        rhs=one_sb,
                        start=True,
                        stop=True,
                    )
                nc.vector.tensor_copy(
                    out=rdent[:, 4 * j + 2 * h : 4 * j + 2 * h + 2],
                    in_=pv[j][:, 2 * h : 2 * h + 2],
                )

            def finalize(j):
                finalize_half(j, 0)
                finalize_half(j, 1)
                freed.append(pv[j])
                y_tiles[j] = ypool.tile(
                    [P, 4, D_MODEL], bf16, tag="y_sb", name=f"y{j}"
                )

            def q_piece(c0, n):
                """Prologue Q projection over columns [c0, c0+n) of slot 0.
                Uses the s_ps pool so pieces rotate PSUM banks instead of
                serializing on the single mm bank."""
                qp = s_ps_pool.tile([D_HEAD, n], fp32, tag="s_ps", name=f"qp{c0}")
                for dc in range(DCH):
                    nc.tensor.matmul(
                        qp,
                        lhsT=w_sb[:, dc * 64 : dc * 64 + 64],
                        rhs=xt_sb[:, c0 : c0 + n, dc],
                        start=(dc == 0),
                        stop=(dc == DCH - 1),
                    )
                nc.vector.tensor_copy(out=qt_sb[:, c0 : c0 + n], in_=qp)

            def kv_piece(c0, n):
                kp = s_ps_pool.tile([P, n], fp32, tag="s_ps", name=f"kp{c0}")
                for dc in range(DCH):
                    nc.tensor.matmul(
                        kp,
                        lhsT=w_sb[:, 512 + dc * 128 : 512 + dc * 128 + 128],
                        rhs=xt_sb[:, c0 : c0 + n, dc],
                        start=(dc == 0),
                        stop=(dc == DCH - 1),
                    )
                nc.vector.tensor_copy(out=kvt[:, c0 : c0 + n], in_=kp)

            def q_proj_steps(j):
                """Generator version of q_proj: one matmul per next()."""
                qp = mm_ps.tile([D_HEAD, 512], fp32, tag="mm")
                for dc in range(DCH):
                    nc.tensor.matmul(
                        qp,
                        lhsT=w_sb[:, dc * 64 : dc * 64 + 64],
                        rhs=xt_sb[:, j * 1024 : j * 1024 + 512, dc],
                        start=(dc == 0),
                        stop=(dc == DCH - 1),
                    )
                    yield
                nc.vector.tensor_copy(
                    out=qt_sb[:, j * 512 : (j + 1) * 512], in_=qp
                )
                yield

            def transpose_steps(sc):
                tp = s_ps_pool.tile([P, 4, D_HEAD], bf16, tag="s_ps", name="tp4")
                for t in range(4):
                    kc = sc * 4 + t
                    nc.tensor.transpose(
                        tp[:, t, :],
                        kvt[D_HEAD:P, kc * P : (kc + 1) * P],
                        ident[D_HEAD:P, :],
                    )
                    yield
                nc.vector.tensor_copy(
                    out=vaug[:, sc * 4 : sc * 4 + 4, :D_HEAD], in_=tp
                )
                yield

            # Q projection for slot j runs as filler inside wave j (its
            # first consumers are that wave's chunks)
            qproj_wave = {1: 1, 2: 2, 3: 3}

            # ---- prologue + wave 0: piece-width projections matched to the
            # DMA bite arrival order (x cols 0:128, 128:384, 384:512). The
            # V-transposes for kc 0..1 must be emitted before chunk (0,3)
            # pops PV(0,0), else the vaug dependency is never recorded. ----
            tg0 = transpose_steps(0)
            q_piece(0, 128)
            kv_piece(0, 128)
            q_piece(128, 256)
            kv_piece(128, 256)
            q_piece(384, 128)
            kv_piece(384, 128)
            next(tg0)
            next(tg0)
            emit_chunk(0, 0, 0)
            emit_chunk(0, 1, 0)
            next(tg0)
            next(tg0)
            emit_chunk(0, 2, 0)
            emit_chunk(0, 3, 0)
            for _ in tg0:  # drain: emits the packed vaug copy for kc 0..3
                pass

            # ---- streamed waves ----
            # Wave sc fillers: [qproj (if due), KV proj for sc, spacer,
            # V transposes for sc], popped two per chunk. Wave 0's proj and
            # transposes ran in the prologue.
            for sc in range(1, NWAVE):
                chunks = WAVES[sc]
                oo = list(out_sched[sc])
                nq = 9 if sc in qproj_wave else 0
                filler = []
                if nq:
                    filler.append(q_proj_steps(qproj_wave[sc]))
                # superchunk 7's projection and transposes run in wave 6
                # (its x arrives ~27us, far earlier) so wave 7 starts with
                # kvt/vaug complete and no filler gating
                kv_scs = (sc,)
                for s in kv_scs:
                    filler.append(kv_proj_mms(s))
                    filler.append(iter([None, None]))  # spacer: kvt drains
                    filler.append(transpose_steps(s))
                total_fill = nq + 16 * len(kv_scs)
                fill_iter = (x for g in filler for x in g)
                pops = [0]

                def pop_fill(upto=None, k=None):
                    tgt = upto if upto is not None else pops[0] + k
                    while pops[0] < min(tgt, total_fill):
                        if next(fill_iter, -1) == -1:
                            pops[0] = total_fill
                            break
                        pops[0] += 1

                # emission-order safety points: a fresh chunk's scores need
                # this wave's kvt copy emitted; a fresh chunk's PV needs its
                # V-transpose emitted
                proj_safe = nq + 9
                n = len(chunks)
                for ci, (j, kc) in enumerate(chunks):
                    if kc // 4 == sc:
                        pop_fill(upto=proj_safe)
                    if len(pending_pv) >= 4:
                        j2, kc2 = pending_pv[0]
                        if kc2 // 4 == sc:
                            pop_fill(upto=total_fill)
                    emit_chunk(j, kc, sc)
                    if oo and ci % 2 == 1:
                        emit_out_op(*oo.pop(0))
                    k = -(-(total_fill - pops[0]) // (n - ci))  # ceil
                    pop_fill(k=min(k, 4))
                pop_fill(upto=total_fill)
                for op in oo:
                    emit_out_op(*op)
                fin = [j for j in range(NSLOT) if last_wave[j] == sc and j != 3]
                if fin:
                    while pending_pv:
                        emit_pv(*pending_pv.pop(0))
                    for j in fin:
                        finalize(j)

            # ---- slot-3 tail: per-128-query-tile pipeline. The ot copy and
            # the reciprocal both read the pv PSUM bank directly (no serial
            # copy->recip dependency), then rdent -> out-proj x2 -> scaled
            # copy (DVE half, ACT half) -> per-tile y DMA, so successive
            # tiles overlap across engines. ----
            while pending_pv:
                emit_pv(*pending_pv.pop(0))
            y_tiles[3] = ypool.tile([P, 4, D_MODEL], bf16, tag="y_sb", name="y3")

            def finalize_q(j, t):
                """Per-128-query finalize: reciprocal reads the pv PSUM
                bank directly; the ot copy is one full-width ACT op emitted
                by the caller."""
                c0 = j * 512 + t * P
                nc.vector.reciprocal(
                    rden[:, c0 : c0 + P],
                    pv[j][D_HEAD : D_HEAD + 1, t * P : (t + 1) * P],
                )
                nc.tensor.matmul(
                    pv[j][:, t : t + 1],
                    lhsT=rden[:, c0 : c0 + P],
                    rhs=one_sb,
                    start=True,
                    stop=True,
                )
                nc.vector.tensor_copy(
                    out=rdent[:, 4 * j + t : 4 * j + t + 1],
                    in_=pv[j][:, t : t + 1],
                )

            # emission order keeps each engine FIFO unblocked: the recip
            # chains for tiles 0-2 all precede the first scale op. Each
            # out-op gets its own PSUM bank (score + mm banks are idle by
            # now) so no matmul waits on a previous scale's drain.
            tail_banks = [
                s_ps_pool.tile([P, 512], fp32, tag="s_ps", name=f"tb{k}")
                for k in range(3)
            ] + [mm_ps.tile([P, 512], fp32, tag="mm", name="tb3")]
            nc.scalar.activation(
                ot[0:D_HEAD, 1536:2048],
                pv[3][0:D_HEAD, :],
                mybir.ActivationFunctionType.Copy,
            )
            nc.vector.reciprocal(
                rden[:, 1536:2048], pv[3][D_HEAD : D_HEAD + 1, :]
            )
            for t in range(4):
                nc.tensor.matmul(
                    pv[3][:, t : t + 1],
                    lhsT=rden[:, 1536 + t * P : 1536 + (t + 1) * P],
                    rhs=one_sb,
                    start=True,
                    stop=True,
                )
            nc.vector.tensor_copy(out=rdent[:, 12:16], in_=pv[3][:, 0:4])
            emit_out_op(3, 0, bank=tail_banks[0])
            emit_out_op(3, 1, bank=tail_banks[1])
            emit_out_op(3, 2, bank=tail_banks[2])
            emit_out_op(3, 3, bank=tail_banks[3])
            emit_out_op(3, 4, bank=pv[0])
            emit_out_op(3, 5, bank=pv[1])
            emit_out_op(3, 6, bank=pv[2])
            # pv[3] frees as soon as the consolidated rdent copy has read
            # cols 0:4 (~right after the last PV), earlier than any scale
            emit_out_op(3, 7, bank=pv[3])

    nc.finalize()
    return nc


def _get_program():
    global _prog
    if _prog is None:
        _prog = _build_program()
    return _prog


def kernel(x, W_q, W_k, W_v, W_o):
    import ml_dtypes
    from concourse.bass_utils import run_bass_kernel_spmd

    bf = ml_dtypes.bfloat16
    nc = _get_program()

    x = np.asarray(x, dtype=np.float32)
    scale = np.float32(1.0 / np.sqrt(D_HEAD))
    wq_s = np.asarray(W_q, dtype=np.float32) * scale
    wkv = np.concatenate(
        [np.asarray(W_k, dtype=np.float32), np.asarray(W_v, dtype=np.float32)],
        axis=1,
    )  # [1024, 128]
    wq_part = wq_s.reshape(DCH, P, 64).transpose(1, 0, 2).reshape(P, 512)
    wkv_part = wkv.reshape(DCH, P, 128).transpose(1, 0, 2).reshape(P, 1024)
    # per-role exp bias for partner-band chunks: -30 kills the whole chunk
    # for role 0 (exp(s - 30) ~ 1e-10), 0 keeps it whole for role 1
    w_hosts = []
    for r in range(2):
        rb = np.full((P, 4), 0.0 if r == 1 else -30.0, dtype=np.float32)
        w_hosts.append(
            np.ascontiguousarray(
                np.concatenate([wq_part, wkv_part, rb], axis=1)
            ).astype(bf)
        )  # [128, 1540]
    wo_host = np.ascontiguousarray(np.asarray(W_o, dtype=np.float32)).astype(bf)

    in_maps = []
    for c in range(NCORES):
        b, r = c // 2, c % 2
        xt_b = x[b].T  # [1024, 4096]
        # permuted key order: position 2k holds this core's query block
        # 2k+r, position 2k+1 holds the partner block 2k+(1-r)
        cols = np.concatenate(
            [
                np.arange(512 * g, 512 * g + 512)
                for k in range(NSLOT)
                for g in (2 * k + r, 2 * k + 1 - r)
            ]
        )
        xt_host = np.ascontiguousarray(
            xt_b[:, cols].reshape(DCH, P, SEQ).transpose(1, 2, 0)
        ).astype(bf)  # [128, 4096, 8]
        in_maps.append(
            {
                "xt": xt_host,
                "w": w_hosts[r],
                "wo": wo_host,
            }
        )

    res = run_bass_kernel_spmd(nc, in_maps, core_ids=list(range(NCORES)))
    out = np.empty((BATCH, SEQ, D_MODEL), dtype=np.float32)
    for c in range(NCORES):
        b, r = c // 2, c % 2
        yv = np.asarray(res.results[c]["y"]).astype(np.float32)
        # y[j, p, t, :] -> query 512*(2j+r) + 128t + p
        yv = yv.transpose(0, 2, 1, 3)  # [j, t, p, m]
        for j in range(NSLOT):
            q0 = 512 * (2 * j + r)
            out[b, q0 : q0 + 512, :] = yv[j].reshape(512, D_MODEL)
    return out



# revision 70
# speedup vs baseline: 1.0138x; 1.0105x over previous
"""Trainium2 Bass kernel for single-head causal attention.

x:[4,4096,1024] f32, W_q/W_k/W_v:[1024,64], W_o:[64,1024].

Sharding: 8 cores = 4 batches x 2 query-stripe roles. Role r of a batch
owns query blocks {2j+r : j=0..3} (512 queries each). Program slot j has
key extent E[j] = (8j+8) 128-key chunks, which exactly covers role 1's
block 2j+1 and over-covers role 0's block 2j by 4 chunks (dead).

All per-core differences (which batch, which stripe, dead chunks) are
carried in the input data; one SPMD program runs on all 8 cores:
- x is shipped as [128, 4096, 8] (d_model-chunk partition, seq, chunk)
  so any column range is a fully contiguous DMA (no sub-512B-element
  descriptor penalty), streamed in arrival-ordered bites.
- The diagonal causal band masks (4 x [128,512]) are generated on the
  otherwise-idle Pool engine with affine_select, not DMAed.
- Partner-block chunks (band >= 4) take no mask op at all: the exp runs
  with a per-partition bias from the w tensor (-30 for role 0 => p ~
  1e-10, 0 for role 1), so role-0's dead chunks vanish from both the PV
  numerator and the denominator row.

bf16 everywhere on the matmul paths (end-to-end rel err ~5e-3 vs the
2e-2 gate). Engines: PE does all matmuls (62.3us busy, the roofline of
this schedule), ACT does exp + the slot-3 ot copies + half its output
scales, DVE does diag masks + kvt/qt copies + finalize + the other
scales. Attention chunks are emitted in waves matched to DMA arrival,
next wave's KV projection interleaved between chunks as PE filler, PV
matmuls lagged 10 chunks behind their scores. The slot-3 tail finalize
is consolidated into full-width ops (one [64,512] ot copy on ACT, one
[1,512] reciprocal, one [P,4] rdent copy, all reading the PV PSUM bank
directly) so all eight out-projection matmuls unblock at once; each
out-op has its own PSUM bank and the y DMAs go out per 128-query tile
to keep the final HWDGE descriptor-gen chain (625ns each, serialized)
short.
"""

import sys

for _p in ("/opt/trn_rl_repo",):
    if _p not in sys.path:
        sys.path.insert(0, _p)

import numpy as np

D_MODEL = 1024
D_HEAD = 64
SEQ = 4096
BATCH = 4
NCORES = 8
NQ = 2048          # queries per core
P = 128
DCH = D_MODEL // P  # 8 contraction chunks
NSLOT = 4           # query slots of 512
E = [8, 16, 24, 32]  # key chunks per slot
NWAVE = 8           # key superchunks of 512

# Attention chunks per wave: matched to DMA arrival order (early waves
# light), per-slot ascending kc, every chunk (j,kc) in wave >= kc//4.
WAVES = [
    [(0, k) for k in range(4)],
    [(0, k) for k in range(4, 8)] + [(1, k) for k in range(4)],
    [(1, k) for k in range(4, 8)] + [(2, k) for k in range(4)],
    [(1, k) for k in range(8, 12)] + [(2, k) for k in range(4, 8)]
    + [(3, k) for k in range(4)],
    [(1, k) for k in range(12, 16)] + [(2, k) for k in range(8, 12)]
    + [(3, k) for k in range(4, 8)],
    [(2, k) for k in range(12, 24)],
    [(3, k) for k in range(8, 22)],
    [(3, k) for k in range(22, 32)],
]

_prog = None


def _check_waves():
    seen = {}
    total = 0
    for w, wv in enumerate(WAVES):
        for j, kc in wv:
            assert kc // 4 <= w, (w, j, kc)
            assert seen.get(j, -1) == kc - 1, (j, kc)
            seen[j] = kc
            total += 1
    assert total == sum(E) == 80
    return {j: max(w for w, wv in enumerate(WAVES) if (j, E[j] - 1) in wv)
            for j in range(NSLOT)}


def _build_program():
    import concourse.bacc as bacc
    import concourse.mybir as mybir
    import concourse.tile as tile
    from concourse.masks import make_identity

    fp32 = mybir.dt.float32
    f32r = mybir.dt.float32r
    bf16 = mybir.dt.bfloat16
    nc = bacc.Bacc("TRN2", target_bir_lowering=False, debug=False)

    # w layout: [wq 0:512 | wkv 512:1536 | rbias 1536:1540]
    # rbias: -30 for role 0, 0 for role 1. Partner-band chunks (band >= 4)
    # run exp with this per-partition bias instead of a 0/1 mask multiply:
    # each slot's band region only ever covers its own partner block, which
    # is entirely dead for role 0 (exp(s - 30) ~ 1e-10) and entirely alive
    # for role 1 (bias 0). Replaces the 16 explicit partner-mask DVE ops.
    xt = nc.dram_tensor("xt", [P, SEQ, DCH], bf16, kind="ExternalInput")
    w = nc.dram_tensor("w", [P, DCH * 192 + 4], bf16, kind="ExternalInput")
    wo = nc.dram_tensor("wo", [D_HEAD, D_MODEL], bf16, kind="ExternalInput")
    y = nc.dram_tensor("y", [NSLOT, P, 4, D_MODEL], bf16, kind="ExternalOutput")

    last_wave = _check_waves()
    # output-projection ops (j, i): slot0 -> waves 2,3; slot1 -> 5,6;
    # slot2 -> wave 6 (where DVE is light: slot-3 chunks kc<24 need no
    # masks); slot3 -> tail (wave index NWAVE)
    out_sched = {wi: [] for wi in range(NWAVE + 1)}
    for j, tgt in ((0, (2, 3)), (1, (5, 6)), (2, (6, 6)), (3, (8, 8))):
        for i in range(8):
            out_sched[tgt[i // 4]].append((j, i))

    with tile.TileContext(nc) as tc:
        with (
            tc.tile_pool(name="singles", bufs=1) as singles,
            tc.tile_pool(name="work", bufs=10) as work,
            tc.tile_pool(name="ypool", bufs=2) as ypool,
            tc.tile_pool(name="mm_ps", bufs=1, space="PSUM") as mm_ps,
            tc.tile_pool(name="s_ps", bufs=3, space="PSUM") as s_ps_pool,
            tc.tile_pool(name="pv_ps", bufs=1, space="PSUM") as pv_pool,
        ):
            # ---- persistent SBUF ----
            w_sb = singles.tile([P, DCH * 192 + 4], bf16, tag="w_sb")
            xt_sb = singles.tile([P, SEQ, DCH], bf16, tag="xt_sb")
            msk_sb = singles.tile([P, 4, 512], bf16, tag="msk_sb")
            wo_sb = singles.tile([D_HEAD, D_MODEL], bf16, tag="wo_sb")
            kvt = singles.tile([P, SEQ], bf16, tag="kvt")  # 0:64 K^T, 64:128 V^T
            qt_sb = singles.tile([D_HEAD, NQ], bf16, tag="qt_sb")
            vaug = singles.tile([P, 32, D_HEAD + 1], bf16, tag="vaug")
            ot = singles.tile([D_HEAD + 1, NQ], bf16, tag="ot")
            rden = singles.tile([1, NQ], fp32, tag="rden")
            rbias = singles.tile([P, 1], fp32, tag="rbias")
            rdent = singles.tile([P, 16], fp32, tag="rdent")
            ident = singles.tile([P, D_HEAD], bf16, tag="ident")
            one_sb = singles.tile([1, 1], fp32, tag="one_sb")

            # ---- input DMAs (SP queue, ordered by first use) ----
            def ld_x(dst, src, s0):
                nc.sync.dma_start(
                    out=dst[:, s0 : s0 + 512, :], in_=src[:, s0 : s0 + 512, :]
                )

            def ld_xh(dst, src, s0, n=512):
                nc.sync.dma_start(
                    out=dst[:, s0 : s0 + n, :], in_=src[:, s0 : s0 + n, :]
                )

            nc.sync.dma_start(out=w_sb[:, 0:512], in_=w[:, 0:512])
            ld_xh(xt_sb, xt, 0, 128)
            nc.sync.dma_start(out=w_sb[:, 512:1536], in_=w[:, 512:1536])
            ld_xh(xt_sb, xt, 128, 256)
            ld_xh(xt_sb, xt, 384, 128)
            nc.sync.dma_start(out=w_sb[:, 1536:1540], in_=w[:, 1536:1540])
            ld_x(xt_sb, xt, 512)       # chunk 1: wave-1 fresh keys
            ld_x(xt_sb, xt, 1024)      # chunk 2: slot-1 queries
            ld_x(xt_sb, xt, 2048)      # chunk 4: slot-2 queries
            nc.sync.dma_start(out=wo_sb, in_=wo[:, :])
            ld_x(xt_sb, xt, 1536)      # chunk 3
            ld_x(xt_sb, xt, 3072)      # chunk 6: slot-3 queries
            ld_x(xt_sb, xt, 2560)      # chunk 5
            ld_x(xt_sb, xt, 3584)      # chunk 7

            nc.vector.memset(one_sb, 1.0)
            nc.vector.memset(vaug[:, :, D_HEAD : D_HEAD + 1], 1.0)
            make_identity(nc, ident[D_HEAD:P, :])
            # fp32 per-partition exp bias (see w layout note)
            nc.vector.tensor_copy(out=rbias, in_=w_sb[:, 1536:1537])
            # causal band masks generated on the idle Pool engine:
            # msk_sb[p, c, jq] = 1 if 128c + p <= jq else 0, per band c
            for c in range(4):
                nc.gpsimd.memset(msk_sb[:, c, :], 1.0)
                nc.gpsimd.affine_select(
                    out=msk_sb[:, c, :],
                    in_=msk_sb[:, c, :],
                    compare_op=mybir.AluOpType.is_ge,
                    fill=0.0,
                    base=-128 * c,
                    channel_multiplier=-1,
                    pattern=[[1, 512]],
                )

            def kv_proj_mms(sc):
                """Generator: one KV-projection matmul per next() call."""
                kp = mm_ps.tile([P, 512], fp32, tag="mm")
                for dc in range(DCH):
                    nc.tensor.matmul(
                        kp,
                        lhsT=w_sb[:, 512 + dc * 128 : 512 + dc * 128 + 128],
                        rhs=xt_sb[:, sc * 512 : (sc + 1) * 512, dc],
                        start=(dc == 0),
                        stop=(dc == DCH - 1),
                    )
                    yield
                nc.vector.tensor_copy(
                    out=kvt[:, sc * 512 : (sc + 1) * 512], in_=kp
                )
                yield

            def transposes(sc):
                for t in range(4):  # V^T 128-col blocks -> natural V chunks
                    kc = sc * 4 + t
                    tp = s_ps_pool.tile([P, D_HEAD], bf16, tag="s_ps")
                    nc.tensor.transpose(
                        tp,
                        kvt[D_HEAD:P, kc * P : (kc + 1) * P],
                        ident[D_HEAD:P, :],
                    )
                    nc.vector.tensor_copy(out=vaug[:, kc, :D_HEAD], in_=tp)

            # PV accumulators: full-bank tiles, PV uses rows 0:65
            pv = [
                pv_pool.tile([P, 512], fp32, tag=f"pv{g}", name=f"pv{g}")
                for g in range(NSLOT)
            ]
            freed = []  # pv banks released by finalized slots
            y_tiles = {}
            ncopy = [0]
            pending_pv = []

            def emit_pv(j, kc):
                nc.tensor.matmul(
                    pv[j][0 : D_HEAD + 1, :],
                    lhsT=vaug[:, kc, :],
                    rhs=pending_pv_pt.pop((j, kc)),
                    start=(kc == 0),
                    stop=(kc == E[j] - 1),
                    skip_group_check=True,
                )

            pending_pv_pt = {}

            def emit_chunk(j, kc, wv=0):
                sps = s_ps_pool.tile([P, 512], fp32, tag="s_ps")
                nc.tensor.matmul(
                    sps,
                    lhsT=kvt[0:D_HEAD, kc * P : (kc + 1) * P],
                    rhs=qt_sb[:, j * 512 : (j + 1) * 512],
                    start=True,
                    stop=True,
                )
                p_t = work.tile([P, 512], bf16, tag="p_t")
                band = kc - (E[j] - 8)
                if band >= 4:
                    # partner block: role-0 kills the whole chunk via the
                    # exp bias (exp(s - 30) ~ 1e-10); role 1 keeps it whole
                    nc.scalar.activation(
                        p_t, sps, mybir.ActivationFunctionType.Exp,
                        bias=rbias[:, 0:1],
                    )
                else:
                    nc.scalar.activation(
                        p_t, sps, mybir.ActivationFunctionType.Exp
                    )
                if 0 <= band < 4:
                    # diagonal band: per-query causal step mask
                    nc.vector.tensor_tensor(
                        p_t, p_t, msk_sb[:, band, :], mybir.AluOpType.mult
                    )
                pending_pv_pt[(j, kc)] = p_t
                pending_pv.append((j, kc))
                if len(pending_pv) > 10:
                    emit_pv(*pending_pv.pop(0))

            def emit_out_op(j, i, bank=None):
                """One output-projection matmul + scaled PSUM->SBUF copy."""
                t, no = i // 2, i % 2
                if bank is None:
                    bank = freed[ncopy[0] % len(freed)]
                    ncopy[0] += 1
                q0 = j * 512 + t * P
                nc.tensor.matmul(
                    bank,
                    lhsT=ot[0:D_HEAD, q0 : q0 + P],
                    rhs=wo_sb[:, no * 512 : (no + 1) * 512],
                    start=True,
                    stop=True,
                )
                if j == 3 and i % 2 == 0:
                    nc.scalar.mul(
                        y_tiles[j][:, t, no * 512 : (no + 1) * 512],
                        bank,
                        rdent[:, 4 * j + t : 4 * j + t + 1],
                    )
                else:
                    nc.vector.tensor_scalar_mul(
                        y_tiles[j][:, t, no * 512 : (no + 1) * 512],
                        bank,
                        rdent[:, 4 * j + t : 4 * j + t + 1],
                    )
                if j == 3:
                    if i % 2 == 1:
                        # per-tile DMA: few enough that HWDGE desc-gen
                        # (625ns each, serialized) stays off the tail path
                        nc.sync.dma_start(
                            out=y[j][:, t : t + 1, :],
                            in_=y_tiles[j][:, t : t + 1, :],
                        )
                elif i == 3:
                    nc.sync.dma_start(
                        out=y[j][:, 0:2, :], in_=y_tiles[j][:, 0:2, :]
                    )
                elif i == 7:
                    nc.sync.dma_start(
                        out=y[j][:, 2:4, :], in_=y_tiles[j][:, 2:4, :]
                    )

            def finalize_half(j, h):
                """Half of slot-j finalize: O^T + den, 1/den, rdent cols."""
                c0 = j * 512 + h * 256
                c1 = c0 + 256
                nc.vector.tensor_copy(
                    out=ot[:, c0:c1], in_=pv[j][0 : D_HEAD + 1, h * 256 : h * 256 + 256]
                )
                nc.vector.reciprocal(
                    rden[:, c0:c1], ot[D_HEAD : D_HEAD + 1, c0:c1]
                )
                for t in (2 * h, 2 * h + 1):
                    nc.tensor.matmul(
                        pv[j][:, t : t + 1],
                        lhsT=rden[:, j * 512 + t * P : j * 512 + (t + 1) * P],
                        rhs=one_sb,
                        start=True,
                        stop=True,
                    )
                nc.vector.tensor_copy(
                    out=rdent[:, 4 * j + 2 * h : 4 * j + 2 * h + 2],
                    in_=pv[j][:, 2 * h : 2 * h + 2],
                )

            def finalize(j):
                finalize_half(j, 0)
                finalize_half(j, 1)
                freed.append(pv[j])
                y_tiles[j] = ypool.tile(
                    [P, 4, D_MODEL], bf16, tag="y_sb", name=f"y{j}"
                )

            def q_piece(c0, n):
                """Prologue Q projection over columns [c0, c0+n) of slot 0.
                Uses the s_ps pool so pieces rotate PSUM banks instead of
                serializing on the single mm bank."""
                qp = s_ps_pool.tile([D_HEAD, n], fp32, tag="s_ps", name=f"qp{c0}")
                for dc in range(DCH):
                    nc.tensor.matmul(
                        qp,
                        lhsT=w_sb[:, dc * 64 : dc * 64 + 64],
                        rhs=xt_sb[:, c0 : c0 + n, dc],
                        start=(dc == 0),
                        stop=(dc == DCH - 1),
                    )
                nc.vector.tensor_copy(out=qt_sb[:, c0 : c0 + n], in_=qp)

            def kv_piece(c0, n):
                kp = s_ps_pool.tile([P, n], fp32, tag="s_ps", name=f"kp{c0}")
                for dc in range(DCH):
                    nc.tensor.matmul(
                        kp,
                        lhsT=w_sb[:, 512 + dc * 128 : 512 + dc * 128 + 128],
                        rhs=xt_sb[:, c0 : c0 + n, dc],
                        start=(dc == 0),
                        stop=(dc == DCH - 1),
                    )
                nc.vector.tensor_copy(out=kvt[:, c0 : c0 + n], in_=kp)

            def q_proj_steps(j):
                """Generator version of q_proj: one matmul per next()."""
                qp = mm_ps.tile([D_HEAD, 512], fp32, tag="mm")
                for dc in range(DCH):
                    nc.tensor.matmul(
                        qp,
                        lhsT=w_sb[:, dc * 64 : dc * 64 + 64],
                        rhs=xt_sb[:, j * 1024 : j * 1024 + 512, dc],
                        start=(dc == 0),
                        stop=(dc == DCH - 1),
                    )
                    yield
                nc.vector.tensor_copy(
                    out=qt_sb[:, j * 512 : (j + 1) * 512], in_=qp
                )
                yield

            def transpose_steps(sc):
                tp = s_ps_pool.tile([P, 4, D_HEAD], bf16, tag="s_ps", name="tp4")
                for t in range(4):
                    kc = sc * 4 + t
                    nc.tensor.transpose(
                        tp[:, t, :],
                        kvt[D_HEAD:P, kc * P : (kc + 1) * P],
                        ident[D_HEAD:P, :],
                    )
                    yield
                nc.vector.tensor_copy(
                    out=vaug[:, sc * 4 : sc * 4 + 4, :D_HEAD], in_=tp
                )
                yield

            # Q projection for slot j runs as filler inside wave j (its
            # first consumers are that wave's chunks)
            qproj_wave = {1: 1, 2: 2, 3: 3}

            # ---- prologue + wave 0: piece-width projections matched to the
            # DMA bite arrival order (x cols 0:128, 128:384, 384:512). The
            # V-transposes for kc 0..1 must be emitted before chunk (0,3)
            # pops PV(0,0), else the vaug dependency is never recorded. ----
            tg0 = transpose_steps(0)
            q_piece(0, 128)
            kv_piece(0, 128)
            q_piece(128, 256)
            kv_piece(128, 256)
            q_piece(384, 128)
            kv_piece(384, 128)
            next(tg0)
            next(tg0)
            emit_chunk(0, 0, 0)
            emit_chunk(0, 1, 0)
            next(tg0)
            next(tg0)
            emit_chunk(0, 2, 0)
            emit_chunk(0, 3, 0)
            for _ in tg0:  # drain: emits the packed vaug copy for kc 0..3
                pass

            # ---- streamed waves ----
            # Wave sc fillers: [qproj (if due), KV proj for sc, spacer,
            # V transposes for sc], popped two per chunk. Wave 0's proj and
            # transposes ran in the prologue.
            for sc in range(1, NWAVE):
                chunks = WAVES[sc]
                oo = list(out_sched[sc])
                nq = 9 if sc in qproj_wave else 0
                filler = []
                if nq:
                    filler.append(q_proj_steps(qproj_wave[sc]))
                # superchunk 7's projection and transposes run in wave 6
                # (its x arrives ~27us, far earlier) so wave 7 starts with
                # kvt/vaug complete and no filler gating
                kv_scs = (sc,)
                for s in kv_scs:
                    filler.append(kv_proj_mms(s))
                    filler.append(iter([None, None]))  # spacer: kvt drains
                    filler.append(transpose_steps(s))
                total_fill = nq + 16 * len(kv_scs)
                fill_iter = (x for g in filler for x in g)
                pops = [0]

                def pop_fill(upto=None, k=None):
                    tgt = upto if upto is not None else pops[0] + k
                    while pops[0] < min(tgt, total_fill):
                        if next(fill_iter, -1) == -1:
                            pops[0] = total_fill
                            break
                        pops[0] += 1

                # emission-order safety points: a fresh chunk's scores need
                # this wave's kvt copy emitted; a fresh chunk's PV needs its
                # V-transpose emitted
                proj_safe = nq + 9
                n = len(chunks)
                for ci, (j, kc) in enumerate(chunks):
                    if kc // 4 == sc:
                        pop_fill(upto=proj_safe)
                    if len(pending_pv) >= 4:
                        j2, kc2 = pending_pv[0]
                        if kc2 // 4 == sc:
                            pop_fill(upto=total_fill)
                    emit_chunk(j, kc, sc)
                    if oo and ci % 2 == 1:
                        emit_out_op(*oo.pop(0))
                    k = -(-(total_fill - pops[0]) // (n - ci))  # ceil
                    pop_fill(k=min(k, 4))
                pop_fill(upto=total_fill)
                for op in oo:
                    emit_out_op(*op)
                fin = [j for j in range(NSLOT) if last_wave[j] == sc and j != 3]
                if fin:
                    while pending_pv:
                        emit_pv(*pending_pv.pop(0))
                    for j in fin:
                        finalize(j)

            # ---- slot-3 tail: per-128-query-tile pipeline. The ot copy and
            # the reciprocal both read the pv PSUM bank directly (no serial
            # copy->recip dependency), then rdent -> out-proj x2 -> scaled
            # copy (DVE half, ACT half) -> per-tile y DMA, so successive
            # tiles overlap across engines. ----
            while pending_pv:
                emit_pv(*pending_pv.pop(0))
            y_tiles[3] = ypool.tile([P, 4, D_MODEL], bf16, tag="y_sb", name="y3")

            def finalize_q(j, t):
                """Per-128-query finalize: reciprocal reads the pv PSUM
                bank directly; the ot copy is one full-width ACT op emitted
                by the caller."""
                c0 = j * 512 + t * P
                nc.vector.reciprocal(
                    rden[:, c0 : c0 + P],
                    pv[j][D_HEAD : D_HEAD + 1, t * P : (t + 1) * P],
                )
                nc.tensor.matmul(
                    pv[j][:, t : t + 1],
                    lhsT=rden[:, c0 : c0 + P],
                    rhs=one_sb,
                    start=True,
                    stop=True,
                )
                nc.vector.tensor_copy(
                    out=rdent[:, 4 * j + t : 4 * j + t + 1],
                    in_=pv[j][:, t : t + 1],
                )

            # emission order keeps each engine FIFO unblocked: the recip
            # chains for tiles 0-2 all precede the first scale op. Each
            # out-op gets its own PSUM bank (score + mm banks are idle by
            # now) so no matmul waits on a previous scale's drain.
            tail_banks = [
                s_ps_pool.tile([P, 512], fp32, tag="s_ps", name=f"tb{k}")
                for k in range(3)
            ] + [mm_ps.tile([P, 512], fp32, tag="mm", name="tb3")]
            nc.scalar.activation(
                ot[0:D_HEAD, 1536:2048],
                pv[3][0:D_HEAD, :],
                mybir.ActivationFunctionType.Copy,
            )
            nc.vector.reciprocal(
                rden[:, 1536:2048], pv[3][D_HEAD : D_HEAD + 1, :]
            )
            for t in range(4):
                nc.tensor.matmul(
                    pv[3][:, t : t + 1],
                    lhsT=rden[:, 1536 + t * P : 1536 + (t + 1) * P],
                    rhs=one_sb,
                    start=True,
                    stop=True,
                )
            nc.vector.tensor_copy(out=rdent[:, 12:16], in_=pv[3][:, 0:4])
            emit_out_op(3, 0, bank=tail_banks[0])
            emit_out_op(3, 1, bank=tail_banks[1])
            emit_out_op(3, 2, bank=tail_banks[2])
            emit_out_op(3, 3, bank=tail_banks[3])
            emit_out_op(3, 4, bank=pv[0])
            emit_out_op(3, 5, bank=pv[1])
            emit_out_op(3, 6, bank=pv[2])
            # pv[3] frees as soon as the consolidated rdent copy has read
            # cols 0:4 (~right after the last PV), earlier than any scale
            emit_out_op(3, 7, bank=pv[3])

    nc.finalize()
    return nc


def _get_program():
    global _prog
    if _prog is None:
        _prog = _build_program()
    return _prog


def kernel(x, W_q, W_k, W_v, W_o):
    import ml_dtypes
    from concourse.bass_utils import run_bass_kernel_spmd

    bf = ml_dtypes.bfloat16
    nc = _get_program()

    x = np.asarray(x, dtype=np.float32)
    scale = np.float32(1.0 / np.sqrt(D_HEAD))
    wq_s = np.asarray(W_q, dtype=np.float32) * scale
    wkv = np.concatenate(
        [np.asarray(W_k, dtype=np.float32), np.asarray(W_v, dtype=np.float32)],
        axis=1,
    )  # [1024, 128]
    wq_part = wq_s.reshape(DCH, P, 64).transpose(1, 0, 2).reshape(P, 512)
    wkv_part = wkv.reshape(DCH, P, 128).transpose(1, 0, 2).reshape(P, 1024)
    # per-role exp bias for partner-band chunks: -30 kills the whole chunk
    # for role 0 (exp(s - 30) ~ 1e-10), 0 keeps it whole for role 1
    w_hosts = []
    for r in range(2):
        rb = np.full((P, 4), 0.0 if r == 1 else -30.0, dtype=np.float32)
        w_hosts.append(
            np.ascontiguousarray(
                np.concatenate([wq_part, wkv_part, rb], axis=1)
            ).astype(bf)
        )  # [128, 1540]
    wo_host = np.ascontiguousarray(np.asarray(W_o, dtype=np.float32)).astype(bf)

    in_maps = []
    for c in range(NCORES):
        b, r = c // 2, c % 2
        xt_b = x[b].T  # [1024, 4096]
        # permuted key order: position 2k holds this core's query block
        # 2k+r, position 2k+1 holds the partner block 2k+(1-r)
        cols = np.concatenate(
            [
                np.arange(512 * g, 512 * g + 512)
                for k in range(NSLOT)
                for g in (2 * k + r, 2 * k + 1 - r)
            ]
        )
        xt_host = np.ascontiguousarray(
            xt_b[:, cols].reshape(DCH, P, SEQ).transpose(1, 2, 0)
        ).astype(bf)  # [128, 4096, 8]
        in_maps.append(
            {
                "xt": xt_host,
                "w": w_hosts[r],
                "wo": wo_host,
            }
        )

    res = run_bass_kernel_spmd(nc, in_maps, core_ids=list(range(NCORES)))
    out = np.empty((BATCH, SEQ, D_MODEL), dtype=np.float32)
    for c in range(NCORES):
        b, r = c // 2, c % 2
        yv = np.asarray(res.results[c]["y"]).astype(np.float32)
        # y[j, p, t, :] -> query 512*(2j+r) + 128t + p
        yv = yv.transpose(0, 2, 1, 3)  # [j, t, p, m]
        for j in range(NSLOT):
            q0 = 512 * (2 * j + r)
            out[b, q0 : q0 + 512, :] = yv[j].reshape(512, D_MODEL)
    return out



# revision 90
# speedup vs baseline: 1.0477x; 1.0334x over previous
"""Trainium2 Bass kernel for single-head causal attention.

x:[4,4096,1024] f32, W_q/W_k/W_v:[1024,64], W_o:[64,1024].

Sharding: 8 cores = 4 batches x 2 query-stripe roles. Role r of a batch
owns query blocks {2j+r : j=0..3} (512 queries each). Program slot j has
key extent E[j] = (8j+8) 128-key chunks, which exactly covers role 1's
block 2j+1 and over-covers role 0's block 2j by 4 chunks (dead).

All per-core differences (which batch, which stripe, dead chunks) are
carried in the input data; one SPMD program runs on all 8 cores:
- x is shipped as [128, 4096, 8] (d_model-chunk partition, seq, chunk)
  so any column range is a fully contiguous DMA (no sub-512B-element
  descriptor penalty), streamed in arrival-ordered bites.
- The diagonal causal band masks (4 x [128,512]) are generated on the
  otherwise-idle Pool engine with affine_select, not DMAed.
- Partner-block chunks (band >= 4) take no mask op at all: the exp runs
  with a per-partition bias from the w tensor (-30 for role 0 => p ~
  1e-10, 0 for role 1), so role-0's dead chunks vanish from both the PV
  numerator and the denominator row.

bf16 everywhere on the matmul paths (end-to-end rel err ~5e-3 vs the
2e-2 gate). Engines: PE does all matmuls (62.3us busy, the roofline of
this schedule), ACT does exp + the slot-3 ot copies + half its output
scales, DVE does diag masks + kvt/qt copies + finalize + the other
scales. Attention chunks are emitted in waves matched to DMA arrival,
next wave's KV projection interleaved between chunks as PE filler, PV
matmuls lagged 10 chunks behind their scores. The slot-3 tail finalize
is consolidated into full-width ops (one [64,512] ot copy on ACT, one
[1,512] reciprocal, one [P,4] rdent copy, all reading the PV PSUM bank
directly) so all eight out-projection matmuls unblock at once; each
out-op has its own PSUM bank and the y DMAs go out per 128-query tile
to keep the final HWDGE descriptor-gen chain (625ns each, serialized)
short.
"""

import sys

for _p in ("/opt/trn_rl_repo",):
    if _p not in sys.path:
        sys.path.insert(0, _p)

import numpy as np

D_MODEL = 1024
D_HEAD = 64
SEQ = 4096
BATCH = 4
NCORES = 8
NQ = 2048          # queries per core
P = 128
DCH = D_MODEL // P  # 8 contraction chunks
NSLOT = 4           # query slots of 512
E = [8, 16, 24, 32]  # key chunks per slot
NWAVE = 8           # key superchunks of 512

# Attention chunks per wave: matched to DMA arrival order (early waves
# light), per-slot ascending kc, every chunk (j,kc) in wave >= kc//4.
WAVES = [
    [(0, k) for k in range(4)],
    [(0, k) for k in range(4, 8)] + [(1, k) for k in range(8)],
    [(1, k) for k in range(8, 12)] + [(2, k) for k in range(8)],
    [(1, k) for k in range(12, 16)] + [(2, k) for k in range(8, 12)]
    + [(3, k) for k in range(4)],
    [(2, k) for k in range(12, 16)] + [(3, k) for k in range(4, 8)],
    [(2, k) for k in range(16, 24)] + [(3, k) for k in range(8, 12)],
    [(3, k) for k in range(12, 16)],
    [(3, k) for k in range(16, 32)],
]

_prog = None


def _check_waves():
    seen = {}
    total = 0
    for w, wv in enumerate(WAVES):
        for j, kc in wv:
            assert kc // 4 <= w, (w, j, kc)
            assert seen.get(j, -1) == kc - 1, (j, kc)
            seen[j] = kc
            total += 1
    assert total == sum(E) == 80
    return {j: max(w for w, wv in enumerate(WAVES) if (j, E[j] - 1) in wv)
            for j in range(NSLOT)}


def _build_program():
    import concourse.bacc as bacc
    import concourse.mybir as mybir
    import concourse.tile as tile
    from concourse.masks import make_identity

    fp32 = mybir.dt.float32
    f32r = mybir.dt.float32r
    bf16 = mybir.dt.bfloat16
    nc = bacc.Bacc("TRN2", target_bir_lowering=False, debug=False)

    # w layout: [wq 0:512 | wkv 512:1536 | rbias 1536:1540]
    # rbias: -30 for role 0, 0 for role 1. Partner-band chunks (band >= 4)
    # run exp with this per-partition bias instead of a 0/1 mask multiply:
    # each slot's band region only ever covers its own partner block, which
    # is entirely dead for role 0 (exp(s - 30) ~ 1e-10) and entirely alive
    # for role 1 (bias 0). Replaces the 16 explicit partner-mask DVE ops.
    xt = nc.dram_tensor("xt", [P, SEQ, DCH], bf16, kind="ExternalInput")
    w = nc.dram_tensor("w", [P, DCH * 192 + 4], bf16, kind="ExternalInput")
    wo = nc.dram_tensor("wo", [D_HEAD, D_MODEL], bf16, kind="ExternalInput")
    y = nc.dram_tensor("y", [NSLOT, P, 4, D_MODEL], bf16, kind="ExternalOutput")

    last_wave = _check_waves()
    # output-projection ops (j, i): slot0 -> waves 2,3; slot1 -> 5,6;
    # slot2 -> wave 6 (where DVE is light: slot-3 chunks kc<24 need no
    # masks); slot3 -> tail (wave index NWAVE)
    out_sched = {wi: [] for wi in range(NWAVE + 1)}
    for j, tgt in ((0, (2, 3)), (1, (5, 6)), (2, (6, 7)), (3, (8, 8))):
        for i in range(8):
            out_sched[tgt[i // 4]].append((j, i))

    with tile.TileContext(nc) as tc:
        with (
            tc.tile_pool(name="singles", bufs=1) as singles,
            tc.tile_pool(name="work", bufs=10) as work,
            tc.tile_pool(name="ypool", bufs=2) as ypool,
            tc.tile_pool(name="mm_ps", bufs=1, space="PSUM") as mm_ps,
            tc.tile_pool(name="s_ps", bufs=3, space="PSUM") as s_ps_pool,
            tc.tile_pool(name="pv_ps", bufs=1, space="PSUM") as pv_pool,
        ):
            # ---- persistent SBUF ----
            w_sb = singles.tile([P, DCH * 192 + 4], bf16, tag="w_sb")
            xt_sb = singles.tile([P, SEQ, DCH], bf16, tag="xt_sb")
            msk_sb = singles.tile([P, 4, 512], bf16, tag="msk_sb")
            wo_sb = singles.tile([D_HEAD, D_MODEL], bf16, tag="wo_sb")
            kvt = singles.tile([P, SEQ], bf16, tag="kvt")  # 0:64 K^T, 64:128 V^T
            qt_sb = singles.tile([D_HEAD, NQ], bf16, tag="qt_sb")
            vaug = singles.tile([P, 32, D_HEAD + 1], bf16, tag="vaug")
            ot = singles.tile([D_HEAD + 1, NQ], bf16, tag="ot")
            rden = singles.tile([1, NQ], fp32, tag="rden")
            rbias = singles.tile([P, 1], fp32, tag="rbias")
            rdent = singles.tile([P, 16], fp32, tag="rdent")
            ident = singles.tile([P, D_HEAD], bf16, tag="ident")
            one_sb = singles.tile([1, 1], fp32, tag="one_sb")

            # ---- input DMAs (SP queue, ordered by first use) ----
            def ld_x(dst, src, s0):
                nc.sync.dma_start(
                    out=dst[:, s0 : s0 + 512, :], in_=src[:, s0 : s0 + 512, :]
                )

            def ld_xh(dst, src, s0, n=512):
                nc.sync.dma_start(
                    out=dst[:, s0 : s0 + n, :], in_=src[:, s0 : s0 + n, :]
                )

            nc.sync.dma_start(out=w_sb[:, 0:512], in_=w[:, 0:512])
            ld_xh(xt_sb, xt, 0, 128)
            nc.sync.dma_start(out=w_sb[:, 512:1536], in_=w[:, 512:1536])
            ld_xh(xt_sb, xt, 128, 256)
            ld_xh(xt_sb, xt, 384, 128)
            nc.sync.dma_start(out=w_sb[:, 1536:1540], in_=w[:, 1536:1540])
            ld_x(xt_sb, xt, 512)       # chunk 1: wave-1 fresh keys
            ld_x(xt_sb, xt, 1024)      # chunk 2: slot-1 queries
            ld_x(xt_sb, xt, 2048)      # chunk 4: slot-2 queries
            nc.sync.dma_start(out=wo_sb, in_=wo[:, :])
            ld_x(xt_sb, xt, 1536)      # chunk 3
            ld_x(xt_sb, xt, 3072)      # chunk 6: slot-3 queries
            ld_x(xt_sb, xt, 2560)      # chunk 5
            ld_x(xt_sb, xt, 3584)      # chunk 7

            nc.vector.memset(one_sb, 1.0)
            nc.vector.memset(vaug[:, :, D_HEAD : D_HEAD + 1], 1.0)
            make_identity(nc, ident[D_HEAD:P, :])
            # fp32 per-partition exp bias (see w layout note)
            nc.vector.tensor_copy(out=rbias, in_=w_sb[:, 1536:1537])
            # causal band masks generated on the idle Pool engine:
            # msk_sb[p, c, jq] = 1 if 128c + p <= jq else 0, per band c
            for c in range(4):
                nc.gpsimd.memset(msk_sb[:, c, :], 1.0)
                nc.gpsimd.affine_select(
                    out=msk_sb[:, c, :],
                    in_=msk_sb[:, c, :],
                    compare_op=mybir.AluOpType.is_ge,
                    fill=0.0,
                    base=-128 * c,
                    channel_multiplier=-1,
                    pattern=[[1, 512]],
                )

            def kv_proj_mms(sc):
                """Generator: one KV-projection matmul per next() call."""
                kp = mm_ps.tile([P, 512], fp32, tag="mm")
                for dc in range(DCH):
                    nc.tensor.matmul(
                        kp,
                        lhsT=w_sb[:, 512 + dc * 128 : 512 + dc * 128 + 128],
                        rhs=xt_sb[:, sc * 512 : (sc + 1) * 512, dc],
                        start=(dc == 0),
                        stop=(dc == DCH - 1),
                    )
                    yield
                nc.vector.tensor_copy(
                    out=kvt[:, sc * 512 : (sc + 1) * 512], in_=kp
                )
                yield

            def transposes(sc):
                for t in range(4):  # V^T 128-col blocks -> natural V chunks
                    kc = sc * 4 + t
                    tp = s_ps_pool.tile([P, D_HEAD], bf16, tag="s_ps")
                    nc.tensor.transpose(
                        tp,
                        kvt[D_HEAD:P, kc * P : (kc + 1) * P],
                        ident[D_HEAD:P, :],
                    )
                    nc.vector.tensor_copy(out=vaug[:, kc, :D_HEAD], in_=tp)

            # PV accumulators: full-bank tiles, PV uses rows 0:65
            pv = [
                pv_pool.tile([P, 512], fp32, tag=f"pv{g}", name=f"pv{g}")
                for g in range(NSLOT)
            ]
            freed = []  # pv banks released by finalized slots
            y_tiles = {}
            ncopy = [0]
            pending_pv = []

            def emit_pv(j, kc):
                nc.tensor.matmul(
                    pv[j][0 : D_HEAD + 1, :],
                    lhsT=vaug[:, kc, :],
                    rhs=pending_pv_pt.pop((j, kc)),
                    start=(kc == 0),
                    stop=(kc == E[j] - 1),
                    skip_group_check=True,
                )

            pending_pv_pt = {}

            def chunk_finish(j, kc, sps):
                p_t = work.tile([P, 512], bf16, tag="p_t")
                band = kc - (E[j] - 8)
                if band >= 4:
                    # partner block: role-0 kills the whole chunk via the
                    # exp bias (exp(s - 30) ~ 1e-10); role 1 keeps it whole
                    nc.scalar.activation(
                        p_t, sps, mybir.ActivationFunctionType.Exp,
                        bias=rbias[:, 0:1],
                    )
                else:
                    nc.scalar.activation(
                        p_t, sps, mybir.ActivationFunctionType.Exp
                    )
                if 0 <= band < 4:
                    # diagonal band: per-query causal step mask
                    nc.vector.tensor_tensor(
                        p_t, p_t, msk_sb[:, band, :], mybir.AluOpType.mult
                    )
                pending_pv_pt[(j, kc)] = p_t
                pending_pv.append((j, kc))
                if len(pending_pv) > 10:
                    emit_pv(*pending_pv.pop(0))

            def emit_chunk(j, kc, wv=0):
                sps = s_ps_pool.tile([P, 512], fp32, tag="s_ps")
                nc.tensor.matmul(
                    sps,
                    lhsT=kvt[0:D_HEAD, kc * P : (kc + 1) * P],
                    rhs=qt_sb[:, j * 512 : (j + 1) * 512],
                    start=True,
                    stop=True,
                )
                chunk_finish(j, kc, sps)

            def emit_out_op(j, i, bank=None):
                """One output-projection matmul + scaled PSUM->SBUF copy."""
                t, no = i // 2, i % 2
                if bank is None:
                    bank = freed[ncopy[0] % len(freed)]
                    ncopy[0] += 1
                q0 = j * 512 + t * P
                nc.tensor.matmul(
                    bank,
                    lhsT=ot[0:D_HEAD, q0 : q0 + P],
                    rhs=wo_sb[:, no * 512 : (no + 1) * 512],
                    start=True,
                    stop=True,
                )
                if j == 3 and i % 2 == 0:
                    nc.scalar.mul(
                        y_tiles[j][:, t, no * 512 : (no + 1) * 512],
                        bank,
                        rdent[:, 4 * j + t : 4 * j + t + 1],
                    )
                else:
                    nc.vector.tensor_scalar_mul(
                        y_tiles[j][:, t, no * 512 : (no + 1) * 512],
                        bank,
                        rdent[:, 4 * j + t : 4 * j + t + 1],
                    )
                if j == 3:
                    if i % 2 == 1:
                        # per-tile DMA: few enough that HWDGE desc-gen
                        # (625ns each, serialized) stays off the tail path
                        nc.sync.dma_start(
                            out=y[j][:, t : t + 1, :],
                            in_=y_tiles[j][:, t : t + 1, :],
                        )
                elif i == 3:
                    nc.sync.dma_start(
                        out=y[j][:, 0:2, :], in_=y_tiles[j][:, 0:2, :]
                    )
                elif i == 7:
                    nc.sync.dma_start(
                        out=y[j][:, 2:4, :], in_=y_tiles[j][:, 2:4, :]
                    )

            def finalize_half(j, h):
                """Half of slot-j finalize: O^T + den, 1/den, rdent cols."""
                c0 = j * 512 + h * 256
                c1 = c0 + 256
                nc.vector.tensor_copy(
                    out=ot[:, c0:c1], in_=pv[j][0 : D_HEAD + 1, h * 256 : h * 256 + 256]
                )
                nc.vector.reciprocal(
                    rden[:, c0:c1], ot[D_HEAD : D_HEAD + 1, c0:c1]
                )
                for t in (2 * h, 2 * h + 1):
                    nc.tensor.matmul(
                        pv[j][:, t : t + 1],
                        lhsT=rden[:, j * 512 + t * P : j * 512 + (t + 1) * P],
                        rhs=one_sb,
                        start=True,
                        stop=True,
                    )
                nc.vector.tensor_copy(
                    out=rdent[:, 4 * j + 2 * h : 4 * j + 2 * h + 2],
                    in_=pv[j][:, 2 * h : 2 * h + 2],
                )

            def finalize(j):
                # consolidated (same shape as the slot-3 tail): full-width
                # ot copy + reciprocal reading the pv PSUM bank directly,
                # then the four rdent transposes and one rdent copy
                nc.vector.tensor_copy(
                    out=ot[0:D_HEAD, j * 512 : (j + 1) * 512],
                    in_=pv[j][0:D_HEAD, :],
                )
                nc.vector.reciprocal(
                    rden[:, j * 512 : (j + 1) * 512],
                    pv[j][D_HEAD : D_HEAD + 1, :],
                )
                for t in range(4):
                    nc.tensor.matmul(
                        pv[j][:, t : t + 1],
                        lhsT=rden[:, j * 512 + t * P : j * 512 + (t + 1) * P],
                        rhs=one_sb,
                        start=True,
                        stop=True,
                    )
                nc.vector.tensor_copy(
                    out=rdent[:, 4 * j : 4 * j + 4], in_=pv[j][:, 0:4]
                )
                freed.append(pv[j])
                y_tiles[j] = ypool.tile(
                    [P, 4, D_MODEL], bf16, tag="y_sb", name=f"y{j}"
                )

            def q_piece(c0, n, bank):
                """Prologue Q projection over columns [c0, c0+n) of slot 0.
                Pieces borrow the (still empty) pv accumulator banks as
                scratch so the s_ps pool is free for the wave-0 score
                tiles."""
                qp = bank[0:D_HEAD, 0:n]
                for dc in range(DCH):
                    nc.tensor.matmul(
                        qp,
                        lhsT=w_sb[:, dc * 64 : dc * 64 + 64],
                        rhs=xt_sb[:, c0 : c0 + n, dc],
                        start=(dc == 0),
                        stop=(dc == DCH - 1),
                        skip_group_check=True,
                    )
                nc.vector.tensor_copy(out=qt_sb[:, c0 : c0 + n], in_=qp)

            def kv_piece(c0, n, bank):
                kp = bank[:, 0:n]
                for dc in range(DCH):
                    nc.tensor.matmul(
                        kp,
                        lhsT=w_sb[:, 512 + dc * 128 : 512 + dc * 128 + 128],
                        rhs=xt_sb[:, c0 : c0 + n, dc],
                        start=(dc == 0),
                        stop=(dc == DCH - 1),
                        skip_group_check=True,
                    )
                nc.vector.tensor_copy(out=kvt[:, c0 : c0 + n], in_=kp)

            def q_proj_steps(j):
                """Generator version of q_proj: one matmul per next()."""
                qp = mm_ps.tile([D_HEAD, 512], fp32, tag="mm")
                for dc in range(DCH):
                    nc.tensor.matmul(
                        qp,
                        lhsT=w_sb[:, dc * 64 : dc * 64 + 64],
                        rhs=xt_sb[:, j * 1024 : j * 1024 + 512, dc],
                        start=(dc == 0),
                        stop=(dc == DCH - 1),
                    )
                    yield
                nc.vector.tensor_copy(
                    out=qt_sb[:, j * 512 : (j + 1) * 512], in_=qp
                )
                yield

            def transpose_steps(sc):
                tp = s_ps_pool.tile([P, 4, D_HEAD], bf16, tag="s_ps", name="tp4")
                for t in range(4):
                    kc = sc * 4 + t
                    nc.tensor.transpose(
                        tp[:, t, :],
                        kvt[D_HEAD:P, kc * P : (kc + 1) * P],
                        ident[D_HEAD:P, :],
                    )
                    yield
                nc.vector.tensor_copy(
                    out=vaug[:, sc * 4 : sc * 4 + 4, :D_HEAD], in_=tp
                )
                yield

            # Q projection for slot j runs as filler inside wave j (its
            # first consumers are that wave's chunks)
            qproj_wave = {1: 1, 2: 2, 3: 3}

            # ---- prologue + wave 0: piece-width projections matched to the
            # DMA bite arrival order (x cols 0:128, 128:384, 384:512). The
            # V-transposes for kc 0..1 must be emitted before chunk (0,3)
            # pops PV(0,0), else the vaug dependency is never recorded. ----
            tg0 = transpose_steps(0)

            def part_score(sps, kc, q0, q1):
                nc.tensor.matmul(
                    sps[:, q0:q1],
                    lhsT=kvt[0:D_HEAD, kc * P : (kc + 1) * P],
                    rhs=qt_sb[:, q0:q1],
                    start=True,
                    stop=True,
                    skip_group_check=True,
                )

            # wave-0 scores are emitted in PARTIAL query ranges matched to
            # the x DMA bites, so the scheduler has legal PE work during
            # the serial input stream instead of idling until a full
            # 512-query slot is projected
            sc_t = [
                s_ps_pool.tile([P, 512], fp32, tag="s_ps", name=f"sp0_{k}")
                for k in range(3)
            ]
            q_piece(0, 128, pv[1])
            kv_piece(0, 128, pv[2])
            part_score(sc_t[0], 0, 0, 128)
            q_piece(128, 256, pv[3])
            kv_piece(128, 256, pv[1])
            next(tg0)
            next(tg0)
            part_score(sc_t[0], 0, 128, 384)
            part_score(sc_t[1], 1, 0, 384)
            q_piece(384, 128, pv[2])
            kv_piece(384, 128, pv[3])
            next(tg0)
            next(tg0)
            part_score(sc_t[0], 0, 384, 512)
            chunk_finish(0, 0, sc_t[0])
            part_score(sc_t[1], 1, 384, 512)
            chunk_finish(0, 1, sc_t[1])
            part_score(sc_t[2], 2, 0, 512)
            chunk_finish(0, 2, sc_t[2])
            emit_chunk(0, 3, 0)
            for _ in tg0:  # drain: emits the packed vaug copy for kc 0..3
                pass

            # ---- streamed waves ----
            # Wave sc fillers: [qproj (if due), KV proj for sc, spacer,
            # V transposes for sc], popped two per chunk. Wave 0's proj and
            # transposes ran in the prologue.
            for sc in range(1, NWAVE):
                chunks = WAVES[sc]
                oo = list(out_sched[sc])
                nq = 9 if sc in qproj_wave else 0
                # kv projection FIRST: both share the single mm PSUM bank,
                # and the q projection may wait on later-arriving x columns
                # — allocated first it would block the kv chain via WAR
                filler = [
                    kv_proj_mms(sc),
                    iter([None, None]),  # spacer: kvt copy drains
                    transpose_steps(sc),
                ]
                if nq:
                    filler.append(q_proj_steps(qproj_wave[sc]))
                total_fill = nq + 16
                fill_iter = (x for g in filler for x in g)
                pops = [0]

                def pop_fill(upto=None, k=None):
                    tgt = upto if upto is not None else pops[0] + k
                    while pops[0] < min(tgt, total_fill):
                        if next(fill_iter, -1) == -1:
                            pops[0] = total_fill
                            break
                        pops[0] += 1

                # emission-order safety points: a fresh chunk's scores need
                # this wave's kvt copy emitted; a fresh chunk's PV needs its
                # V-transpose emitted
                n = len(chunks)
                for ci, (j, kc) in enumerate(chunks):
                    if kc // 4 == sc:
                        pop_fill(upto=9)
                    if sc in qproj_wave and j == qproj_wave[sc]:
                        pop_fill(upto=total_fill)
                    if len(pending_pv) >= 4:
                        j2, kc2 = pending_pv[0]
                        if kc2 // 4 == sc:
                            pop_fill(upto=total_fill)
                    emit_chunk(j, kc, sc)
                    if oo and ci % 2 == 1:
                        emit_out_op(*oo.pop(0))
                    k = -(-(total_fill - pops[0]) // (n - ci))  # ceil
                    pop_fill(k=min(k, 4))
                pop_fill(upto=total_fill)
                for op in oo:
                    emit_out_op(*op)
                fin = [j for j in range(NSLOT) if last_wave[j] == sc and j != 3]
                if fin:
                    while pending_pv:
                        emit_pv(*pending_pv.pop(0))
                    for j in fin:
                        finalize(j)

            # ---- slot-3 tail: per-128-query-tile pipeline. The ot copy and
            # the reciprocal both read the pv PSUM bank directly (no serial
            # copy->recip dependency), then rdent -> out-proj x2 -> scaled
            # copy (DVE half, ACT half) -> per-tile y DMA, so successive
            # tiles overlap across engines. ----
            while pending_pv:
                emit_pv(*pending_pv.pop(0))
            y_tiles[3] = ypool.tile([P, 4, D_MODEL], bf16, tag="y_sb", name="y3")

            def finalize_q(j, t):
                """Per-128-query finalize: reciprocal reads the pv PSUM
                bank directly; the ot copy is one full-width ACT op emitted
                by the caller."""
                c0 = j * 512 + t * P
                nc.vector.reciprocal(
                    rden[:, c0 : c0 + P],
                    pv[j][D_HEAD : D_HEAD + 1, t * P : (t + 1) * P],
                )
                nc.tensor.matmul(
                    pv[j][:, t : t + 1],
                    lhsT=rden[:, c0 : c0 + P],
                    rhs=one_sb,
                    start=True,
                    stop=True,
                )
                nc.vector.tensor_copy(
                    out=rdent[:, 4 * j + t : 4 * j + t + 1],
                    in_=pv[j][:, t : t + 1],
                )

            # emission order keeps each engine FIFO unblocked: the recip
            # chains for tiles 0-2 all precede the first scale op. Each
            # out-op gets its own PSUM bank (score + mm banks are idle by
            # now) so no matmul waits on a previous scale's drain.
            tail_banks = [
                s_ps_pool.tile([P, 512], fp32, tag="s_ps", name=f"tb{k}")
                for k in range(3)
            ] + [mm_ps.tile([P, 512], fp32, tag="mm", name="tb3")]
            nc.scalar.activation(
                ot[0:D_HEAD, 1536:2048],
                pv[3][0:D_HEAD, :],
                mybir.ActivationFunctionType.Copy,
            )
            nc.vector.reciprocal(
                rden[:, 1536:2048], pv[3][D_HEAD : D_HEAD + 1, :]
            )
            for t in range(4):
                nc.tensor.matmul(
                    pv[3][:, t : t + 1],
                    lhsT=rden[:, 1536 + t * P : 1536 + (t + 1) * P],
                    rhs=one_sb,
                    start=True,
                    stop=True,
                )
            nc.vector.tensor_copy(out=rdent[:, 12:16], in_=pv[3][:, 0:4])
            emit_out_op(3, 0, bank=tail_banks[0])
            emit_out_op(3, 1, bank=tail_banks[1])
            emit_out_op(3, 2, bank=tail_banks[2])
            emit_out_op(3, 3, bank=tail_banks[3])
            emit_out_op(3, 4, bank=pv[0])
            emit_out_op(3, 5, bank=pv[1])
            emit_out_op(3, 6, bank=pv[2])
            # pv[3] frees as soon as the consolidated rdent copy has read
            # cols 0:4 (~right after the last PV), earlier than any scale
            emit_out_op(3, 7, bank=pv[3])

    nc.finalize()
    return nc


def _get_program():
    global _prog
    if _prog is None:
        _prog = _build_program()
    return _prog


def kernel(x, W_q, W_k, W_v, W_o):
    import ml_dtypes
    from concourse.bass_utils import run_bass_kernel_spmd

    bf = ml_dtypes.bfloat16
    nc = _get_program()

    x = np.asarray(x, dtype=np.float32)
    scale = np.float32(1.0 / np.sqrt(D_HEAD))
    wq_s = np.asarray(W_q, dtype=np.float32) * scale
    wkv = np.concatenate(
        [np.asarray(W_k, dtype=np.float32), np.asarray(W_v, dtype=np.float32)],
        axis=1,
    )  # [1024, 128]
    wq_part = wq_s.reshape(DCH, P, 64).transpose(1, 0, 2).reshape(P, 512)
    wkv_part = wkv.reshape(DCH, P, 128).transpose(1, 0, 2).reshape(P, 1024)
    # per-role exp bias for partner-band chunks: -30 kills the whole chunk
    # for role 0 (exp(s - 30) ~ 1e-10), 0 keeps it whole for role 1
    w_hosts = []
    for r in range(2):
        rb = np.full((P, 4), 0.0 if r == 1 else -30.0, dtype=np.float32)
        w_hosts.append(
            np.ascontiguousarray(
                np.concatenate([wq_part, wkv_part, rb], axis=1)
            ).astype(bf)
        )  # [128, 1540]
    wo_host = np.ascontiguousarray(np.asarray(W_o, dtype=np.float32)).astype(bf)

    in_maps = []
    for c in range(NCORES):
        b, r = c // 2, c % 2
        xt_b = x[b].T  # [1024, 4096]
        # permuted key order: position 2k holds this core's query block
        # 2k+r, position 2k+1 holds the partner block 2k+(1-r)
        cols = np.concatenate(
            [
                np.arange(512 * g, 512 * g + 512)
                for k in range(NSLOT)
                for g in (2 * k + r, 2 * k + 1 - r)
            ]
        )
        xt_host = np.ascontiguousarray(
            xt_b[:, cols].reshape(DCH, P, SEQ).transpose(1, 2, 0)
        ).astype(bf)  # [128, 4096, 8]
        in_maps.append(
            {
                "xt": xt_host,
                "w": w_hosts[r],
                "wo": wo_host,
            }
        )

    res = run_bass_kernel_spmd(nc, in_maps, core_ids=list(range(NCORES)))
    out = np.empty((BATCH, SEQ, D_MODEL), dtype=np.float32)
    for c in range(NCORES):
        b, r = c // 2, c % 2
        yv = np.asarray(res.results[c]["y"]).astype(np.float32)
        # y[j, p, t, :] -> query 512*(2j+r) + 128t + p
        yv = yv.transpose(0, 2, 1, 3)  # [j, t, p, m]
        for j in range(NSLOT):
            q0 = 512 * (2 * j + r)
            out[b, q0 : q0 + 512, :] = yv[j].reshape(512, D_MODEL)
    return out



# revision 98
# speedup vs baseline: 1.0495x; 1.0017x over previous
"""Trainium2 Bass kernel for single-head causal attention.

x:[4,4096,1024] f32, W_q/W_k/W_v:[1024,64], W_o:[64,1024].

Sharding: 8 cores = 4 batches x 2 query-stripe roles. Role r of a batch
owns query blocks {2j+r : j=0..3} (512 queries each). Program slot j has
key extent E[j] = (8j+8) 128-key chunks, which exactly covers role 1's
block 2j+1 and over-covers role 0's block 2j by 4 chunks (dead).

All per-core differences (which batch, which stripe, dead chunks) are
carried in the input data; one SPMD program runs on all 8 cores:
- x is shipped as [128, 4096, 8] (d_model-chunk partition, seq, chunk)
  so any column range is a fully contiguous DMA (no sub-512B-element
  descriptor penalty), streamed in arrival-ordered bites.
- The diagonal causal band masks (4 x [128,512]) are generated on the
  otherwise-idle Pool engine with affine_select, not DMAed.
- Partner-block chunks (band >= 4) take no mask op at all: the exp runs
  with a per-partition bias from the w tensor (-30 for role 0 => p ~
  1e-10, 0 for role 1), so role-0's dead chunks vanish from both the PV
  numerator and the denominator row.

bf16 everywhere on the matmul paths (end-to-end rel err ~5e-3 vs the
2e-2 gate). Engines: PE does all matmuls (62.3us busy, the roofline of
this schedule), ACT does exp + the slot-3 ot copies + half its output
scales, DVE does diag masks + kvt/qt copies + finalize + the other
scales. Attention chunks are emitted in waves matched to DMA arrival,
next wave's KV projection interleaved between chunks as PE filler, PV
matmuls lagged 10 chunks behind their scores. Waves are front-loaded
(each chunk sits in the earliest wave whose x data can feed it) with
the KV projection emitted before the Q projection inside each wave's
filler — they share the single mm PSUM bank and Q may wait on
later-arriving x, so allocated first it would block the KV chain via
WAR. Every slot finalize is consolidated into full-width ops (one
[64,512] ot copy, one [1,512] reciprocal, four 1-col rdent transposes,
one [P,4] rdent copy, reading the PV PSUM bank directly) so all eight
out-projection matmuls of a slot unblock at once; tail out-ops get
distinct PSUM banks and the y DMAs go out per 128-query tile to keep
the final HWDGE descriptor-gen chain (625ns each, serialized) short.
"""

import sys

for _p in ("/opt/trn_rl_repo",):
    if _p not in sys.path:
        sys.path.insert(0, _p)

import numpy as np

D_MODEL = 1024
D_HEAD = 64
SEQ = 4096
BATCH = 4
NCORES = 8
NQ = 2048          # queries per core
P = 128
DCH = D_MODEL // P  # 8 contraction chunks
NSLOT = 4           # query slots of 512
E = [8, 16, 24, 32]  # key chunks per slot
NWAVE = 8           # key superchunks of 512

# Attention chunks per wave: matched to DMA arrival order (early waves
# light), per-slot ascending kc, every chunk (j,kc) in wave >= kc//4.
WAVES = [
    [(0, k) for k in range(4)],
    [(0, k) for k in range(4, 8)] + [(1, k) for k in range(8)],
    [(1, k) for k in range(8, 12)] + [(2, k) for k in range(8)],
    [(1, k) for k in range(12, 16)] + [(2, k) for k in range(8, 12)]
    + [(3, k) for k in range(4)],
    [(2, k) for k in range(12, 16)] + [(3, k) for k in range(4, 12)],
    [(2, k) for k in range(16, 24)],
    [(3, k) for k in range(12, 16)],
    [(3, k) for k in range(16, 32)],
]

_prog = None


def _check_waves():
    seen = {}
    total = 0
    for w, wv in enumerate(WAVES):
        for j, kc in wv:
            assert kc // 4 <= w, (w, j, kc)
            assert seen.get(j, -1) == kc - 1, (j, kc)
            seen[j] = kc
            total += 1
    assert total == sum(E) == 80
    return {j: max(w for w, wv in enumerate(WAVES) if (j, E[j] - 1) in wv)
            for j in range(NSLOT)}


def _build_program():
    import concourse.bacc as bacc
    import concourse.mybir as mybir
    import concourse.tile as tile
    from concourse.masks import make_identity

    fp32 = mybir.dt.float32
    f32r = mybir.dt.float32r
    bf16 = mybir.dt.bfloat16
    nc = bacc.Bacc("TRN2", target_bir_lowering=False, debug=False)

    # w layout: [wq 0:512 | wkv 512:1536 | rbias 1536:1540]
    # rbias: -30 for role 0, 0 for role 1. Partner-band chunks (band >= 4)
    # run exp with this per-partition bias instead of a 0/1 mask multiply:
    # each slot's band region only ever covers its own partner block, which
    # is entirely dead for role 0 (exp(s - 30) ~ 1e-10) and entirely alive
    # for role 1 (bias 0). Replaces the 16 explicit partner-mask DVE ops.
    xt = nc.dram_tensor("xt", [P, SEQ, DCH], bf16, kind="ExternalInput")
    w = nc.dram_tensor("w", [P, DCH * 192 + 4], bf16, kind="ExternalInput")
    wo = nc.dram_tensor("wo", [D_HEAD, D_MODEL], bf16, kind="ExternalInput")
    y = nc.dram_tensor("y", [NSLOT, P, 4, D_MODEL], bf16, kind="ExternalOutput")

    last_wave = _check_waves()
    # output-projection ops (j, i): slot0 -> waves 2,3; slot1 -> 5,6;
    # slot2 -> wave 6 (where DVE is light: slot-3 chunks kc<24 need no
    # masks); slot3 -> tail (wave index NWAVE)
    out_sched = {wi: [] for wi in range(NWAVE + 1)}
    for j, tgt in ((0, (2, 3)), (1, (5, 6)), (2, (6, 6)), (3, (8, 8))):
        for i in range(8):
            out_sched[tgt[i // 4]].append((j, i))

    with tile.TileContext(nc) as tc:
        with (
            tc.tile_pool(name="singles", bufs=1) as singles,
            tc.tile_pool(name="work", bufs=10) as work,
            tc.tile_pool(name="ypool", bufs=2) as ypool,
            tc.tile_pool(name="mm_ps", bufs=1, space="PSUM") as mm_ps,
            tc.tile_pool(name="s_ps", bufs=3, space="PSUM") as s_ps_pool,
            tc.tile_pool(name="pv_ps", bufs=1, space="PSUM") as pv_pool,
        ):
            # ---- persistent SBUF ----
            w_sb = singles.tile([P, DCH * 192 + 4], bf16, tag="w_sb")
            xt_sb = singles.tile([P, SEQ, DCH], bf16, tag="xt_sb")
            msk_sb = singles.tile([P, 4, 512], bf16, tag="msk_sb")
            wo_sb = singles.tile([D_HEAD, D_MODEL], bf16, tag="wo_sb")
            kvt = singles.tile([P, SEQ], bf16, tag="kvt")  # 0:64 K^T, 64:128 V^T
            qt_sb = singles.tile([D_HEAD, NQ], bf16, tag="qt_sb")
            vaug = singles.tile([P, 32, D_HEAD + 1], bf16, tag="vaug")
            ot = singles.tile([D_HEAD + 1, NQ], bf16, tag="ot")
            rden = singles.tile([1, NQ], fp32, tag="rden")
            rbias = singles.tile([P, 1], fp32, tag="rbias")
            rdent = singles.tile([P, 16], fp32, tag="rdent")
            ident = singles.tile([P, D_HEAD], bf16, tag="ident")
            one_sb = singles.tile([1, 1], fp32, tag="one_sb")

            # ---- input DMAs (SP queue, ordered by first use) ----
            def ld_x(dst, src, s0):
                nc.sync.dma_start(
                    out=dst[:, s0 : s0 + 512, :], in_=src[:, s0 : s0 + 512, :]
                )

            def ld_xh(dst, src, s0, n=512):
                nc.sync.dma_start(
                    out=dst[:, s0 : s0 + n, :], in_=src[:, s0 : s0 + n, :]
                )

            nc.sync.dma_start(out=w_sb[:, 0:512], in_=w[:, 0:512])
            ld_xh(xt_sb, xt, 0, 128)
            nc.sync.dma_start(out=w_sb[:, 512:1536], in_=w[:, 512:1536])
            ld_xh(xt_sb, xt, 128, 256)
            ld_xh(xt_sb, xt, 384, 128)
            nc.sync.dma_start(out=w_sb[:, 1536:1540], in_=w[:, 1536:1540])
            ld_x(xt_sb, xt, 512)       # chunk 1: wave-1 fresh keys
            ld_x(xt_sb, xt, 1024)      # chunk 2: slot-1 queries
            ld_x(xt_sb, xt, 2048)      # chunk 4: slot-2 queries
            nc.sync.dma_start(out=wo_sb, in_=wo[:, :])
            ld_x(xt_sb, xt, 1536)      # chunk 3
            ld_x(xt_sb, xt, 3072)      # chunk 6: slot-3 queries
            ld_x(xt_sb, xt, 2560)      # chunk 5
            ld_x(xt_sb, xt, 3584)      # chunk 7

            nc.vector.memset(one_sb, 1.0)
            nc.vector.memset(vaug[:, :, D_HEAD : D_HEAD + 1], 1.0)
            make_identity(nc, ident[D_HEAD:P, :])
            # fp32 per-partition exp bias (see w layout note)
            nc.vector.tensor_copy(out=rbias, in_=w_sb[:, 1536:1537])
            # causal band masks generated on the idle Pool engine:
            # msk_sb[p, c, jq] = 1 if 128c + p <= jq else 0, per band c
            for c in range(4):
                nc.gpsimd.memset(msk_sb[:, c, :], 1.0)
                nc.gpsimd.affine_select(
                    out=msk_sb[:, c, :],
                    in_=msk_sb[:, c, :],
                    compare_op=mybir.AluOpType.is_ge,
                    fill=0.0,
                    base=-128 * c,
                    channel_multiplier=-1,
                    pattern=[[1, 512]],
                )

            def kv_proj_mms(sc):
                """Generator: one KV-projection matmul per next() call."""
                kp = mm_ps.tile([P, 512], fp32, tag="mm")
                for dc in range(DCH):
                    nc.tensor.matmul(
                        kp,
                        lhsT=w_sb[:, 512 + dc * 128 : 512 + dc * 128 + 128],
                        rhs=xt_sb[:, sc * 512 : (sc + 1) * 512, dc],
                        start=(dc == 0),
                        stop=(dc == DCH - 1),
                    )
                    yield
                nc.vector.tensor_copy(
                    out=kvt[:, sc * 512 : (sc + 1) * 512], in_=kp
                )
                yield

            def transposes(sc):
                for t in range(4):  # V^T 128-col blocks -> natural V chunks
                    kc = sc * 4 + t
                    tp = s_ps_pool.tile([P, D_HEAD], bf16, tag="s_ps")
                    nc.tensor.transpose(
                        tp,
                        kvt[D_HEAD:P, kc * P : (kc + 1) * P],
                        ident[D_HEAD:P, :],
                    )
                    nc.vector.tensor_copy(out=vaug[:, kc, :D_HEAD], in_=tp)

            # PV accumulators: full-bank tiles, PV uses rows 0:65
            pv = [
                pv_pool.tile([P, 512], fp32, tag=f"pv{g}", name=f"pv{g}")
                for g in range(NSLOT)
            ]
            freed = []  # pv banks released by finalized slots
            y_tiles = {}
            ncopy = [0]
            pending_pv = []

            def emit_pv(j, kc):
                nc.tensor.matmul(
                    pv[j][0 : D_HEAD + 1, :],
                    lhsT=vaug[:, kc, :],
                    rhs=pending_pv_pt.pop((j, kc)),
                    start=(kc == 0),
                    stop=(kc == E[j] - 1),
                    skip_group_check=True,
                )

            pending_pv_pt = {}

            def emit_chunk(j, kc, wv=0):
                sps = s_ps_pool.tile([P, 512], fp32, tag="s_ps")
                nc.tensor.matmul(
                    sps,
                    lhsT=kvt[0:D_HEAD, kc * P : (kc + 1) * P],
                    rhs=qt_sb[:, j * 512 : (j + 1) * 512],
                    start=True,
                    stop=True,
                )
                p_t = work.tile([P, 512], bf16, tag="p_t")
                band = kc - (E[j] - 8)
                if band >= 4:
                    # partner block: role-0 kills the whole chunk via the
                    # exp bias (exp(s - 30) ~ 1e-10); role 1 keeps it whole
                    nc.scalar.activation(
                        p_t, sps, mybir.ActivationFunctionType.Exp,
                        bias=rbias[:, 0:1],
                    )
                else:
                    nc.scalar.activation(
                        p_t, sps, mybir.ActivationFunctionType.Exp
                    )
                if 0 <= band < 4:
                    # diagonal band: per-query causal step mask
                    nc.vector.tensor_tensor(
                        p_t, p_t, msk_sb[:, band, :], mybir.AluOpType.mult
                    )
                pending_pv_pt[(j, kc)] = p_t
                pending_pv.append((j, kc))
                if len(pending_pv) > 10:
                    emit_pv(*pending_pv.pop(0))

            def emit_out_op(j, i, bank=None):
                """One output-projection matmul + scaled PSUM->SBUF copy."""
                t, no = i // 2, i % 2
                if bank is None:
                    bank = freed[ncopy[0] % len(freed)]
                    ncopy[0] += 1
                q0 = j * 512 + t * P
                nc.tensor.matmul(
                    bank,
                    lhsT=ot[0:D_HEAD, q0 : q0 + P],
                    rhs=wo_sb[:, no * 512 : (no + 1) * 512],
                    start=True,
                    stop=True,
                )
                if j == 3 and i % 2 == 0:
                    nc.scalar.mul(
                        y_tiles[j][:, t, no * 512 : (no + 1) * 512],
                        bank,
                        rdent[:, 4 * j + t : 4 * j + t + 1],
                    )
                else:
                    nc.vector.tensor_scalar_mul(
                        y_tiles[j][:, t, no * 512 : (no + 1) * 512],
                        bank,
                        rdent[:, 4 * j + t : 4 * j + t + 1],
                    )
                if j == 3:
                    if i % 2 == 1:
                        # per-tile DMA: few enough that HWDGE desc-gen
                        # (625ns each, serialized) stays off the tail path
                        nc.sync.dma_start(
                            out=y[j][:, t : t + 1, :],
                            in_=y_tiles[j][:, t : t + 1, :],
                        )
                elif i == 3:
                    nc.sync.dma_start(
                        out=y[j][:, 0:2, :], in_=y_tiles[j][:, 0:2, :]
                    )
                elif i == 7:
                    nc.sync.dma_start(
                        out=y[j][:, 2:4, :], in_=y_tiles[j][:, 2:4, :]
                    )

            def finalize_half(j, h):
                """Half of slot-j finalize: O^T + den, 1/den, rdent cols."""
                c0 = j * 512 + h * 256
                c1 = c0 + 256
                nc.vector.tensor_copy(
                    out=ot[:, c0:c1], in_=pv[j][0 : D_HEAD + 1, h * 256 : h * 256 + 256]
                )
                nc.vector.reciprocal(
                    rden[:, c0:c1], ot[D_HEAD : D_HEAD + 1, c0:c1]
                )
                for t in (2 * h, 2 * h + 1):
                    nc.tensor.matmul(
                        pv[j][:, t : t + 1],
                        lhsT=rden[:, j * 512 + t * P : j * 512 + (t + 1) * P],
                        rhs=one_sb,
                        start=True,
                        stop=True,
                    )
                nc.vector.tensor_copy(
                    out=rdent[:, 4 * j + 2 * h : 4 * j + 2 * h + 2],
                    in_=pv[j][:, 2 * h : 2 * h + 2],
                )

            def finalize(j):
                # consolidated (same shape as the slot-3 tail): full-width
                # ot copy + reciprocal reading the pv PSUM bank directly,
                # then the four rdent transposes and one rdent copy
                nc.vector.tensor_copy(
                    out=ot[0:D_HEAD, j * 512 : (j + 1) * 512],
                    in_=pv[j][0:D_HEAD, :],
                )
                nc.vector.reciprocal(
                    rden[:, j * 512 : (j + 1) * 512],
                    pv[j][D_HEAD : D_HEAD + 1, :],
                )
                for t in range(4):
                    nc.tensor.matmul(
                        pv[j][:, t : t + 1],
                        lhsT=rden[:, j * 512 + t * P : j * 512 + (t + 1) * P],
                        rhs=one_sb,
                        start=True,
                        stop=True,
                    )
                nc.vector.tensor_copy(
                    out=rdent[:, 4 * j : 4 * j + 4], in_=pv[j][:, 0:4]
                )
                freed.append(pv[j])
                y_tiles[j] = ypool.tile(
                    [P, 4, D_MODEL], bf16, tag="y_sb", name=f"y{j}"
                )

            def q_piece(c0, n):
                """Prologue Q projection over columns [c0, c0+n) of slot 0.
                Uses the s_ps pool so pieces rotate PSUM banks instead of
                serializing on the single mm bank."""
                qp = s_ps_pool.tile([D_HEAD, n], fp32, tag="s_ps", name=f"qp{c0}")
                for dc in range(DCH):
                    nc.tensor.matmul(
                        qp,
                        lhsT=w_sb[:, dc * 64 : dc * 64 + 64],
                        rhs=xt_sb[:, c0 : c0 + n, dc],
                        start=(dc == 0),
                        stop=(dc == DCH - 1),
                    )
                nc.vector.tensor_copy(out=qt_sb[:, c0 : c0 + n], in_=qp)

            def kv_piece(c0, n):
                kp = s_ps_pool.tile([P, n], fp32, tag="s_ps", name=f"kp{c0}")
                for dc in range(DCH):
                    nc.tensor.matmul(
                        kp,
                        lhsT=w_sb[:, 512 + dc * 128 : 512 + dc * 128 + 128],
                        rhs=xt_sb[:, c0 : c0 + n, dc],
                        start=(dc == 0),
                        stop=(dc == DCH - 1),
                    )
                nc.vector.tensor_copy(out=kvt[:, c0 : c0 + n], in_=kp)

            def q_proj_steps(j):
                """Generator version of q_proj: one matmul per next()."""
                qp = mm_ps.tile([D_HEAD, 512], fp32, tag="mm")
                for dc in range(DCH):
                    nc.tensor.matmul(
                        qp,
                        lhsT=w_sb[:, dc * 64 : dc * 64 + 64],
                        rhs=xt_sb[:, j * 1024 : j * 1024 + 512, dc],
                        start=(dc == 0),
                        stop=(dc == DCH - 1),
                    )
                    yield
                nc.vector.tensor_copy(
                    out=qt_sb[:, j * 512 : (j + 1) * 512], in_=qp
                )
                yield

            def transpose_steps(sc):
                tp = s_ps_pool.tile([P, 4, D_HEAD], bf16, tag="s_ps", name="tp4")
                for t in range(4):
                    kc = sc * 4 + t
                    nc.tensor.transpose(
                        tp[:, t, :],
                        kvt[D_HEAD:P, kc * P : (kc + 1) * P],
                        ident[D_HEAD:P, :],
                    )
                    yield
                nc.vector.tensor_copy(
                    out=vaug[:, sc * 4 : sc * 4 + 4, :D_HEAD], in_=tp
                )
                yield

            # Q projection for slot j runs as filler inside wave j (its
            # first consumers are that wave's chunks)
            qproj_wave = {1: 1, 2: 2, 3: 3}

            # ---- prologue + wave 0: piece-width projections matched to the
            # DMA bite arrival order (x cols 0:128, 128:384, 384:512). The
            # V-transposes for kc 0..1 must be emitted before chunk (0,3)
            # pops PV(0,0), else the vaug dependency is never recorded. ----
            tg0 = transpose_steps(0)
            q_piece(0, 128)
            kv_piece(0, 128)
            q_piece(128, 256)
            kv_piece(128, 256)
            q_piece(384, 128)
            kv_piece(384, 128)
            next(tg0)
            next(tg0)
            emit_chunk(0, 0, 0)
            emit_chunk(0, 1, 0)
            next(tg0)
            next(tg0)
            emit_chunk(0, 2, 0)
            emit_chunk(0, 3, 0)
            for _ in tg0:  # drain: emits the packed vaug copy for kc 0..3
                pass

            # ---- streamed waves ----
            # Wave sc fillers: [qproj (if due), KV proj for sc, spacer,
            # V transposes for sc], popped two per chunk. Wave 0's proj and
            # transposes ran in the prologue.
            for sc in range(1, NWAVE):
                chunks = WAVES[sc]
                oo = list(out_sched[sc])
                nq = 9 if sc in qproj_wave else 0
                # kv projection FIRST: both share the single mm PSUM bank,
                # and the q projection may wait on later-arriving x columns
                # — allocated first it would block the kv chain via WAR
                filler = [
                    kv_proj_mms(sc),
                    iter([None, None]),  # spacer: kvt copy drains
                    transpose_steps(sc),
                ]
                if nq:
                    filler.append(q_proj_steps(qproj_wave[sc]))
                total_fill = nq + 16
                fill_iter = (x for g in filler for x in g)
                pops = [0]

                def pop_fill(upto=None, k=None):
                    tgt = upto if upto is not None else pops[0] + k
                    while pops[0] < min(tgt, total_fill):
                        if next(fill_iter, -1) == -1:
                            pops[0] = total_fill
                            break
                        pops[0] += 1

                # emission-order safety points: a fresh chunk's scores need
                # this wave's kvt copy emitted; a fresh chunk's PV needs its
                # V-transpose emitted
                n = len(chunks)
                for ci, (j, kc) in enumerate(chunks):
                    if kc // 4 == sc:
                        pop_fill(upto=9)
                    if sc in qproj_wave and j == qproj_wave[sc]:
                        pop_fill(upto=total_fill)
                    if len(pending_pv) >= 4:
                        j2, kc2 = pending_pv[0]
                        if kc2 // 4 == sc:
                            pop_fill(upto=total_fill)
                    emit_chunk(j, kc, sc)
                    if oo and ci % 2 == 1:
                        emit_out_op(*oo.pop(0))
                    k = -(-(total_fill - pops[0]) // (n - ci))  # ceil
                    pop_fill(k=min(k, 4))
                pop_fill(upto=total_fill)
                for op in oo:
                    emit_out_op(*op)
                fin = [j for j in range(NSLOT) if last_wave[j] == sc and j != 3]
                if fin:
                    while pending_pv:
                        emit_pv(*pending_pv.pop(0))
                    for j in fin:
                        finalize(j)

            # ---- slot-3 tail: per-128-query-tile pipeline. The ot copy and
            # the reciprocal both read the pv PSUM bank directly (no serial
            # copy->recip dependency), then rdent -> out-proj x2 -> scaled
            # copy (DVE half, ACT half) -> per-tile y DMA, so successive
            # tiles overlap across engines. ----
            while pending_pv:
                emit_pv(*pending_pv.pop(0))
            y_tiles[3] = ypool.tile([P, 4, D_MODEL], bf16, tag="y_sb", name="y3")

            def finalize_q(j, t):
                """Per-128-query finalize: reciprocal reads the pv PSUM
                bank directly; the ot copy is one full-width ACT op emitted
                by the caller."""
                c0 = j * 512 + t * P
                nc.vector.reciprocal(
                    rden[:, c0 : c0 + P],
                    pv[j][D_HEAD : D_HEAD + 1, t * P : (t + 1) * P],
                )
                nc.tensor.matmul(
                    pv[j][:, t : t + 1],
                    lhsT=rden[:, c0 : c0 + P],
                    rhs=one_sb,
                    start=True,
                    stop=True,
                )
                nc.vector.tensor_copy(
                    out=rdent[:, 4 * j + t : 4 * j + t + 1],
                    in_=pv[j][:, t : t + 1],
                )

            # emission order keeps each engine FIFO unblocked: the recip
            # chains for tiles 0-2 all precede the first scale op. Each
            # out-op gets its own PSUM bank (score + mm banks are idle by
            # now) so no matmul waits on a previous scale's drain.
            tail_banks = [
                s_ps_pool.tile([P, 512], fp32, tag="s_ps", name=f"tb{k}")
                for k in range(3)
            ] + [mm_ps.tile([P, 512], fp32, tag="mm", name="tb3")]
            nc.vector.tensor_copy(
                out=ot[0:D_HEAD, 1536:2048], in_=pv[3][0:D_HEAD, :]
            )
            nc.vector.reciprocal(
                rden[:, 1536:2048], pv[3][D_HEAD : D_HEAD + 1, :]
            )
            for t in range(4):
                nc.tensor.matmul(
                    pv[3][:, t : t + 1],
                    lhsT=rden[:, 1536 + t * P : 1536 + (t + 1) * P],
                    rhs=one_sb,
                    start=True,
                    stop=True,
                )
            nc.vector.tensor_copy(out=rdent[:, 12:16], in_=pv[3][:, 0:4])
            emit_out_op(3, 0, bank=tail_banks[0])
            emit_out_op(3, 1, bank=tail_banks[1])
            emit_out_op(3, 2, bank=tail_banks[2])
            emit_out_op(3, 3, bank=tail_banks[3])
            emit_out_op(3, 4, bank=pv[0])
            emit_out_op(3, 5, bank=pv[1])
            emit_out_op(3, 6, bank=pv[2])
            # pv[3] frees as soon as the consolidated rdent copy has read
            # cols 0:4 (~right after the last PV), earlier than any scale
            emit_out_op(3, 7, bank=pv[3])

    nc.finalize()
    return nc


def _get_program():
    global _prog
    if _prog is None:
        _prog = _build_program()
    return _prog


def kernel(x, W_q, W_k, W_v, W_o):
    import ml_dtypes
    from concourse.bass_utils import run_bass_kernel_spmd

    bf = ml_dtypes.bfloat16
    nc = _get_program()

    x = np.asarray(x, dtype=np.float32)
    scale = np.float32(1.0 / np.sqrt(D_HEAD))
    wq_s = np.asarray(W_q, dtype=np.float32) * scale
    wkv = np.concatenate(
        [np.asarray(W_k, dtype=np.float32), np.asarray(W_v, dtype=np.float32)],
        axis=1,
    )  # [1024, 128]
    wq_part = wq_s.reshape(DCH, P, 64).transpose(1, 0, 2).reshape(P, 512)
    wkv_part = wkv.reshape(DCH, P, 128).transpose(1, 0, 2).reshape(P, 1024)
    # per-role exp bias for partner-band chunks: -30 kills the whole chunk
    # for role 0 (exp(s - 30) ~ 1e-10), 0 keeps it whole for role 1
    w_hosts = []
    for r in range(2):
        rb = np.full((P, 4), 0.0 if r == 1 else -30.0, dtype=np.float32)
        w_hosts.append(
            np.ascontiguousarray(
                np.concatenate([wq_part, wkv_part, rb], axis=1)
            ).astype(bf)
        )  # [128, 1540]
    wo_host = np.ascontiguousarray(np.asarray(W_o, dtype=np.float32)).astype(bf)

    in_maps = []
    for c in range(NCORES):
        b, r = c // 2, c % 2
        xt_b = x[b].T  # [1024, 4096]
        # permuted key order: position 2k holds this core's query block
        # 2k+r, position 2k+1 holds the partner block 2k+(1-r)
        cols = np.concatenate(
            [
                np.arange(512 * g, 512 * g + 512)
                for k in range(NSLOT)
                for g in (2 * k + r, 2 * k + 1 - r)
            ]
        )
        xt_host = np.ascontiguousarray(
            xt_b[:, cols].reshape(DCH, P, SEQ).transpose(1, 2, 0)
        ).astype(bf)  # [128, 4096, 8]
        in_maps.append(
            {
                "xt": xt_host,
                "w": w_hosts[r],
                "wo": wo_host,
            }
        )

    res = run_bass_kernel_spmd(nc, in_maps, core_ids=list(range(NCORES)))
    out = np.empty((BATCH, SEQ, D_MODEL), dtype=np.float32)
    for c in range(NCORES):
        b, r = c // 2, c % 2
        yv = np.asarray(res.results[c]["y"]).astype(np.float32)
        # y[j, p, t, :] -> query 512*(2j+r) + 128t + p
        yv = yv.transpose(0, 2, 1, 3)  # [j, t, p, m]
        for j in range(NSLOT):
            q0 = 512 * (2 * j + r)
            out[b, q0 : q0 + 512, :] = yv[j].reshape(512, D_MODEL)
    return out



# revision 103
# speedup vs baseline: 1.0510x; 1.0015x over previous
"""Trainium2 Bass kernel for single-head causal attention.

x:[4,4096,1024] f32, W_q/W_k/W_v:[1024,64], W_o:[64,1024].

Sharding: 8 cores = 4 batches x 2 query-stripe roles. Role r of a batch
owns query blocks {2j+r : j=0..3} (512 queries each). Program slot j has
key extent E[j] = (8j+8) 128-key chunks, which exactly covers role 1's
block 2j+1 and over-covers role 0's block 2j by 4 chunks (dead).

All per-core differences (which batch, which stripe, dead chunks) are
carried in the input data; one SPMD program runs on all 8 cores:
- x is shipped as [128, 4096, 8] (d_model-chunk partition, seq, chunk)
  so any column range is a fully contiguous DMA (no sub-512B-element
  descriptor penalty), streamed in arrival-ordered bites.
- The diagonal causal band masks (4 x [128,512]) are generated on the
  otherwise-idle Pool engine with affine_select, not DMAed.
- Partner-block chunks (band >= 4) take no mask op at all: the exp runs
  with a per-partition bias from the w tensor (-30 for role 0 => p ~
  1e-10, 0 for role 1), so role-0's dead chunks vanish from both the PV
  numerator and the denominator row.

bf16 everywhere on the matmul paths (end-to-end rel err ~5e-3 vs the
2e-2 gate). Engines: PE does all matmuls (62.3us busy, the roofline of
this schedule), ACT does exp + the slot-3 ot copies + half its output
scales, DVE does diag masks + kvt/qt copies + finalize + the other
scales. Attention chunks are emitted in waves matched to DMA arrival,
next wave's KV projection interleaved between chunks as PE filler, PV
matmuls lagged 10 chunks behind their scores. Waves are front-loaded
(each chunk sits in the earliest wave whose x data can feed it) with
the KV projection emitted before the Q projection inside each wave's
filler — they share the single mm PSUM bank and Q may wait on
later-arriving x, so allocated first it would block the KV chain via
WAR. Every slot finalize is consolidated into full-width ops (one
[64,512] ot copy, one [1,512] reciprocal, four 1-col rdent transposes,
one [P,4] rdent copy, reading the PV PSUM bank directly) so all eight
out-projection matmuls of a slot unblock at once; tail out-ops get
distinct PSUM banks and the y DMAs go out per 128-query tile to keep
the final HWDGE descriptor-gen chain (625ns each, serialized) short.
"""

import sys

for _p in ("/opt/trn_rl_repo",):
    if _p not in sys.path:
        sys.path.insert(0, _p)

import numpy as np

D_MODEL = 1024
D_HEAD = 64
SEQ = 4096
BATCH = 4
NCORES = 8
NQ = 2048          # queries per core
P = 128
DCH = D_MODEL // P  # 8 contraction chunks
NSLOT = 4           # query slots of 512
E = [8, 16, 24, 32]  # key chunks per slot
NWAVE = 8           # key superchunks of 512

# Attention chunks per wave: matched to DMA arrival order (early waves
# light), per-slot ascending kc, every chunk (j,kc) in wave >= kc//4.
WAVES = [
    [(0, k) for k in range(4)],
    [(0, k) for k in range(4, 8)] + [(1, k) for k in range(8)],
    [(1, k) for k in range(8, 12)] + [(2, k) for k in range(12)],
    [(1, k) for k in range(12, 16)] + [(3, k) for k in range(4)],
    [(2, k) for k in range(12, 16)] + [(3, k) for k in range(4, 12)],
    [(2, k) for k in range(16, 24)],
    [(3, k) for k in range(12, 16)],
    [(3, k) for k in range(16, 32)],
]

_prog = None


def _check_waves():
    seen = {}
    total = 0
    for w, wv in enumerate(WAVES):
        for j, kc in wv:
            assert kc // 4 <= w, (w, j, kc)
            assert seen.get(j, -1) == kc - 1, (j, kc)
            seen[j] = kc
            total += 1
    assert total == sum(E) == 80
    return {j: max(w for w, wv in enumerate(WAVES) if (j, E[j] - 1) in wv)
            for j in range(NSLOT)}


def _build_program():
    import concourse.bacc as bacc
    import concourse.mybir as mybir
    import concourse.tile as tile
    from concourse.masks import make_identity

    fp32 = mybir.dt.float32
    f32r = mybir.dt.float32r
    bf16 = mybir.dt.bfloat16
    nc = bacc.Bacc("TRN2", target_bir_lowering=False, debug=False)

    # w layout: [wq 0:512 | wkv 512:1536 | rbias 1536:1540]
    # rbias: -30 for role 0, 0 for role 1. Partner-band chunks (band >= 4)
    # run exp with this per-partition bias instead of a 0/1 mask multiply:
    # each slot's band region only ever covers its own partner block, which
    # is entirely dead for role 0 (exp(s - 30) ~ 1e-10) and entirely alive
    # for role 1 (bias 0). Replaces the 16 explicit partner-mask DVE ops.
    xt = nc.dram_tensor("xt", [P, SEQ, DCH], bf16, kind="ExternalInput")
    w = nc.dram_tensor("w", [P, DCH * 192 + 4], bf16, kind="ExternalInput")
    wo = nc.dram_tensor("wo", [D_HEAD, D_MODEL], bf16, kind="ExternalInput")
    y = nc.dram_tensor("y", [NSLOT, P, 4, D_MODEL], bf16, kind="ExternalOutput")

    last_wave = _check_waves()
    # output-projection ops (j, i): slot0 -> waves 2,3; slot1 -> 5,6;
    # slot2 -> wave 6 (where DVE is light: slot-3 chunks kc<24 need no
    # masks); slot3 -> tail (wave index NWAVE)
    out_sched = {wi: [] for wi in range(NWAVE + 1)}
    for j, tgt in ((0, (2, 3)), (1, (5, 6)), (2, (6, 6)), (3, (8, 8))):
        for i in range(8):
            out_sched[tgt[i // 4]].append((j, i))

    with tile.TileContext(nc) as tc:
        with (
            tc.tile_pool(name="singles", bufs=1) as singles,
            tc.tile_pool(name="work", bufs=10) as work,
            tc.tile_pool(name="ypool", bufs=2) as ypool,
            tc.tile_pool(name="mm_ps", bufs=1, space="PSUM") as mm_ps,
            tc.tile_pool(name="s_ps", bufs=3, space="PSUM") as s_ps_pool,
            tc.tile_pool(name="pv_ps", bufs=1, space="PSUM") as pv_pool,
        ):
            # ---- persistent SBUF ----
            w_sb = singles.tile([P, DCH * 192 + 4], bf16, tag="w_sb")
            xt_sb = singles.tile([P, SEQ, DCH], bf16, tag="xt_sb")
            msk_sb = singles.tile([P, 4, 512], bf16, tag="msk_sb")
            wo_sb = singles.tile([D_HEAD, D_MODEL], bf16, tag="wo_sb")
            kvt = singles.tile([P, SEQ], bf16, tag="kvt")  # 0:64 K^T, 64:128 V^T
            qt_sb = singles.tile([D_HEAD, NQ], bf16, tag="qt_sb")
            vaug = singles.tile([P, 32, D_HEAD + 1], bf16, tag="vaug")
            ot = singles.tile([D_HEAD + 1, NQ], bf16, tag="ot")
            rden = singles.tile([1, NQ], fp32, tag="rden")
            rbias = singles.tile([P, 1], fp32, tag="rbias")
            rdent = singles.tile([P, 16], fp32, tag="rdent")
            ident = singles.tile([P, D_HEAD], bf16, tag="ident")
            one_sb = singles.tile([1, 1], fp32, tag="one_sb")

            # ---- input DMAs (SP queue, ordered by first use) ----
            def ld_x(dst, src, s0):
                nc.sync.dma_start(
                    out=dst[:, s0 : s0 + 512, :], in_=src[:, s0 : s0 + 512, :]
                )

            def ld_xh(dst, src, s0, n=512):
                nc.sync.dma_start(
                    out=dst[:, s0 : s0 + n, :], in_=src[:, s0 : s0 + n, :]
                )

            nc.sync.dma_start(out=w_sb[:, 0:512], in_=w[:, 0:512])
            ld_xh(xt_sb, xt, 0, 128)
            nc.sync.dma_start(out=w_sb[:, 512:1536], in_=w[:, 512:1536])
            ld_xh(xt_sb, xt, 128, 256)
            ld_xh(xt_sb, xt, 384, 128)
            nc.sync.dma_start(out=w_sb[:, 1536:1540], in_=w[:, 1536:1540])
            ld_x(xt_sb, xt, 512)       # chunk 1: wave-1 fresh keys
            ld_x(xt_sb, xt, 1024)      # chunk 2: slot-1 queries
            ld_x(xt_sb, xt, 2048)      # chunk 4: slot-2 queries
            nc.sync.dma_start(out=wo_sb, in_=wo[:, :])
            ld_x(xt_sb, xt, 1536)      # chunk 3
            ld_x(xt_sb, xt, 3072)      # chunk 6: slot-3 queries
            ld_x(xt_sb, xt, 2560)      # chunk 5
            ld_x(xt_sb, xt, 3584)      # chunk 7

            nc.vector.memset(one_sb, 1.0)
            nc.vector.memset(vaug[:, :, D_HEAD : D_HEAD + 1], 1.0)
            make_identity(nc, ident[D_HEAD:P, :])
            # fp32 per-partition exp bias (see w layout note)
            nc.vector.tensor_copy(out=rbias, in_=w_sb[:, 1536:1537])
            # causal band masks generated on the idle Pool engine:
            # msk_sb[p, c, jq] = 1 if 128c + p <= jq else 0, per band c
            for c in range(4):
                nc.gpsimd.memset(msk_sb[:, c, :], 1.0)
                nc.gpsimd.affine_select(
                    out=msk_sb[:, c, :],
                    in_=msk_sb[:, c, :],
                    compare_op=mybir.AluOpType.is_ge,
                    fill=0.0,
                    base=-128 * c,
                    channel_multiplier=-1,
                    pattern=[[1, 512]],
                )

            def kv_proj_mms(sc):
                """Generator: one KV-projection matmul per next() call."""
                kp = mm_ps.tile([P, 512], fp32, tag="mm")
                for dc in range(DCH):
                    nc.tensor.matmul(
                        kp,
                        lhsT=w_sb[:, 512 + dc * 128 : 512 + dc * 128 + 128],
                        rhs=xt_sb[:, sc * 512 : (sc + 1) * 512, dc],
                        start=(dc == 0),
                        stop=(dc == DCH - 1),
                    )
                    yield
                nc.vector.tensor_copy(
                    out=kvt[:, sc * 512 : (sc + 1) * 512], in_=kp
                )
                yield

            def transposes(sc):
                for t in range(4):  # V^T 128-col blocks -> natural V chunks
                    kc = sc * 4 + t
                    tp = s_ps_pool.tile([P, D_HEAD], bf16, tag="s_ps")
                    nc.tensor.transpose(
                        tp,
                        kvt[D_HEAD:P, kc * P : (kc + 1) * P],
                        ident[D_HEAD:P, :],
                    )
                    nc.vector.tensor_copy(out=vaug[:, kc, :D_HEAD], in_=tp)

            # PV accumulators: full-bank tiles, PV uses rows 0:65
            pv = [
                pv_pool.tile([P, 512], fp32, tag=f"pv{g}", name=f"pv{g}")
                for g in range(NSLOT)
            ]
            freed = []  # pv banks released by finalized slots
            y_tiles = {}
            ncopy = [0]
            pending_pv = []

            def emit_pv(j, kc):
                nc.tensor.matmul(
                    pv[j][0 : D_HEAD + 1, :],
                    lhsT=vaug[:, kc, :],
                    rhs=pending_pv_pt.pop((j, kc)),
                    start=(kc == 0),
                    stop=(kc == E[j] - 1),
                    skip_group_check=True,
                )

            pending_pv_pt = {}

            def emit_chunk(j, kc, wv=0):
                sps = s_ps_pool.tile([P, 512], fp32, tag="s_ps")
                nc.tensor.matmul(
                    sps,
                    lhsT=kvt[0:D_HEAD, kc * P : (kc + 1) * P],
                    rhs=qt_sb[:, j * 512 : (j + 1) * 512],
                    start=True,
                    stop=True,
                )
                p_t = work.tile([P, 512], bf16, tag="p_t")
                band = kc - (E[j] - 8)
                if band >= 4:
                    # partner block: role-0 kills the whole chunk via the
                    # exp bias (exp(s - 30) ~ 1e-10); role 1 keeps it whole
                    nc.scalar.activation(
                        p_t, sps, mybir.ActivationFunctionType.Exp,
                        bias=rbias[:, 0:1],
                    )
                else:
                    nc.scalar.activation(
                        p_t, sps, mybir.ActivationFunctionType.Exp
                    )
                if 0 <= band < 4:
                    # diagonal band: per-query causal step mask
                    nc.vector.tensor_tensor(
                        p_t, p_t, msk_sb[:, band, :], mybir.AluOpType.mult
                    )
                pending_pv_pt[(j, kc)] = p_t
                pending_pv.append((j, kc))
                if len(pending_pv) > 10:
                    emit_pv(*pending_pv.pop(0))

            def emit_out_op(j, i, bank=None):
                """One output-projection matmul + scaled PSUM->SBUF copy."""
                t, no = i // 2, i % 2
                if bank is None:
                    bank = freed[ncopy[0] % len(freed)]
                    ncopy[0] += 1
                q0 = j * 512 + t * P
                nc.tensor.matmul(
                    bank,
                    lhsT=ot[0:D_HEAD, q0 : q0 + P],
                    rhs=wo_sb[:, no * 512 : (no + 1) * 512],
                    start=True,
                    stop=True,
                )
                if j == 3 and i % 2 == 0:
                    nc.scalar.mul(
                        y_tiles[j][:, t, no * 512 : (no + 1) * 512],
                        bank,
                        rdent[:, 4 * j + t : 4 * j + t + 1],
                    )
                else:
                    nc.vector.tensor_scalar_mul(
                        y_tiles[j][:, t, no * 512 : (no + 1) * 512],
                        bank,
                        rdent[:, 4 * j + t : 4 * j + t + 1],
                    )
                if j == 3:
                    if i % 2 == 1:
                        # per-tile DMA: few enough that HWDGE desc-gen
                        # (625ns each, serialized) stays off the tail path
                        nc.sync.dma_start(
                            out=y[j][:, t : t + 1, :],
                            in_=y_tiles[j][:, t : t + 1, :],
                        )
                elif i == 3:
                    nc.sync.dma_start(
                        out=y[j][:, 0:2, :], in_=y_tiles[j][:, 0:2, :]
                    )
                elif i == 7:
                    nc.sync.dma_start(
                        out=y[j][:, 2:4, :], in_=y_tiles[j][:, 2:4, :]
                    )

            def finalize_half(j, h):
                """Half of slot-j finalize: O^T + den, 1/den, rdent cols."""
                c0 = j * 512 + h * 256
                c1 = c0 + 256
                nc.vector.tensor_copy(
                    out=ot[:, c0:c1], in_=pv[j][0 : D_HEAD + 1, h * 256 : h * 256 + 256]
                )
                nc.vector.reciprocal(
                    rden[:, c0:c1], ot[D_HEAD : D_HEAD + 1, c0:c1]
                )
                for t in (2 * h, 2 * h + 1):
                    nc.tensor.matmul(
                        pv[j][:, t : t + 1],
                        lhsT=rden[:, j * 512 + t * P : j * 512 + (t + 1) * P],
                        rhs=one_sb,
                        start=True,
                        stop=True,
                    )
                nc.vector.tensor_copy(
                    out=rdent[:, 4 * j + 2 * h : 4 * j + 2 * h + 2],
                    in_=pv[j][:, 2 * h : 2 * h + 2],
                )

            def finalize(j):
                # consolidated (same shape as the slot-3 tail): full-width
                # ot copy + reciprocal reading the pv PSUM bank directly,
                # then the four rdent transposes and one rdent copy
                nc.vector.tensor_copy(
                    out=ot[0:D_HEAD, j * 512 : (j + 1) * 512],
                    in_=pv[j][0:D_HEAD, :],
                )
                nc.vector.reciprocal(
                    rden[:, j * 512 : (j + 1) * 512],
                    pv[j][D_HEAD : D_HEAD + 1, :],
                )
                for t in range(4):
                    nc.tensor.matmul(
                        pv[j][:, t : t + 1],
                        lhsT=rden[:, j * 512 + t * P : j * 512 + (t + 1) * P],
                        rhs=one_sb,
                        start=True,
                        stop=True,
                    )
                nc.vector.tensor_copy(
                    out=rdent[:, 4 * j : 4 * j + 4], in_=pv[j][:, 0:4]
                )
                freed.append(pv[j])
                y_tiles[j] = ypool.tile(
                    [P, 4, D_MODEL], bf16, tag="y_sb", name=f"y{j}"
                )

            def q_piece(c0, n):
                """Prologue Q projection over columns [c0, c0+n) of slot 0.
                Uses the s_ps pool so pieces rotate PSUM banks instead of
                serializing on the single mm bank."""
                qp = s_ps_pool.tile([D_HEAD, n], fp32, tag="s_ps", name=f"qp{c0}")
                for dc in range(DCH):
                    nc.tensor.matmul(
                        qp,
                        lhsT=w_sb[:, dc * 64 : dc * 64 + 64],
                        rhs=xt_sb[:, c0 : c0 + n, dc],
                        start=(dc == 0),
                        stop=(dc == DCH - 1),
                    )
                nc.vector.tensor_copy(out=qt_sb[:, c0 : c0 + n], in_=qp)

            def kv_piece(c0, n):
                kp = s_ps_pool.tile([P, n], fp32, tag="s_ps", name=f"kp{c0}")
                for dc in range(DCH):
                    nc.tensor.matmul(
                        kp,
                        lhsT=w_sb[:, 512 + dc * 128 : 512 + dc * 128 + 128],
                        rhs=xt_sb[:, c0 : c0 + n, dc],
                        start=(dc == 0),
                        stop=(dc == DCH - 1),
                    )
                nc.vector.tensor_copy(out=kvt[:, c0 : c0 + n], in_=kp)

            def q_proj_steps(j):
                """Generator version of q_proj: one matmul per next()."""
                qp = mm_ps.tile([D_HEAD, 512], fp32, tag="mm")
                for dc in range(DCH):
                    nc.tensor.matmul(
                        qp,
                        lhsT=w_sb[:, dc * 64 : dc * 64 + 64],
                        rhs=xt_sb[:, j * 1024 : j * 1024 + 512, dc],
                        start=(dc == 0),
                        stop=(dc == DCH - 1),
                    )
                    yield
                nc.vector.tensor_copy(
                    out=qt_sb[:, j * 512 : (j + 1) * 512], in_=qp
                )
                yield

            def transpose_steps(sc):
                tp = s_ps_pool.tile([P, 4, D_HEAD], bf16, tag="s_ps", name="tp4")
                for t in range(4):
                    kc = sc * 4 + t
                    nc.tensor.transpose(
                        tp[:, t, :],
                        kvt[D_HEAD:P, kc * P : (kc + 1) * P],
                        ident[D_HEAD:P, :],
                    )
                    yield
                nc.vector.tensor_copy(
                    out=vaug[:, sc * 4 : sc * 4 + 4, :D_HEAD], in_=tp
                )
                yield

            # Q projection for slot j runs as filler inside wave j (its
            # first consumers are that wave's chunks)
            qproj_wave = {1: 1, 2: 2, 3: 3}

            # ---- prologue + wave 0: piece-width projections matched to the
            # DMA bite arrival order (x cols 0:128, 128:384, 384:512). The
            # V-transposes for kc 0..1 must be emitted before chunk (0,3)
            # pops PV(0,0), else the vaug dependency is never recorded. ----
            tg0 = transpose_steps(0)
            q_piece(0, 128)
            kv_piece(0, 128)
            q_piece(128, 256)
            kv_piece(128, 256)
            q_piece(384, 128)
            kv_piece(384, 128)
            next(tg0)
            next(tg0)
            emit_chunk(0, 0, 0)
            emit_chunk(0, 1, 0)
            next(tg0)
            next(tg0)
            emit_chunk(0, 2, 0)
            emit_chunk(0, 3, 0)
            for _ in tg0:  # drain: emits the packed vaug copy for kc 0..3
                pass

            # ---- streamed waves ----
            # Wave sc fillers: [qproj (if due), KV proj for sc, spacer,
            # V transposes for sc], popped two per chunk. Wave 0's proj and
            # transposes ran in the prologue.
            for sc in range(1, NWAVE):
                chunks = WAVES[sc]
                oo = list(out_sched[sc])
                nq = 9 if sc in qproj_wave else 0
                # kv projection FIRST: both share the single mm PSUM bank,
                # and the q projection may wait on later-arriving x columns
                # — allocated first it would block the kv chain via WAR
                filler = [
                    kv_proj_mms(sc),
                    iter([None, None]),  # spacer: kvt copy drains
                    transpose_steps(sc),
                ]
                if nq:
                    filler.append(q_proj_steps(qproj_wave[sc]))
                total_fill = nq + 16
                fill_iter = (x for g in filler for x in g)
                pops = [0]

                def pop_fill(upto=None, k=None):
                    tgt = upto if upto is not None else pops[0] + k
                    while pops[0] < min(tgt, total_fill):
                        if next(fill_iter, -1) == -1:
                            pops[0] = total_fill
                            break
                        pops[0] += 1

                # emission-order safety points: a fresh chunk's scores need
                # this wave's kvt copy emitted; a fresh chunk's PV needs its
                # V-transpose emitted
                n = len(chunks)
                for ci, (j, kc) in enumerate(chunks):
                    if kc // 4 == sc:
                        pop_fill(upto=9)
                    if sc in qproj_wave and j == qproj_wave[sc]:
                        pop_fill(upto=total_fill)
                    if len(pending_pv) >= 4:
                        j2, kc2 = pending_pv[0]
                        if kc2 // 4 == sc:
                            pop_fill(upto=total_fill)
                    emit_chunk(j, kc, sc)
                    if oo and ci % 2 == 1:
                        emit_out_op(*oo.pop(0))
                    k = -(-(total_fill - pops[0]) // (n - ci))  # ceil
                    pop_fill(k=min(k, 4))
                pop_fill(upto=total_fill)
                for op in oo:
                    emit_out_op(*op)
                fin = [j for j in range(NSLOT) if last_wave[j] == sc and j != 3]
                if fin:
                    while pending_pv:
                        emit_pv(*pending_pv.pop(0))
                    for j in fin:
                        finalize(j)

            # ---- slot-3 tail: per-128-query-tile pipeline. The ot copy and
            # the reciprocal both read the pv PSUM bank directly (no serial
            # copy->recip dependency), then rdent -> out-proj x2 -> scaled
            # copy (DVE half, ACT half) -> per-tile y DMA, so successive
            # tiles overlap across engines. ----
            while pending_pv:
                emit_pv(*pending_pv.pop(0))
            y_tiles[3] = ypool.tile([P, 4, D_MODEL], bf16, tag="y_sb", name="y3")

            def finalize_q(j, t):
                """Per-128-query finalize: reciprocal reads the pv PSUM
                bank directly; the ot copy is one full-width ACT op emitted
                by the caller."""
                c0 = j * 512 + t * P
                nc.vector.reciprocal(
                    rden[:, c0 : c0 + P],
                    pv[j][D_HEAD : D_HEAD + 1, t * P : (t + 1) * P],
                )
                nc.tensor.matmul(
                    pv[j][:, t : t + 1],
                    lhsT=rden[:, c0 : c0 + P],
                    rhs=one_sb,
                    start=True,
                    stop=True,
                )
                nc.vector.tensor_copy(
                    out=rdent[:, 4 * j + t : 4 * j + t + 1],
                    in_=pv[j][:, t : t + 1],
                )

            # emission order keeps each engine FIFO unblocked: the recip
            # chains for tiles 0-2 all precede the first scale op. Each
            # out-op gets its own PSUM bank (score + mm banks are idle by
            # now) so no matmul waits on a previous scale's drain.
            tail_banks = [
                s_ps_pool.tile([P, 512], fp32, tag="s_ps", name=f"tb{k}")
                for k in range(3)
            ] + [mm_ps.tile([P, 512], fp32, tag="mm", name="tb3")]
            nc.vector.tensor_copy(
                out=ot[0:D_HEAD, 1536:2048], in_=pv[3][0:D_HEAD, :]
            )
            nc.vector.reciprocal(
                rden[:, 1536:2048], pv[3][D_HEAD : D_HEAD + 1, :]
            )
            for t in range(4):
                nc.tensor.matmul(
                    pv[3][:, t : t + 1],
                    lhsT=rden[:, 1536 + t * P : 1536 + (t + 1) * P],
                    rhs=one_sb,
                    start=True,
                    stop=True,
                )
            nc.vector.tensor_copy(out=rdent[:, 12:16], in_=pv[3][:, 0:4])
            emit_out_op(3, 0, bank=tail_banks[0])
            emit_out_op(3, 1, bank=tail_banks[1])
            emit_out_op(3, 2, bank=tail_banks[2])
            emit_out_op(3, 3, bank=tail_banks[3])
            emit_out_op(3, 4, bank=pv[0])
            emit_out_op(3, 5, bank=pv[1])
            emit_out_op(3, 6, bank=pv[2])
            # pv[3] frees as soon as the consolidated rdent copy has read
            # cols 0:4 (~right after the last PV), earlier than any scale
            emit_out_op(3, 7, bank=pv[3])

    nc.finalize()
    return nc


def _get_program():
    global _prog
    if _prog is None:
        _prog = _build_program()
    return _prog


def kernel(x, W_q, W_k, W_v, W_o):
    import ml_dtypes
    from concourse.bass_utils import run_bass_kernel_spmd

    bf = ml_dtypes.bfloat16
    nc = _get_program()

    x = np.asarray(x, dtype=np.float32)
    scale = np.float32(1.0 / np.sqrt(D_HEAD))
    wq_s = np.asarray(W_q, dtype=np.float32) * scale
    wkv = np.concatenate(
        [np.asarray(W_k, dtype=np.float32), np.asarray(W_v, dtype=np.float32)],
        axis=1,
    )  # [1024, 128]
    wq_part = wq_s.reshape(DCH, P, 64).transpose(1, 0, 2).reshape(P, 512)
    wkv_part = wkv.reshape(DCH, P, 128).transpose(1, 0, 2).reshape(P, 1024)
    # per-role exp bias for partner-band chunks: -30 kills the whole chunk
    # for role 0 (exp(s - 30) ~ 1e-10), 0 keeps it whole for role 1
    w_hosts = []
    for r in range(2):
        rb = np.full((P, 4), 0.0 if r == 1 else -30.0, dtype=np.float32)
        w_hosts.append(
            np.ascontiguousarray(
                np.concatenate([wq_part, wkv_part, rb], axis=1)
            ).astype(bf)
        )  # [128, 1540]
    wo_host = np.ascontiguousarray(np.asarray(W_o, dtype=np.float32)).astype(bf)

    in_maps = []
    for c in range(NCORES):
        b, r = c // 2, c % 2
        xt_b = x[b].T  # [1024, 4096]
        # permuted key order: position 2k holds this core's query block
        # 2k+r, position 2k+1 holds the partner block 2k+(1-r)
        cols = np.concatenate(
            [
                np.arange(512 * g, 512 * g + 512)
                for k in range(NSLOT)
                for g in (2 * k + r, 2 * k + 1 - r)
            ]
        )
        xt_host = np.ascontiguousarray(
            xt_b[:, cols].reshape(DCH, P, SEQ).transpose(1, 2, 0)
        ).astype(bf)  # [128, 4096, 8]
        in_maps.append(
            {
                "xt": xt_host,
                "w": w_hosts[r],
                "wo": wo_host,
            }
        )

    res = run_bass_kernel_spmd(nc, in_maps, core_ids=list(range(NCORES)))
    out = np.empty((BATCH, SEQ, D_MODEL), dtype=np.float32)
    for c in range(NCORES):
        b, r = c // 2, c % 2
        yv = np.asarray(res.results[c]["y"]).astype(np.float32)
        # y[j, p, t, :] -> query 512*(2j+r) + 128t + p
        yv = yv.transpose(0, 2, 1, 3)  # [j, t, p, m]
        for j in range(NSLOT):
            q0 = 512 * (2 * j + r)
            out[b, q0 : q0 + 512, :] = yv[j].reshape(512, D_MODEL)
    return out



# revision 109
# speedup vs baseline: 1.0561x; 1.0048x over previous
"""Trainium2 Bass kernel for single-head causal attention.

x:[4,4096,1024] f32, W_q/W_k/W_v:[1024,64], W_o:[64,1024].

Sharding: 8 cores = 4 batches x 2 query-stripe roles. Role r of a batch
owns query blocks {2j+r : j=0..3} (512 queries each). Program slot j has
key extent E[j] = (8j+8) 128-key chunks, which exactly covers role 1's
block 2j+1 and over-covers role 0's block 2j by 4 chunks (dead).

All per-core differences (which batch, which stripe, dead chunks) are
carried in the input data; one SPMD program runs on all 8 cores:
- x is shipped as [128, 4096, 8] (d_model-chunk partition, seq, chunk)
  so any column range is a fully contiguous DMA (no sub-512B-element
  descriptor penalty), streamed in arrival-ordered bites.
- The diagonal causal band masks (4 x [128,512]) are generated on the
  otherwise-idle Pool engine with affine_select, not DMAed.
- Partner-block chunks (band >= 4) take no mask op at all: the exp runs
  with a per-partition bias from the w tensor (-30 for role 0 => p ~
  1e-10, 0 for role 1), so role-0's dead chunks vanish from both the PV
  numerator and the denominator row.

bf16 everywhere on the matmul paths (end-to-end rel err ~5e-3 vs the
2e-2 gate). Engines: PE does all matmuls (62.3us busy, the roofline of
this schedule), ACT does exp + the slot-3 ot copies + half its output
scales, DVE does diag masks + kvt/qt copies + finalize + the other
scales. Attention chunks are emitted in waves matched to DMA arrival,
next wave's KV projection interleaved between chunks as PE filler, PV
matmuls lagged 10 chunks behind their scores. Waves are front-loaded
(each chunk sits in the earliest wave whose x data can feed it) with
the KV projection emitted before the Q projection inside each wave's
filler — they share the single mm PSUM bank and Q may wait on
later-arriving x, so allocated first it would block the KV chain via
WAR. Every slot finalize is consolidated into full-width ops (one
[64,512] ot copy, one [1,512] reciprocal, four 1-col rdent transposes,
one [P,4] rdent copy, reading the PV PSUM bank directly) so all eight
out-projection matmuls of a slot unblock at once; tail out-ops get
distinct PSUM banks and the y DMAs go out per 128-query tile to keep
the final HWDGE descriptor-gen chain (625ns each, serialized) short.
"""

import sys

for _p in ("/opt/trn_rl_repo",):
    if _p not in sys.path:
        sys.path.insert(0, _p)

import numpy as np

D_MODEL = 1024
D_HEAD = 64
SEQ = 4096
BATCH = 4
NCORES = 8
NQ = 2048          # queries per core
P = 128
DCH = D_MODEL // P  # 8 contraction chunks
NSLOT = 4           # query slots of 512
E = [8, 16, 24, 32]  # key chunks per slot
NWAVE = 8           # key superchunks of 512

# Attention chunks per wave: matched to DMA arrival order (early waves
# light), per-slot ascending kc, every chunk (j,kc) in wave >= kc//4.
WAVES = [
    [(0, k) for k in range(4)],
    [(0, k) for k in range(4, 8)] + [(1, k) for k in range(8)],
    [(1, k) for k in range(8, 12)] + [(2, k) for k in range(12)],
    [(1, k) for k in range(12, 16)] + [(3, k) for k in range(4)],
    [(2, k) for k in range(12, 16)] + [(3, k) for k in range(4, 12)],
    [(2, k) for k in range(16, 24)],
    [(3, k) for k in range(12, 16)],
    [(3, k) for k in range(16, 32)],
]

_prog = None


def _check_waves():
    seen = {}
    total = 0
    for w, wv in enumerate(WAVES):
        for j, kc in wv:
            assert kc // 4 <= w, (w, j, kc)
            assert seen.get(j, -1) == kc - 1, (j, kc)
            seen[j] = kc
            total += 1
    assert total == sum(E) == 80
    return {j: max(w for w, wv in enumerate(WAVES) if (j, E[j] - 1) in wv)
            for j in range(NSLOT)}


def _build_program():
    import concourse.bacc as bacc
    import concourse.mybir as mybir
    import concourse.tile as tile
    from concourse.masks import make_identity

    fp32 = mybir.dt.float32
    f32r = mybir.dt.float32r
    bf16 = mybir.dt.bfloat16
    nc = bacc.Bacc("TRN2", target_bir_lowering=False, debug=False)

    # w layout: [wq 0:512 | wkv 512:1536 | rbias 1536:1540]
    # rbias: -30 for role 0, 0 for role 1. Partner-band chunks (band >= 4)
    # run exp with this per-partition bias instead of a 0/1 mask multiply:
    # each slot's band region only ever covers its own partner block, which
    # is entirely dead for role 0 (exp(s - 30) ~ 1e-10) and entirely alive
    # for role 1 (bias 0). Replaces the 16 explicit partner-mask DVE ops.
    xt = nc.dram_tensor("xt", [P, SEQ, DCH], bf16, kind="ExternalInput")
    w = nc.dram_tensor("w", [P, DCH * 192 + 4], bf16, kind="ExternalInput")
    wo = nc.dram_tensor("wo", [D_HEAD, D_MODEL], bf16, kind="ExternalInput")
    y = nc.dram_tensor("y", [NSLOT, P, 4, D_MODEL], bf16, kind="ExternalOutput")

    last_wave = _check_waves()
    # output-projection ops (j, i): slot0 -> waves 2,3; slot1 -> 5,6;
    # slot2 -> wave 6 (where DVE is light: slot-3 chunks kc<24 need no
    # masks); slot3 -> tail (wave index NWAVE)
    out_sched = {wi: [] for wi in range(NWAVE + 1)}
    for j, tgt in ((0, (2, 3)), (1, (5, 6)), (2, (6, 6)), (3, (8, 8))):
        for i in range(8):
            out_sched[tgt[i // 4]].append((j, i))

    with tile.TileContext(nc) as tc:
        with (
            tc.tile_pool(name="singles", bufs=1) as singles,
            tc.tile_pool(name="work", bufs=10) as work,
            tc.tile_pool(name="ypool", bufs=2) as ypool,
            tc.tile_pool(name="mm_ps", bufs=1, space="PSUM") as mm_ps,
            tc.tile_pool(name="s_ps", bufs=3, space="PSUM") as s_ps_pool,
            tc.tile_pool(name="pv_ps", bufs=1, space="PSUM") as pv_pool,
        ):
            # ---- persistent SBUF ----
            w_sb = singles.tile([P, DCH * 192 + 4], bf16, tag="w_sb")
            xt_sb = singles.tile([P, SEQ, DCH], bf16, tag="xt_sb")
            msk_sb = singles.tile([P, 4, 512], bf16, tag="msk_sb")
            wo_sb = singles.tile([D_HEAD, D_MODEL], bf16, tag="wo_sb")
            kvt = singles.tile([P, SEQ], bf16, tag="kvt")  # 0:64 K^T, 64:128 V^T
            qt_sb = singles.tile([D_HEAD, NQ], bf16, tag="qt_sb")
            vaug = singles.tile([P, 32, D_HEAD + 1], bf16, tag="vaug")
            ot = singles.tile([D_HEAD + 1, NQ], bf16, tag="ot")
            rden = singles.tile([1, NQ], fp32, tag="rden")
            rbias = singles.tile([P, 1], fp32, tag="rbias")
            rdent = singles.tile([P, 16], fp32, tag="rdent")
            ident = singles.tile([P, D_HEAD], bf16, tag="ident")
            one_sb = singles.tile([1, 1], fp32, tag="one_sb")

            # ---- input DMAs (SP queue, ordered by first use) ----
            def ld_x(dst, src, s0):
                nc.sync.dma_start(
                    out=dst[:, s0 : s0 + 512, :], in_=src[:, s0 : s0 + 512, :]
                )

            def ld_xh(dst, src, s0, n=512):
                nc.sync.dma_start(
                    out=dst[:, s0 : s0 + n, :], in_=src[:, s0 : s0 + n, :]
                )

            nc.sync.dma_start(out=w_sb[:, 0:512], in_=w[:, 0:512])
            ld_xh(xt_sb, xt, 0, 128)
            ld_xh(xt_sb, xt, 128, 128)
            ld_xh(xt_sb, xt, 256, 128)
            nc.sync.dma_start(out=w_sb[:, 512:1536], in_=w[:, 512:1536])
            ld_xh(xt_sb, xt, 384, 128)
            nc.sync.dma_start(out=w_sb[:, 1536:1540], in_=w[:, 1536:1540])
            ld_x(xt_sb, xt, 512)       # chunk 1: wave-1 fresh keys
            ld_x(xt_sb, xt, 1024)      # chunk 2: slot-1 queries
            ld_x(xt_sb, xt, 2048)      # chunk 4: slot-2 queries
            nc.sync.dma_start(out=wo_sb, in_=wo[:, :])
            ld_x(xt_sb, xt, 1536)      # chunk 3
            ld_x(xt_sb, xt, 3072)      # chunk 6: slot-3 queries
            ld_x(xt_sb, xt, 2560)      # chunk 5
            ld_x(xt_sb, xt, 3584)      # chunk 7

            nc.vector.memset(one_sb, 1.0)
            nc.vector.memset(vaug[:, :, D_HEAD : D_HEAD + 1], 1.0)
            make_identity(nc, ident[D_HEAD:P, :])
            # fp32 per-partition exp bias (see w layout note)
            nc.vector.tensor_copy(out=rbias, in_=w_sb[:, 1536:1537])
            # causal band masks generated on the idle Pool engine:
            # msk_sb[p, c, jq] = 1 if 128c + p <= jq else 0, per band c
            for c in range(4):
                nc.gpsimd.memset(msk_sb[:, c, :], 1.0)
                nc.gpsimd.affine_select(
                    out=msk_sb[:, c, :],
                    in_=msk_sb[:, c, :],
                    compare_op=mybir.AluOpType.is_ge,
                    fill=0.0,
                    base=-128 * c,
                    channel_multiplier=-1,
                    pattern=[[1, 512]],
                )

            def kv_proj_mms(sc):
                """Generator: one KV-projection matmul per next() call."""
                kp = mm_ps.tile([P, 512], fp32, tag="mm")
                for dc in range(DCH):
                    nc.tensor.matmul(
                        kp,
                        lhsT=w_sb[:, 512 + dc * 128 : 512 + dc * 128 + 128],
                        rhs=xt_sb[:, sc * 512 : (sc + 1) * 512, dc],
                        start=(dc == 0),
                        stop=(dc == DCH - 1),
                    )
                    yield
                nc.vector.tensor_copy(
                    out=kvt[:, sc * 512 : (sc + 1) * 512], in_=kp
                )
                yield

            def transposes(sc):
                for t in range(4):  # V^T 128-col blocks -> natural V chunks
                    kc = sc * 4 + t
                    tp = s_ps_pool.tile([P, D_HEAD], bf16, tag="s_ps")
                    nc.tensor.transpose(
                        tp,
                        kvt[D_HEAD:P, kc * P : (kc + 1) * P],
                        ident[D_HEAD:P, :],
                    )
                    nc.vector.tensor_copy(out=vaug[:, kc, :D_HEAD], in_=tp)

            # PV accumulators: full-bank tiles, PV uses rows 0:65
            pv = [
                pv_pool.tile([P, 512], fp32, tag=f"pv{g}", name=f"pv{g}")
                for g in range(NSLOT)
            ]
            freed = []  # pv banks released by finalized slots
            y_tiles = {}
            ncopy = [0]
            pending_pv = []

            def emit_pv(j, kc):
                nc.tensor.matmul(
                    pv[j][0 : D_HEAD + 1, :],
                    lhsT=vaug[:, kc, :],
                    rhs=pending_pv_pt.pop((j, kc)),
                    start=(kc == 0),
                    stop=(kc == E[j] - 1),
                    skip_group_check=True,
                )

            pending_pv_pt = {}

            def emit_chunk(j, kc, wv=0):
                sps = s_ps_pool.tile([P, 512], fp32, tag="s_ps")
                nc.tensor.matmul(
                    sps,
                    lhsT=kvt[0:D_HEAD, kc * P : (kc + 1) * P],
                    rhs=qt_sb[:, j * 512 : (j + 1) * 512],
                    start=True,
                    stop=True,
                )
                p_t = work.tile([P, 512], bf16, tag="p_t")
                band = kc - (E[j] - 8)
                if band >= 4:
                    # partner block: role-0 kills the whole chunk via the
                    # exp bias (exp(s - 30) ~ 1e-10); role 1 keeps it whole
                    nc.scalar.activation(
                        p_t, sps, mybir.ActivationFunctionType.Exp,
                        bias=rbias[:, 0:1],
                    )
                else:
                    nc.scalar.activation(
                        p_t, sps, mybir.ActivationFunctionType.Exp
                    )
                if 0 <= band < 4:
                    # diagonal band: per-query causal step mask
                    nc.vector.tensor_tensor(
                        p_t, p_t, msk_sb[:, band, :], mybir.AluOpType.mult
                    )
                pending_pv_pt[(j, kc)] = p_t
                pending_pv.append((j, kc))
                if len(pending_pv) > 10:
                    emit_pv(*pending_pv.pop(0))

            def emit_out_op(j, i, bank=None):
                """One output-projection matmul + scaled PSUM->SBUF copy."""
                t, no = i // 2, i % 2
                if bank is None:
                    bank = freed[ncopy[0] % len(freed)]
                    ncopy[0] += 1
                q0 = j * 512 + t * P
                nc.tensor.matmul(
                    bank,
                    lhsT=ot[0:D_HEAD, q0 : q0 + P],
                    rhs=wo_sb[:, no * 512 : (no + 1) * 512],
                    start=True,
                    stop=True,
                )
                if j == 3 and i % 2 == 0:
                    nc.scalar.mul(
                        y_tiles[j][:, t, no * 512 : (no + 1) * 512],
                        bank,
                        rdent[:, 4 * j + t : 4 * j + t + 1],
                    )
                else:
                    nc.vector.tensor_scalar_mul(
                        y_tiles[j][:, t, no * 512 : (no + 1) * 512],
                        bank,
                        rdent[:, 4 * j + t : 4 * j + t + 1],
                    )
                if j == 3:
                    if i % 2 == 1:
                        # per-tile DMA: few enough that HWDGE desc-gen
                        # (625ns each, serialized) stays off the tail path
                        nc.sync.dma_start(
                            out=y[j][:, t : t + 1, :],
                            in_=y_tiles[j][:, t : t + 1, :],
                        )
                elif i == 3:
                    nc.sync.dma_start(
                        out=y[j][:, 0:2, :], in_=y_tiles[j][:, 0:2, :]
                    )
                elif i == 7:
                    nc.sync.dma_start(
                        out=y[j][:, 2:4, :], in_=y_tiles[j][:, 2:4, :]
                    )

            def finalize_half(j, h):
                """Half of slot-j finalize: O^T + den, 1/den, rdent cols."""
                c0 = j * 512 + h * 256
                c1 = c0 + 256
                nc.vector.tensor_copy(
                    out=ot[:, c0:c1], in_=pv[j][0 : D_HEAD + 1, h * 256 : h * 256 + 256]
                )
                nc.vector.reciprocal(
                    rden[:, c0:c1], ot[D_HEAD : D_HEAD + 1, c0:c1]
                )
                for t in (2 * h, 2 * h + 1):
                    nc.tensor.matmul(
                        pv[j][:, t : t + 1],
                        lhsT=rden[:, j * 512 + t * P : j * 512 + (t + 1) * P],
                        rhs=one_sb,
                        start=True,
                        stop=True,
                    )
                nc.vector.tensor_copy(
                    out=rdent[:, 4 * j + 2 * h : 4 * j + 2 * h + 2],
                    in_=pv[j][:, 2 * h : 2 * h + 2],
                )

            def finalize(j):
                # consolidated (same shape as the slot-3 tail): full-width
                # ot copy + reciprocal reading the pv PSUM bank directly,
                # then the four rdent transposes and one rdent copy
                nc.vector.tensor_copy(
                    out=ot[0:D_HEAD, j * 512 : (j + 1) * 512],
                    in_=pv[j][0:D_HEAD, :],
                )
                nc.vector.reciprocal(
                    rden[:, j * 512 : (j + 1) * 512],
                    pv[j][D_HEAD : D_HEAD + 1, :],
                )
                for t in range(4):
                    nc.tensor.matmul(
                        pv[j][:, t : t + 1],
                        lhsT=rden[:, j * 512 + t * P : j * 512 + (t + 1) * P],
                        rhs=one_sb,
                        start=True,
                        stop=True,
                    )
                nc.vector.tensor_copy(
                    out=rdent[:, 4 * j : 4 * j + 4], in_=pv[j][:, 0:4]
                )
                freed.append(pv[j])
                y_tiles[j] = ypool.tile(
                    [P, 4, D_MODEL], bf16, tag="y_sb", name=f"y{j}"
                )

            def q_piece(c0, n):
                """Prologue Q projection over columns [c0, c0+n) of slot 0.
                Uses the s_ps pool so pieces rotate PSUM banks instead of
                serializing on the single mm bank."""
                qp = s_ps_pool.tile([D_HEAD, n], fp32, tag="s_ps", name=f"qp{c0}")
                for dc in range(DCH):
                    nc.tensor.matmul(
                        qp,
                        lhsT=w_sb[:, dc * 64 : dc * 64 + 64],
                        rhs=xt_sb[:, c0 : c0 + n, dc],
                        start=(dc == 0),
                        stop=(dc == DCH - 1),
                    )
                nc.vector.tensor_copy(out=qt_sb[:, c0 : c0 + n], in_=qp)

            def kv_piece(c0, n):
                kp = s_ps_pool.tile([P, n], fp32, tag="s_ps", name=f"kp{c0}")
                for dc in range(DCH):
                    nc.tensor.matmul(
                        kp,
                        lhsT=w_sb[:, 512 + dc * 128 : 512 + dc * 128 + 128],
                        rhs=xt_sb[:, c0 : c0 + n, dc],
                        start=(dc == 0),
                        stop=(dc == DCH - 1),
                    )
                nc.vector.tensor_copy(out=kvt[:, c0 : c0 + n], in_=kp)

            def q_proj_steps(j):
                """Generator version of q_proj: one matmul per next()."""
                qp = mm_ps.tile([D_HEAD, 512], fp32, tag="mm")
                for dc in range(DCH):
                    nc.tensor.matmul(
                        qp,
                        lhsT=w_sb[:, dc * 64 : dc * 64 + 64],
                        rhs=xt_sb[:, j * 1024 : j * 1024 + 512, dc],
                        start=(dc == 0),
                        stop=(dc == DCH - 1),
                    )
                    yield
                nc.vector.tensor_copy(
                    out=qt_sb[:, j * 512 : (j + 1) * 512], in_=qp
                )
                yield

            def transpose_steps(sc):
                tp = s_ps_pool.tile([P, 4, D_HEAD], bf16, tag="s_ps", name="tp4")
                for t in range(4):
                    kc = sc * 4 + t
                    nc.tensor.transpose(
                        tp[:, t, :],
                        kvt[D_HEAD:P, kc * P : (kc + 1) * P],
                        ident[D_HEAD:P, :],
                    )
                    yield
                nc.vector.tensor_copy(
                    out=vaug[:, sc * 4 : sc * 4 + 4, :D_HEAD], in_=tp
                )
                yield

            # Q projection for slot j runs as filler inside wave j (its
            # first consumers are that wave's chunks)
            qproj_wave = {1: 1, 2: 2, 3: 3}

            # ---- prologue + wave 0: piece-width projections matched to the
            # DMA bite arrival order (x cols 0:128, 128:384, 384:512). The
            # V-transposes for kc 0..1 must be emitted before chunk (0,3)
            # pops PV(0,0), else the vaug dependency is never recorded. ----
            tg0 = transpose_steps(0)
            q_piece(0, 128)
            q_piece(128, 128)
            q_piece(256, 128)
            kv_piece(0, 128)
            kv_piece(128, 256)
            q_piece(384, 128)
            kv_piece(384, 128)
            next(tg0)
            next(tg0)
            emit_chunk(0, 0, 0)
            emit_chunk(0, 1, 0)
            next(tg0)
            next(tg0)
            emit_chunk(0, 2, 0)
            emit_chunk(0, 3, 0)
            for _ in tg0:  # drain: emits the packed vaug copy for kc 0..3
                pass

            # ---- streamed waves ----
            # Wave sc fillers: [qproj (if due), KV proj for sc, spacer,
            # V transposes for sc], popped two per chunk. Wave 0's proj and
            # transposes ran in the prologue.
            for sc in range(1, NWAVE):
                chunks = WAVES[sc]
                oo = list(out_sched[sc])
                nq = 9 if sc in qproj_wave else 0
                # kv projection FIRST: both share the single mm PSUM bank,
                # and the q projection may wait on later-arriving x columns
                # — allocated first it would block the kv chain via WAR
                filler = [
                    kv_proj_mms(sc),
                    iter([None, None]),  # spacer: kvt copy drains
                    transpose_steps(sc),
                ]
                if nq:
                    filler.append(q_proj_steps(qproj_wave[sc]))
                total_fill = nq + 16
                fill_iter = (x for g in filler for x in g)
                pops = [0]

                def pop_fill(upto=None, k=None):
                    tgt = upto if upto is not None else pops[0] + k
                    while pops[0] < min(tgt, total_fill):
                        if next(fill_iter, -1) == -1:
                            pops[0] = total_fill
                            break
                        pops[0] += 1

                # emission-order safety points: a fresh chunk's scores need
                # this wave's kvt copy emitted; a fresh chunk's PV needs its
                # V-transpose emitted
                n = len(chunks)
                for ci, (j, kc) in enumerate(chunks):
                    if kc // 4 == sc:
                        pop_fill(upto=9)
                    if sc in qproj_wave and j == qproj_wave[sc]:
                        pop_fill(upto=total_fill)
                    if len(pending_pv) >= 4:
                        j2, kc2 = pending_pv[0]
                        if kc2 // 4 == sc:
                            pop_fill(upto=total_fill)
                    emit_chunk(j, kc, sc)
                    if oo and ci % 2 == 1:
                        emit_out_op(*oo.pop(0))
                    k = -(-(total_fill - pops[0]) // (n - ci))  # ceil
                    pop_fill(k=min(k, 4))
                pop_fill(upto=total_fill)
                for op in oo:
                    emit_out_op(*op)
                fin = [j for j in range(NSLOT) if last_wave[j] == sc and j != 3]
                if fin:
                    while pending_pv:
                        emit_pv(*pending_pv.pop(0))
                    for j in fin:
                        finalize(j)

            # ---- slot-3 tail: per-128-query-tile pipeline. The ot copy and
            # the reciprocal both read the pv PSUM bank directly (no serial
            # copy->recip dependency), then rdent -> out-proj x2 -> scaled
            # copy (DVE half, ACT half) -> per-tile y DMA, so successive
            # tiles overlap across engines. ----
            while pending_pv:
                emit_pv(*pending_pv.pop(0))
            y_tiles[3] = ypool.tile([P, 4, D_MODEL], bf16, tag="y_sb", name="y3")

            def finalize_q(j, t):
                """Per-128-query finalize: reciprocal reads the pv PSUM
                bank directly; the ot copy is one full-width ACT op emitted
                by the caller."""
                c0 = j * 512 + t * P
                nc.vector.reciprocal(
                    rden[:, c0 : c0 + P],
                    pv[j][D_HEAD : D_HEAD + 1, t * P : (t + 1) * P],
                )
                nc.tensor.matmul(
                    pv[j][:, t : t + 1],
                    lhsT=rden[:, c0 : c0 + P],
                    rhs=one_sb,
                    start=True,
                    stop=True,
                )
                nc.vector.tensor_copy(
                    out=rdent[:, 4 * j + t : 4 * j + t + 1],
                    in_=pv[j][:, t : t + 1],
                )

            # emission order keeps each engine FIFO unblocked: the recip
            # chains for tiles 0-2 all precede the first scale op. Each
            # out-op gets its own PSUM bank (score + mm banks are idle by
            # now) so no matmul waits on a previous scale's drain.
            tail_banks = [
                s_ps_pool.tile([P, 512], fp32, tag="s_ps", name=f"tb{k}")
                for k in range(3)
            ] + [mm_ps.tile([P, 512], fp32, tag="mm", name="tb3")]
            nc.vector.tensor_copy(
                out=ot[0:D_HEAD, 1536:2048], in_=pv[3][0:D_HEAD, :]
            )
            nc.vector.reciprocal(
                rden[:, 1536:2048], pv[3][D_HEAD : D_HEAD + 1, :]
            )
            for t in range(4):
                nc.tensor.matmul(
                    pv[3][:, t : t + 1],
                    lhsT=rden[:, 1536 + t * P : 1536 + (t + 1) * P],
                    rhs=one_sb,
                    start=True,
                    stop=True,
                )
            nc.vector.tensor_copy(out=rdent[:, 12:16], in_=pv[3][:, 0:4])
            emit_out_op(3, 0, bank=tail_banks[0])
            emit_out_op(3, 1, bank=tail_banks[1])
            emit_out_op(3, 2, bank=tail_banks[2])
            emit_out_op(3, 3, bank=tail_banks[3])
            emit_out_op(3, 4, bank=pv[0])
            emit_out_op(3, 5, bank=pv[1])
            emit_out_op(3, 6, bank=pv[2])
            # pv[3] frees as soon as the consolidated rdent copy has read
            # cols 0:4 (~right after the last PV), earlier than any scale
            emit_out_op(3, 7, bank=pv[3])

    nc.finalize()
    return nc


def _get_program():
    global _prog
    if _prog is None:
        _prog = _build_program()
    return _prog


def kernel(x, W_q, W_k, W_v, W_o):
    import ml_dtypes
    from concourse.bass_utils import run_bass_kernel_spmd

    bf = ml_dtypes.bfloat16
    nc = _get_program()

    x = np.asarray(x, dtype=np.float32)
    scale = np.float32(1.0 / np.sqrt(D_HEAD))
    wq_s = np.asarray(W_q, dtype=np.float32) * scale
    wkv = np.concatenate(
        [np.asarray(W_k, dtype=np.float32), np.asarray(W_v, dtype=np.float32)],
        axis=1,
    )  # [1024, 128]
    wq_part = wq_s.reshape(DCH, P, 64).transpose(1, 0, 2).reshape(P, 512)
    wkv_part = wkv.reshape(DCH, P, 128).transpose(1, 0, 2).reshape(P, 1024)
    # per-role exp bias for partner-band chunks: -30 kills the whole chunk
    # for role 0 (exp(s - 30) ~ 1e-10), 0 keeps it whole for role 1
    w_hosts = []
    for r in range(2):
        rb = np.full((P, 4), 0.0 if r == 1 else -30.0, dtype=np.float32)
        w_hosts.append(
            np.ascontiguousarray(
                np.concatenate([wq_part, wkv_part, rb], axis=1)
            ).astype(bf)
        )  # [128, 1540]
    wo_host = np.ascontiguousarray(np.asarray(W_o, dtype=np.float32)).astype(bf)

    in_maps = []
    for c in range(NCORES):
        b, r = c // 2, c % 2
        xt_b = x[b].T  # [1024, 4096]
        # permuted key order: position 2k holds this core's query block
        # 2k+r, position 2k+1 holds the partner block 2k+(1-r)
        cols = np.concatenate(
            [
                np.arange(512 * g, 512 * g + 512)
                for k in range(NSLOT)
                for g in (2 * k + r, 2 * k + 1 - r)
            ]
        )
        xt_host = np.ascontiguousarray(
            xt_b[:, cols].reshape(DCH, P, SEQ).transpose(1, 2, 0)
        ).astype(bf)  # [128, 4096, 8]
        in_maps.append(
            {
                "xt": xt_host,
                "w": w_hosts[r],
                "wo": wo_host,
            }
        )

    res = run_bass_kernel_spmd(nc, in_maps, core_ids=list(range(NCORES)))
    out = np.empty((BATCH, SEQ, D_MODEL), dtype=np.float32)
    for c in range(NCORES):
        b, r = c // 2, c % 2
        yv = np.asarray(res.results[c]["y"]).astype(np.float32)
        # y[j, p, t, :] -> query 512*(2j+r) + 128t + p
        yv = yv.transpose(0, 2, 1, 3)  # [j, t, p, m]
        for j in range(NSLOT):
            q0 = 512 * (2 * j + r)
            out[b, q0 : q0 + 512, :] = yv[j].reshape(512, D_MODEL)
    return out



# revision 121
# speedup vs baseline: 1.0589x; 1.0027x over previous
"""Trainium2 Bass kernel for single-head causal attention.

x:[4,4096,1024] f32, W_q/W_k/W_v:[1024,64], W_o:[64,1024].

Sharding: 8 cores = 4 batches x 2 query-stripe roles. Role r of a batch
owns query blocks {2j+r : j=0..3} (512 queries each). Program slot j has
key extent E[j] = (8j+8) 128-key chunks, which exactly covers role 1's
block 2j+1 and over-covers role 0's block 2j by 4 chunks (dead).

All per-core differences (which batch, which stripe, dead chunks) are
carried in the input data; one SPMD program runs on all 8 cores:
- x is shipped as [128, 4096, 8] (d_model-chunk partition, seq, chunk)
  so any column range is a fully contiguous DMA (no sub-512B-element
  descriptor penalty), streamed in arrival-ordered bites.
- The diagonal causal band masks (4 x [128,512]) are generated on the
  otherwise-idle Pool engine with affine_select, not DMAed.
- Partner-block chunks (band >= 4) take no mask op at all: the exp runs
  with a per-partition bias from the w tensor (-30 for role 0 => p ~
  1e-10, 0 for role 1), so role-0's dead chunks vanish from both the PV
  numerator and the denominator row.

bf16 everywhere on the matmul paths (end-to-end rel err ~5e-3 vs the
2e-2 gate). Engines: PE does all matmuls (62.3us busy, the roofline of
this schedule), ACT does exp + the slot-3 ot copies + half its output
scales, DVE does diag masks + kvt/qt copies + finalize + the other
scales. Attention chunks are emitted in waves matched to DMA arrival,
next wave's KV projection interleaved between chunks as PE filler, PV
matmuls lagged 10 chunks behind their scores. Waves are front-loaded
(each chunk sits in the earliest wave whose x data can feed it) with
the KV projection emitted before the Q projection inside each wave's
filler — they share the single mm PSUM bank and Q may wait on
later-arriving x, so allocated first it would block the KV chain via
WAR. Every slot finalize is consolidated into full-width ops (one
[64,512] ot copy, one [1,512] reciprocal, four 1-col rdent transposes,
one [P,4] rdent copy, reading the PV PSUM bank directly) so all eight
out-projection matmuls of a slot unblock at once; tail out-ops get
distinct PSUM banks and the y DMAs go out per 128-query tile to keep
the final HWDGE descriptor-gen chain (625ns each, serialized) short.
"""

import sys

for _p in ("/opt/trn_rl_repo",):
    if _p not in sys.path:
        sys.path.insert(0, _p)

import numpy as np

D_MODEL = 1024
D_HEAD = 64
SEQ = 4096
BATCH = 4
NCORES = 8
NQ = 2048          # queries per core
P = 128
DCH = D_MODEL // P  # 8 contraction chunks
NSLOT = 4           # query slots of 512
E = [8, 16, 24, 32]  # key chunks per slot
NWAVE = 8           # key superchunks of 512

# Attention chunks per wave: matched to DMA arrival order (early waves
# light), per-slot ascending kc, every chunk (j,kc) in wave >= kc//4.
WAVES = [
    [(0, k) for k in range(4)],
    [(0, k) for k in range(4, 8)] + [(1, k) for k in range(8)],
    [(1, k) for k in range(8, 12)] + [(2, k) for k in range(12)],
    [(1, k) for k in range(12, 16)] + [(3, k) for k in range(4)],
    [(2, k) for k in range(12, 16)] + [(3, k) for k in range(4, 12)],
    [(2, k) for k in range(16, 24)],
    [(3, k) for k in range(12, 16)],
    [(3, k) for k in range(16, 32)],
]

_prog = None


def _check_waves():
    seen = {}
    total = 0
    for w, wv in enumerate(WAVES):
        for j, kc in wv:
            assert kc // 4 <= w, (w, j, kc)
            assert seen.get(j, -1) == kc - 1, (j, kc)
            seen[j] = kc
            total += 1
    assert total == sum(E) == 80
    return {j: max(w for w, wv in enumerate(WAVES) if (j, E[j] - 1) in wv)
            for j in range(NSLOT)}


def _build_program():
    import concourse.bacc as bacc
    import concourse.mybir as mybir
    import concourse.tile as tile
    from concourse.masks import make_identity

    fp32 = mybir.dt.float32
    f32r = mybir.dt.float32r
    bf16 = mybir.dt.bfloat16
    nc = bacc.Bacc("TRN2", target_bir_lowering=False, debug=False)

    # w layout: [wq 0:512 | wkv 512:1536 | rbias 1536:1540]
    # rbias: -30 for role 0, 0 for role 1. Partner-band chunks (band >= 4)
    # run exp with this per-partition bias instead of a 0/1 mask multiply:
    # each slot's band region only ever covers its own partner block, which
    # is entirely dead for role 0 (exp(s - 30) ~ 1e-10) and entirely alive
    # for role 1 (bias 0). Replaces the 16 explicit partner-mask DVE ops.
    xt = nc.dram_tensor("xt", [P, SEQ, DCH], bf16, kind="ExternalInput")
    w = nc.dram_tensor("w", [P, DCH * 192 + 4], bf16, kind="ExternalInput")
    wo = nc.dram_tensor("wo", [D_HEAD, D_MODEL], bf16, kind="ExternalInput")
    y = nc.dram_tensor("y", [NSLOT, P, 4, D_MODEL], bf16, kind="ExternalOutput")

    last_wave = _check_waves()
    # output-projection ops (j, i): slot0 -> waves 2,3; slot1 -> 5,6;
    # slot2 -> wave 6 (where DVE is light: slot-3 chunks kc<24 need no
    # masks); slot3 -> tail (wave index NWAVE)
    out_sched = {wi: [] for wi in range(NWAVE + 1)}
    for j, tgt in ((0, (2, 2)), (1, (5, 6)), (2, (6, 6)), (3, (8, 8))):
        for i in range(8):
            out_sched[tgt[i // 4]].append((j, i))

    with tile.TileContext(nc) as tc:
        with (
            tc.tile_pool(name="singles", bufs=1) as singles,
            tc.tile_pool(name="work", bufs=10) as work,
            tc.tile_pool(name="ypool", bufs=2) as ypool,
            tc.tile_pool(name="mm_ps", bufs=1, space="PSUM") as mm_ps,
            tc.tile_pool(name="s_ps", bufs=4, space="PSUM") as s_ps_pool,
            tc.tile_pool(name="pv_ps", bufs=1, space="PSUM") as pv_pool,
        ):
            # ---- persistent SBUF ----
            w_sb = singles.tile([P, DCH * 192 + 4], bf16, tag="w_sb")
            xt_sb = singles.tile([P, SEQ, DCH], bf16, tag="xt_sb")
            msk_sb = singles.tile([P, 4, 512], bf16, tag="msk_sb")
            wo_sb = singles.tile([D_HEAD, D_MODEL], bf16, tag="wo_sb")
            kvt = singles.tile([P, SEQ], bf16, tag="kvt")  # 0:64 K^T, 64:128 V^T
            qt_sb = singles.tile([D_HEAD, NQ], bf16, tag="qt_sb")
            vaug = singles.tile([P, 32, D_HEAD + 1], bf16, tag="vaug")
            ot = singles.tile([D_HEAD + 1, NQ], bf16, tag="ot")
            rden = singles.tile([1, NQ], fp32, tag="rden")
            rbias = singles.tile([P, 1], fp32, tag="rbias")
            rdent = singles.tile([P, 16], fp32, tag="rdent")
            ident = singles.tile([P, D_HEAD], bf16, tag="ident")
            one_sb = singles.tile([1, 1], fp32, tag="one_sb")

            # ---- input DMAs (SP queue, ordered by first use) ----
            def ld_x(dst, src, s0):
                nc.sync.dma_start(
                    out=dst[:, s0 : s0 + 512, :], in_=src[:, s0 : s0 + 512, :]
                )

            def ld_xh(dst, src, s0, n=512):
                nc.sync.dma_start(
                    out=dst[:, s0 : s0 + n, :], in_=src[:, s0 : s0 + n, :]
                )

            nc.sync.dma_start(out=w_sb[:, 0:512], in_=w[:, 0:512])
            ld_xh(xt_sb, xt, 0, 128)
            ld_xh(xt_sb, xt, 128, 128)
            ld_xh(xt_sb, xt, 256, 128)
            nc.sync.dma_start(out=w_sb[:, 512:1536], in_=w[:, 512:1536])
            ld_xh(xt_sb, xt, 384, 128)
            nc.sync.dma_start(out=w_sb[:, 1536:1540], in_=w[:, 1536:1540])
            ld_x(xt_sb, xt, 512)       # chunk 1: wave-1 fresh keys
            ld_x(xt_sb, xt, 1024)      # chunk 2: slot-1 queries
            ld_x(xt_sb, xt, 2048)      # chunk 4: slot-2 queries
            nc.sync.dma_start(out=wo_sb, in_=wo[:, :])
            ld_x(xt_sb, xt, 1536)      # chunk 3
            ld_x(xt_sb, xt, 3072)      # chunk 6: slot-3 queries
            ld_x(xt_sb, xt, 2560)      # chunk 5
            ld_x(xt_sb, xt, 3584)      # chunk 7

            nc.vector.memset(one_sb, 1.0)
            nc.vector.memset(vaug[:, :, D_HEAD : D_HEAD + 1], 1.0)
            make_identity(nc, ident[D_HEAD:P, :])
            # fp32 per-partition exp bias (see w layout note)
            nc.vector.tensor_copy(out=rbias, in_=w_sb[:, 1536:1537])
            # causal band masks generated on the idle Pool engine:
            # msk_sb[p, c, jq] = 1 if 128c + p <= jq else 0, per band c
            for c in range(4):
                nc.gpsimd.memset(msk_sb[:, c, :], 1.0)
                nc.gpsimd.affine_select(
                    out=msk_sb[:, c, :],
                    in_=msk_sb[:, c, :],
                    compare_op=mybir.AluOpType.is_ge,
                    fill=0.0,
                    base=-128 * c,
                    channel_multiplier=-1,
                    pattern=[[1, 512]],
                )

            def kv_proj_mms(sc):
                """Generator: one KV-projection matmul per next() call."""
                kp = mm_ps.tile([P, 512], fp32, tag="mm")
                for dc in range(DCH):
                    nc.tensor.matmul(
                        kp,
                        lhsT=w_sb[:, 512 + dc * 128 : 512 + dc * 128 + 128],
                        rhs=xt_sb[:, sc * 512 : (sc + 1) * 512, dc],
                        start=(dc == 0),
                        stop=(dc == DCH - 1),
                    )
                    yield
                nc.vector.tensor_copy(
                    out=kvt[:, sc * 512 : (sc + 1) * 512], in_=kp
                )
                yield

            def transposes(sc):
                for t in range(4):  # V^T 128-col blocks -> natural V chunks
                    kc = sc * 4 + t
                    tp = s_ps_pool.tile([P, D_HEAD], bf16, tag="s_ps")
                    nc.tensor.transpose(
                        tp,
                        kvt[D_HEAD:P, kc * P : (kc + 1) * P],
                        ident[D_HEAD:P, :],
                    )
                    nc.vector.tensor_copy(out=vaug[:, kc, :D_HEAD], in_=tp)

            # PV accumulators: full-bank tiles, PV uses rows 0:65.
            # Slot 3 shares slot 0's bank: slot-0 accumulation ends in wave
            # 1 and its out-ops are confined to wave 2, while slot-3
            # accumulation starts in wave 3 (start=True clears the bank).
            pv = [
                pv_pool.tile([P, 512], fp32, tag=f"pv{g}", name=f"pv{g}")
                for g in range(3)
            ]
            pv.append(pv[0])
            freed = []  # pv banks released by finalized slots
            y_tiles = {}
            ncopy = [0]
            pending_pv = []

            def emit_pv(j, kc):
                nc.tensor.matmul(
                    pv[j][0 : D_HEAD + 1, :],
                    lhsT=vaug[:, kc, :],
                    rhs=pending_pv_pt.pop((j, kc)),
                    start=(kc == 0),
                    stop=(kc == E[j] - 1),
                    skip_group_check=True,
                )

            pending_pv_pt = {}

            def emit_chunk(j, kc, wv=0):
                sps = s_ps_pool.tile([P, 512], fp32, tag="s_ps")
                nc.tensor.matmul(
                    sps,
                    lhsT=kvt[0:D_HEAD, kc * P : (kc + 1) * P],
                    rhs=qt_sb[:, j * 512 : (j + 1) * 512],
                    start=True,
                    stop=True,
                )
                p_t = work.tile([P, 512], bf16, tag="p_t")
                band = kc - (E[j] - 8)
                if band >= 4:
                    # partner block: role-0 kills the whole chunk via the
                    # exp bias (exp(s - 30) ~ 1e-10); role 1 keeps it whole
                    nc.scalar.activation(
                        p_t, sps, mybir.ActivationFunctionType.Exp,
                        bias=rbias[:, 0:1],
                    )
                else:
                    nc.scalar.activation(
                        p_t, sps, mybir.ActivationFunctionType.Exp
                    )
                if 0 <= band < 4:
                    # diagonal band: per-query causal step mask
                    nc.vector.tensor_tensor(
                        p_t, p_t, msk_sb[:, band, :], mybir.AluOpType.mult
                    )
                pending_pv_pt[(j, kc)] = p_t
                pending_pv.append((j, kc))
                if len(pending_pv) > 10:
                    emit_pv(*pending_pv.pop(0))

            def emit_out_op(j, i, bank=None):
                """One output-projection matmul + scaled PSUM->SBUF copy."""
                t, no = i // 2, i % 2
                if bank is None:
                    bank = freed[ncopy[0] % len(freed)]
                    ncopy[0] += 1
                q0 = j * 512 + t * P
                nc.tensor.matmul(
                    bank,
                    lhsT=ot[0:D_HEAD, q0 : q0 + P],
                    rhs=wo_sb[:, no * 512 : (no + 1) * 512],
                    start=True,
                    stop=True,
                )
                if j == 3 and i % 2 == 0:
                    nc.scalar.mul(
                        y_tiles[j][:, t, no * 512 : (no + 1) * 512],
                        bank,
                        rdent[:, 4 * j + t : 4 * j + t + 1],
                    )
                else:
                    nc.vector.tensor_scalar_mul(
                        y_tiles[j][:, t, no * 512 : (no + 1) * 512],
                        bank,
                        rdent[:, 4 * j + t : 4 * j + t + 1],
                    )
                if j == 3:
                    if i % 2 == 1:
                        # per-tile DMA: few enough that HWDGE desc-gen
                        # (625ns each, serialized) stays off the tail path
                        nc.sync.dma_start(
                            out=y[j][:, t : t + 1, :],
                            in_=y_tiles[j][:, t : t + 1, :],
                        )
                elif i == 3:
                    nc.sync.dma_start(
                        out=y[j][:, 0:2, :], in_=y_tiles[j][:, 0:2, :]
                    )
                elif i == 7:
                    nc.sync.dma_start(
                        out=y[j][:, 2:4, :], in_=y_tiles[j][:, 2:4, :]
                    )

            def finalize_half(j, h):
                """Half of slot-j finalize: O^T + den, 1/den, rdent cols."""
                c0 = j * 512 + h * 256
                c1 = c0 + 256
                nc.vector.tensor_copy(
                    out=ot[:, c0:c1], in_=pv[j][0 : D_HEAD + 1, h * 256 : h * 256 + 256]
                )
                nc.vector.reciprocal(
                    rden[:, c0:c1], ot[D_HEAD : D_HEAD + 1, c0:c1]
                )
                for t in (2 * h, 2 * h + 1):
                    nc.tensor.matmul(
                        pv[j][:, t : t + 1],
                        lhsT=rden[:, j * 512 + t * P : j * 512 + (t + 1) * P],
                        rhs=one_sb,
                        start=True,
                        stop=True,
                    )
                nc.vector.tensor_copy(
                    out=rdent[:, 4 * j + 2 * h : 4 * j + 2 * h + 2],
                    in_=pv[j][:, 2 * h : 2 * h + 2],
                )

            def finalize(j):
                # consolidated (same shape as the slot-3 tail): full-width
                # ot copy + reciprocal reading the pv PSUM bank directly,
                # then the four rdent transposes and one rdent copy
                nc.vector.tensor_copy(
                    out=ot[0:D_HEAD, j * 512 : (j + 1) * 512],
                    in_=pv[j][0:D_HEAD, :],
                )
                nc.vector.reciprocal(
                    rden[:, j * 512 : (j + 1) * 512],
                    pv[j][D_HEAD : D_HEAD + 1, :],
                )
                for t in range(4):
                    nc.tensor.matmul(
                        pv[j][:, t : t + 1],
                        lhsT=rden[:, j * 512 + t * P : j * 512 + (t + 1) * P],
                        rhs=one_sb,
                        start=True,
                        stop=True,
                    )
                nc.vector.tensor_copy(
                    out=rdent[:, 4 * j : 4 * j + 4], in_=pv[j][:, 0:4]
                )
                freed.append(pv[j])
                y_tiles[j] = ypool.tile(
                    [P, 4, D_MODEL], bf16, tag="y_sb", name=f"y{j}"
                )

            def q_piece(c0, n):
                """Prologue Q projection over columns [c0, c0+n) of slot 0.
                Uses the s_ps pool so pieces rotate PSUM banks instead of
                serializing on the single mm bank."""
                qp = s_ps_pool.tile([D_HEAD, n], fp32, tag="s_ps", name=f"qp{c0}")
                for dc in range(DCH):
                    nc.tensor.matmul(
                        qp,
                        lhsT=w_sb[:, dc * 64 : dc * 64 + 64],
                        rhs=xt_sb[:, c0 : c0 + n, dc],
                        start=(dc == 0),
                        stop=(dc == DCH - 1),
                    )
                nc.vector.tensor_copy(out=qt_sb[:, c0 : c0 + n], in_=qp)

            def kv_piece(c0, n):
                kp = s_ps_pool.tile([P, n], fp32, tag="s_ps", name=f"kp{c0}")
                for dc in range(DCH):
                    nc.tensor.matmul(
                        kp,
                        lhsT=w_sb[:, 512 + dc * 128 : 512 + dc * 128 + 128],
                        rhs=xt_sb[:, c0 : c0 + n, dc],
                        start=(dc == 0),
                        stop=(dc == DCH - 1),
                    )
                nc.vector.tensor_copy(out=kvt[:, c0 : c0 + n], in_=kp)

            def q_proj_steps(j):
                """Generator version of q_proj: one matmul per next()."""
                qp = mm_ps.tile([D_HEAD, 512], fp32, tag="mm")
                for dc in range(DCH):
                    nc.tensor.matmul(
                        qp,
                        lhsT=w_sb[:, dc * 64 : dc * 64 + 64],
                        rhs=xt_sb[:, j * 1024 : j * 1024 + 512, dc],
                        start=(dc == 0),
                        stop=(dc == DCH - 1),
                    )
                    yield
                nc.vector.tensor_copy(
                    out=qt_sb[:, j * 512 : (j + 1) * 512], in_=qp
                )
                yield

            def transpose_steps(sc):
                tp = s_ps_pool.tile([P, 4, D_HEAD], bf16, tag="s_ps", name="tp4")
                for t in range(4):
                    kc = sc * 4 + t
                    nc.tensor.transpose(
                        tp[:, t, :],
                        kvt[D_HEAD:P, kc * P : (kc + 1) * P],
                        ident[D_HEAD:P, :],
                    )
                    yield
                nc.vector.tensor_copy(
                    out=vaug[:, sc * 4 : sc * 4 + 4, :D_HEAD], in_=tp
                )
                yield

            # Q projection for slot j runs as filler inside wave j (its
            # first consumers are that wave's chunks)
            qproj_wave = {1: 1, 2: 2, 3: 3}

            # ---- prologue + wave 0: piece-width projections matched to the
            # DMA bite arrival order (x cols 0:128, 128:384, 384:512). The
            # V-transposes for kc 0..1 must be emitted before chunk (0,3)
            # pops PV(0,0), else the vaug dependency is never recorded. ----
            tg0 = transpose_steps(0)
            q_piece(0, 128)
            q_piece(128, 128)
            q_piece(256, 128)
            kv_piece(0, 128)
            kv_piece(128, 256)
            q_piece(384, 128)
            kv_piece(384, 128)
            next(tg0)
            next(tg0)
            emit_chunk(0, 0, 0)
            emit_chunk(0, 1, 0)
            next(tg0)
            next(tg0)
            emit_chunk(0, 2, 0)
            emit_chunk(0, 3, 0)
            for _ in tg0:  # drain: emits the packed vaug copy for kc 0..3
                pass

            # ---- streamed waves ----
            # Wave sc fillers: [qproj (if due), KV proj for sc, spacer,
            # V transposes for sc], popped two per chunk. Wave 0's proj and
            # transposes ran in the prologue.
            for sc in range(1, NWAVE):
                chunks = WAVES[sc]
                oo = list(out_sched[sc])
                nq = 9 if sc in qproj_wave else 0
                # kv projection FIRST: both share the single mm PSUM bank,
                # and the q projection may wait on later-arriving x columns
                # — allocated first it would block the kv chain via WAR
                filler = [
                    kv_proj_mms(sc),
                    iter([None, None]),  # spacer: kvt copy drains
                    transpose_steps(sc),
                ]
                if nq:
                    filler.append(q_proj_steps(qproj_wave[sc]))
                total_fill = nq + 16
                fill_iter = (x for g in filler for x in g)
                pops = [0]

                def pop_fill(upto=None, k=None):
                    tgt = upto if upto is not None else pops[0] + k
                    while pops[0] < min(tgt, total_fill):
                        if next(fill_iter, -1) == -1:
                            pops[0] = total_fill
                            break
                        pops[0] += 1

                # emission-order safety points: a fresh chunk's scores need
                # this wave's kvt copy emitted; a fresh chunk's PV needs its
                # V-transpose emitted
                n = len(chunks)
                for ci, (j, kc) in enumerate(chunks):
                    if kc // 4 == sc:
                        pop_fill(upto=9)
                    if sc in qproj_wave and j == qproj_wave[sc]:
                        pop_fill(upto=total_fill)
                    if len(pending_pv) >= 4:
                        j2, kc2 = pending_pv[0]
                        if kc2 // 4 == sc:
                            pop_fill(upto=total_fill)
                    emit_chunk(j, kc, sc)
                    if oo and ci % 2 == 1:
                        emit_out_op(*oo.pop(0))
                    k = -(-(total_fill - pops[0]) // (n - ci))  # ceil
                    pop_fill(k=min(k, 4))
                pop_fill(upto=total_fill)
                for op in oo:
                    emit_out_op(*op)
                if sc == 2:
                    # pv[0] now belongs to slot-3 accumulation
                    freed.clear()
                fin = [j for j in range(NSLOT) if last_wave[j] == sc and j != 3]
                if fin:
                    while pending_pv:
                        emit_pv(*pending_pv.pop(0))
                    for j in fin:
                        finalize(j)

            # ---- slot-3 tail: per-128-query-tile pipeline. The ot copy and
            # the reciprocal both read the pv PSUM bank directly (no serial
            # copy->recip dependency), then rdent -> out-proj x2 -> scaled
            # copy (DVE half, ACT half) -> per-tile y DMA, so successive
            # tiles overlap across engines. ----
            while pending_pv:
                emit_pv(*pending_pv.pop(0))
            y_tiles[3] = ypool.tile([P, 4, D_MODEL], bf16, tag="y_sb", name="y3")

            def finalize_q(j, t):
                """Per-128-query finalize: reciprocal reads the pv PSUM
                bank directly; the ot copy is one full-width ACT op emitted
                by the caller."""
                c0 = j * 512 + t * P
                nc.vector.reciprocal(
                    rden[:, c0 : c0 + P],
                    pv[j][D_HEAD : D_HEAD + 1, t * P : (t + 1) * P],
                )
                nc.tensor.matmul(
                    pv[j][:, t : t + 1],
                    lhsT=rden[:, c0 : c0 + P],
                    rhs=one_sb,
                    start=True,
                    stop=True,
                )
                nc.vector.tensor_copy(
                    out=rdent[:, 4 * j + t : 4 * j + t + 1],
                    in_=pv[j][:, t : t + 1],
                )

            # emission order keeps each engine FIFO unblocked: the recip
            # chains for tiles 0-2 all precede the first scale op. Each
            # out-op gets its own PSUM bank (score + mm banks are idle by
            # now) so no matmul waits on a previous scale's drain.
            tail_banks = [
                s_ps_pool.tile([P, 512], fp32, tag="s_ps", name=f"tb{k}")
                for k in range(4)
            ] + [mm_ps.tile([P, 512], fp32, tag="mm", name="tb4")]
            nc.vector.tensor_copy(
                out=ot[0:D_HEAD, 1536:2048], in_=pv[3][0:D_HEAD, :]
            )
            nc.vector.reciprocal(
                rden[:, 1536:2048], pv[3][D_HEAD : D_HEAD + 1, :]
            )
            for t in range(4):
                nc.tensor.matmul(
                    pv[3][:, t : t + 1],
                    lhsT=rden[:, 1536 + t * P : 1536 + (t + 1) * P],
                    rhs=one_sb,
                    start=True,
                    stop=True,
                )
            nc.vector.tensor_copy(out=rdent[:, 12:16], in_=pv[3][:, 0:4])
            emit_out_op(3, 0, bank=tail_banks[0])
            emit_out_op(3, 1, bank=tail_banks[1])
            emit_out_op(3, 2, bank=tail_banks[2])
            emit_out_op(3, 3, bank=tail_banks[3])
            emit_out_op(3, 4, bank=tail_banks[4])
            emit_out_op(3, 5, bank=pv[1])
            emit_out_op(3, 6, bank=pv[2])
            # pv[3] frees as soon as the consolidated rdent copy has read
            # cols 0:4 (~right after the last PV), earlier than any scale
            emit_out_op(3, 7, bank=pv[3])

    nc.finalize()
    return nc


def _get_program():
    global _prog
    if _prog is None:
        _prog = _build_program()
    return _prog


def kernel(x, W_q, W_k, W_v, W_o):
    import ml_dtypes
    from concourse.bass_utils import run_bass_kernel_spmd

    bf = ml_dtypes.bfloat16
    nc = _get_program()

    x = np.asarray(x, dtype=np.float32)
    scale = np.float32(1.0 / np.sqrt(D_HEAD))
    wq_s = np.asarray(W_q, dtype=np.float32) * scale
    wkv = np.concatenate(
        [np.asarray(W_k, dtype=np.float32), np.asarray(W_v, dtype=np.float32)],
        axis=1,
    )  # [1024, 128]
    wq_part = wq_s.reshape(DCH, P, 64).transpose(1, 0, 2).reshape(P, 512)
    wkv_part = wkv.reshape(DCH, P, 128).transpose(1, 0, 2).reshape(P, 1024)
    # per-role exp bias for partner-band chunks: -30 kills the whole chunk
    # for role 0 (exp(s - 30) ~ 1e-10), 0 keeps it whole for role 1
    w_hosts = []
    for r in range(2):
        rb = np.full((P, 4), 0.0 if r == 1 else -30.0, dtype=np.float32)
        w_hosts.append(
            np.ascontiguousarray(
                np.concatenate([wq_part, wkv_part, rb], axis=1)
            ).astype(bf)
        )  # [128, 1540]
    wo_host = np.ascontiguousarray(np.asarray(W_o, dtype=np.float32)).astype(bf)

    in_maps = []
    for c in range(NCORES):
        b, r = c // 2, c % 2
        xt_b = x[b].T  # [1024, 4096]
        # permuted key order: position 2k holds this core's query block
        # 2k+r, position 2k+1 holds the partner block 2k+(1-r)
        cols = np.concatenate(
            [
                np.arange(512 * g, 512 * g + 512)
                for k in range(NSLOT)
                for g in (2 * k + r, 2 * k + 1 - r)
            ]
        )
        xt_host = np.ascontiguousarray(
            xt_b[:, cols].reshape(DCH, P, SEQ).transpose(1, 2, 0)
        ).astype(bf)  # [128, 4096, 8]
        in_maps.append(
            {
                "xt": xt_host,
                "w": w_hosts[r],
                "wo": wo_host,
            }
        )

    res = run_bass_kernel_spmd(nc, in_maps, core_ids=list(range(NCORES)))
    out = np.empty((BATCH, SEQ, D_MODEL), dtype=np.float32)
    for c in range(NCORES):
        b, r = c // 2, c % 2
        yv = np.asarray(res.results[c]["y"]).astype(np.float32)
        # y[j, p, t, :] -> query 512*(2j+r) + 128t + p
        yv = yv.transpose(0, 2, 1, 3)  # [j, t, p, m]
        for j in range(NSLOT):
            q0 = 512 * (2 * j + r)
            out[b, q0 : q0 + 512, :] = yv[j].reshape(512, D_MODEL)
    return out



# revision 124
# speedup vs baseline: 1.0609x; 1.0018x over previous
"""Trainium2 Bass kernel for single-head causal attention.

x:[4,4096,1024] f32, W_q/W_k/W_v:[1024,64], W_o:[64,1024].

Sharding: 8 cores = 4 batches x 2 query-stripe roles. Role r of a batch
owns query blocks {2j+r : j=0..3} (512 queries each). Program slot j has
key extent E[j] = (8j+8) 128-key chunks, which exactly covers role 1's
block 2j+1 and over-covers role 0's block 2j by 4 chunks (dead).

All per-core differences (which batch, which stripe, dead chunks) are
carried in the input data; one SPMD program runs on all 8 cores:
- x is shipped as [128, 4096, 8] (d_model-chunk partition, seq, chunk)
  so any column range is a fully contiguous DMA (no sub-512B-element
  descriptor penalty), streamed in arrival-ordered bites.
- The diagonal causal band masks (4 x [128,512]) are generated on the
  otherwise-idle Pool engine with affine_select, not DMAed.
- Partner-block chunks (band >= 4) take no mask op at all: the exp runs
  with a per-partition bias from the w tensor (-30 for role 0 => p ~
  1e-10, 0 for role 1), so role-0's dead chunks vanish from both the PV
  numerator and the denominator row.

bf16 everywhere on the matmul paths (end-to-end rel err ~5e-3 vs the
2e-2 gate). Engines: PE does all matmuls (62.3us busy, the roofline of
this schedule), ACT does exp + the slot-3 ot copies + half its output
scales, DVE does diag masks + kvt/qt copies + finalize + the other
scales. Attention chunks are emitted in waves matched to DMA arrival,
next wave's KV projection interleaved between chunks as PE filler, PV
matmuls lagged 10 chunks behind their scores. Waves are front-loaded
(each chunk sits in the earliest wave whose x data can feed it) with
the KV projection emitted before the Q projection inside each wave's
filler — they share the single mm PSUM bank and Q may wait on
later-arriving x, so allocated first it would block the KV chain via
WAR. Every slot finalize is consolidated into full-width ops (one
[64,512] ot copy, one [1,512] reciprocal, four 1-col rdent transposes,
one [P,4] rdent copy, reading the PV PSUM bank directly) so all eight
out-projection matmuls of a slot unblock at once; tail out-ops get
distinct PSUM banks and the y DMAs go out per 128-query tile to keep
the final HWDGE descriptor-gen chain (625ns each, serialized) short.
"""

import sys

for _p in ("/opt/trn_rl_repo",):
    if _p not in sys.path:
        sys.path.insert(0, _p)

import numpy as np

D_MODEL = 1024
D_HEAD = 64
SEQ = 4096
BATCH = 4
NCORES = 8
NQ = 2048          # queries per core
P = 128
DCH = D_MODEL // P  # 8 contraction chunks
NSLOT = 4           # query slots of 512
E = [8, 16, 24, 32]  # key chunks per slot
NWAVE = 8           # key superchunks of 512

# Attention chunks per wave: matched to DMA arrival order (early waves
# light), per-slot ascending kc, every chunk (j,kc) in wave >= kc//4.
WAVES = [
    [(0, k) for k in range(4)],
    [(0, k) for k in range(4, 8)] + [(1, k) for k in range(8)],
    [(1, k) for k in range(8, 12)] + [(2, k) for k in range(12)],
    [(1, k) for k in range(12, 16)] + [(3, k) for k in range(4)],
    [(2, k) for k in range(12, 16)] + [(3, k) for k in range(4, 12)],
    [(2, k) for k in range(16, 24)],
    [(3, k) for k in range(12, 16)],
    [(3, k) for k in range(16, 32)],
]

_prog = None


def _check_waves():
    seen = {}
    total = 0
    for w, wv in enumerate(WAVES):
        for j, kc in wv:
            assert kc // 4 <= w, (w, j, kc)
            assert seen.get(j, -1) == kc - 1, (j, kc)
            seen[j] = kc
            total += 1
    assert total == sum(E) == 80
    return {j: max(w for w, wv in enumerate(WAVES) if (j, E[j] - 1) in wv)
            for j in range(NSLOT)}


def _build_program():
    import concourse.bacc as bacc
    import concourse.mybir as mybir
    import concourse.tile as tile
    from concourse.masks import make_identity

    fp32 = mybir.dt.float32
    f32r = mybir.dt.float32r
    bf16 = mybir.dt.bfloat16
    nc = bacc.Bacc("TRN2", target_bir_lowering=False, debug=False)

    # w layout: [wq 0:512 | wkv 512:1536 | rbias 1536:1540]
    # rbias: -30 for role 0, 0 for role 1. Partner-band chunks (band >= 4)
    # run exp with this per-partition bias instead of a 0/1 mask multiply:
    # each slot's band region only ever covers its own partner block, which
    # is entirely dead for role 0 (exp(s - 30) ~ 1e-10) and entirely alive
    # for role 1 (bias 0). Replaces the 16 explicit partner-mask DVE ops.
    xt = nc.dram_tensor("xt", [P, SEQ, DCH], bf16, kind="ExternalInput")
    w = nc.dram_tensor("w", [P, DCH * 192 + 4], bf16, kind="ExternalInput")
    wo = nc.dram_tensor("wo", [D_HEAD, D_MODEL], bf16, kind="ExternalInput")
    y = nc.dram_tensor("y", [NSLOT, P, 4, D_MODEL], bf16, kind="ExternalOutput")

    last_wave = _check_waves()
    # output-projection ops (j, i): slot0 -> waves 2,3; slot1 -> 5,6;
    # slot2 -> wave 6 (where DVE is light: slot-3 chunks kc<24 need no
    # masks); slot3 -> tail (wave index NWAVE)
    out_sched = {wi: [] for wi in range(NWAVE + 1)}
    for j, tgt in ((0, (2, 2)), (1, (5, 6)), (2, (6, 6)), (3, (8, 8))):
        for i in range(8):
            out_sched[tgt[i // 4]].append((j, i))

    with tile.TileContext(nc) as tc:
        with (
            tc.tile_pool(name="singles", bufs=1) as singles,
            tc.tile_pool(name="work", bufs=10) as work,
            tc.tile_pool(name="ypool", bufs=2) as ypool,
            tc.tile_pool(name="mm_ps", bufs=2, space="PSUM") as mm_ps,
            tc.tile_pool(name="s_ps", bufs=3, space="PSUM") as s_ps_pool,
            tc.tile_pool(name="pv_ps", bufs=1, space="PSUM") as pv_pool,
        ):
            # ---- persistent SBUF ----
            w_sb = singles.tile([P, DCH * 192 + 4], bf16, tag="w_sb")
            xt_sb = singles.tile([P, SEQ, DCH], bf16, tag="xt_sb")
            msk_sb = singles.tile([P, 4, 512], bf16, tag="msk_sb")
            wo_sb = singles.tile([D_HEAD, D_MODEL], bf16, tag="wo_sb")
            kvt = singles.tile([P, SEQ], bf16, tag="kvt")  # 0:64 K^T, 64:128 V^T
            qt_sb = singles.tile([D_HEAD, NQ], bf16, tag="qt_sb")
            vaug = singles.tile([P, 32, D_HEAD + 1], bf16, tag="vaug")
            ot = singles.tile([D_HEAD + 1, NQ], bf16, tag="ot")
            rden = singles.tile([1, NQ], fp32, tag="rden")
            rbias = singles.tile([P, 1], fp32, tag="rbias")
            rdent = singles.tile([P, 16], fp32, tag="rdent")
            ident = singles.tile([P, D_HEAD], bf16, tag="ident")
            one_sb = singles.tile([1, 1], fp32, tag="one_sb")

            # ---- input DMAs (SP queue, ordered by first use) ----
            def ld_x(dst, src, s0):
                nc.sync.dma_start(
                    out=dst[:, s0 : s0 + 512, :], in_=src[:, s0 : s0 + 512, :]
                )

            def ld_xh(dst, src, s0, n=512):
                nc.sync.dma_start(
                    out=dst[:, s0 : s0 + n, :], in_=src[:, s0 : s0 + n, :]
                )

            nc.sync.dma_start(out=w_sb[:, 0:512], in_=w[:, 0:512])
            ld_xh(xt_sb, xt, 0, 128)
            ld_xh(xt_sb, xt, 128, 128)
            ld_xh(xt_sb, xt, 256, 128)
            nc.sync.dma_start(out=w_sb[:, 512:1536], in_=w[:, 512:1536])
            ld_xh(xt_sb, xt, 384, 128)
            nc.sync.dma_start(out=w_sb[:, 1536:1540], in_=w[:, 1536:1540])
            ld_x(xt_sb, xt, 512)       # chunk 1: wave-1 fresh keys
            ld_x(xt_sb, xt, 1024)      # chunk 2: slot-1 queries
            ld_x(xt_sb, xt, 2048)      # chunk 4: slot-2 queries
            nc.sync.dma_start(out=wo_sb, in_=wo[:, :])
            ld_x(xt_sb, xt, 1536)      # chunk 3
            ld_x(xt_sb, xt, 3072)      # chunk 6: slot-3 queries
            ld_x(xt_sb, xt, 2560)      # chunk 5
            ld_x(xt_sb, xt, 3584)      # chunk 7

            nc.vector.memset(one_sb, 1.0)
            nc.vector.memset(vaug[:, :, D_HEAD : D_HEAD + 1], 1.0)
            make_identity(nc, ident[D_HEAD:P, :])
            # fp32 per-partition exp bias (see w layout note)
            nc.vector.tensor_copy(out=rbias, in_=w_sb[:, 1536:1537])
            # causal band masks generated on the idle Pool engine:
            # msk_sb[p, c, jq] = 1 if 128c + p <= jq else 0, per band c
            for c in range(4):
                nc.gpsimd.memset(msk_sb[:, c, :], 1.0)
                nc.gpsimd.affine_select(
                    out=msk_sb[:, c, :],
                    in_=msk_sb[:, c, :],
                    compare_op=mybir.AluOpType.is_ge,
                    fill=0.0,
                    base=-128 * c,
                    channel_multiplier=-1,
                    pattern=[[1, 512]],
                )

            def kv_proj_mms(sc):
                """Generator: one KV-projection matmul per next() call."""
                kp = mm_ps.tile([P, 512], fp32, tag="mm")
                for dc in range(DCH):
                    nc.tensor.matmul(
                        kp,
                        lhsT=w_sb[:, 512 + dc * 128 : 512 + dc * 128 + 128],
                        rhs=xt_sb[:, sc * 512 : (sc + 1) * 512, dc],
                        start=(dc == 0),
                        stop=(dc == DCH - 1),
                    )
                    yield
                nc.vector.tensor_copy(
                    out=kvt[:, sc * 512 : (sc + 1) * 512], in_=kp
                )
                yield

            def transposes(sc):
                for t in range(4):  # V^T 128-col blocks -> natural V chunks
                    kc = sc * 4 + t
                    tp = s_ps_pool.tile([P, D_HEAD], bf16, tag="s_ps")
                    nc.tensor.transpose(
                        tp,
                        kvt[D_HEAD:P, kc * P : (kc + 1) * P],
                        ident[D_HEAD:P, :],
                    )
                    nc.vector.tensor_copy(out=vaug[:, kc, :D_HEAD], in_=tp)

            # PV accumulators: full-bank tiles, PV uses rows 0:65.
            # Slot 3 shares slot 0's bank: slot-0 accumulation ends in wave
            # 1 and its out-ops are confined to wave 2, while slot-3
            # accumulation starts in wave 3 (start=True clears the bank).
            pv = [
                pv_pool.tile([P, 512], fp32, tag=f"pv{g}", name=f"pv{g}")
                for g in range(3)
            ]
            pv.append(pv[0])
            freed = []  # pv banks released by finalized slots
            y_tiles = {}
            ncopy = [0]
            pending_pv = []

            def emit_pv(j, kc):
                nc.tensor.matmul(
                    pv[j][0 : D_HEAD + 1, :],
                    lhsT=vaug[:, kc, :],
                    rhs=pending_pv_pt.pop((j, kc)),
                    start=(kc == 0),
                    stop=(kc == E[j] - 1),
                    skip_group_check=True,
                )

            pending_pv_pt = {}

            def emit_chunk(j, kc, wv=0):
                sps = s_ps_pool.tile([P, 512], fp32, tag="s_ps")
                nc.tensor.matmul(
                    sps,
                    lhsT=kvt[0:D_HEAD, kc * P : (kc + 1) * P],
                    rhs=qt_sb[:, j * 512 : (j + 1) * 512],
                    start=True,
                    stop=True,
                )
                p_t = work.tile([P, 512], bf16, tag="p_t")
                band = kc - (E[j] - 8)
                if band >= 4:
                    # partner block: role-0 kills the whole chunk via the
                    # exp bias (exp(s - 30) ~ 1e-10); role 1 keeps it whole
                    nc.scalar.activation(
                        p_t, sps, mybir.ActivationFunctionType.Exp,
                        bias=rbias[:, 0:1],
                    )
                else:
                    nc.scalar.activation(
                        p_t, sps, mybir.ActivationFunctionType.Exp
                    )
                if 0 <= band < 4:
                    # diagonal band: per-query causal step mask
                    nc.vector.tensor_tensor(
                        p_t, p_t, msk_sb[:, band, :], mybir.AluOpType.mult
                    )
                pending_pv_pt[(j, kc)] = p_t
                pending_pv.append((j, kc))
                if len(pending_pv) > 10:
                    emit_pv(*pending_pv.pop(0))

            def emit_out_op(j, i, bank=None):
                """One output-projection matmul + scaled PSUM->SBUF copy."""
                t, no = i // 2, i % 2
                if bank is None:
                    bank = freed[ncopy[0] % len(freed)]
                    ncopy[0] += 1
                q0 = j * 512 + t * P
                nc.tensor.matmul(
                    bank,
                    lhsT=ot[0:D_HEAD, q0 : q0 + P],
                    rhs=wo_sb[:, no * 512 : (no + 1) * 512],
                    start=True,
                    stop=True,
                )
                if j == 3 and i % 2 == 0:
                    nc.scalar.mul(
                        y_tiles[j][:, t, no * 512 : (no + 1) * 512],
                        bank,
                        rdent[:, 4 * j + t : 4 * j + t + 1],
                    )
                else:
                    nc.vector.tensor_scalar_mul(
                        y_tiles[j][:, t, no * 512 : (no + 1) * 512],
                        bank,
                        rdent[:, 4 * j + t : 4 * j + t + 1],
                    )
                if j == 3:
                    if i % 2 == 1:
                        # per-tile DMA: few enough that HWDGE desc-gen
                        # (625ns each, serialized) stays off the tail path
                        nc.sync.dma_start(
                            out=y[j][:, t : t + 1, :],
                            in_=y_tiles[j][:, t : t + 1, :],
                        )
                elif i == 3:
                    nc.sync.dma_start(
                        out=y[j][:, 0:2, :], in_=y_tiles[j][:, 0:2, :]
                    )
                elif i == 7:
                    nc.sync.dma_start(
                        out=y[j][:, 2:4, :], in_=y_tiles[j][:, 2:4, :]
                    )

            def finalize_half(j, h):
                """Half of slot-j finalize: O^T + den, 1/den, rdent cols."""
                c0 = j * 512 + h * 256
                c1 = c0 + 256
                nc.vector.tensor_copy(
                    out=ot[:, c0:c1], in_=pv[j][0 : D_HEAD + 1, h * 256 : h * 256 + 256]
                )
                nc.vector.reciprocal(
                    rden[:, c0:c1], ot[D_HEAD : D_HEAD + 1, c0:c1]
                )
                for t in (2 * h, 2 * h + 1):
                    nc.tensor.matmul(
                        pv[j][:, t : t + 1],
                        lhsT=rden[:, j * 512 + t * P : j * 512 + (t + 1) * P],
                        rhs=one_sb,
                        start=True,
                        stop=True,
                    )
                nc.vector.tensor_copy(
                    out=rdent[:, 4 * j + 2 * h : 4 * j + 2 * h + 2],
                    in_=pv[j][:, 2 * h : 2 * h + 2],
                )

            def finalize(j):
                # consolidated (same shape as the slot-3 tail): full-width
                # ot copy + reciprocal reading the pv PSUM bank directly,
                # then the four rdent transposes and one rdent copy
                nc.vector.tensor_copy(
                    out=ot[0:D_HEAD, j * 512 : (j + 1) * 512],
                    in_=pv[j][0:D_HEAD, :],
                )
                nc.vector.reciprocal(
                    rden[:, j * 512 : (j + 1) * 512],
                    pv[j][D_HEAD : D_HEAD + 1, :],
                )
                for t in range(4):
                    nc.tensor.matmul(
                        pv[j][:, t : t + 1],
                        lhsT=rden[:, j * 512 + t * P : j * 512 + (t + 1) * P],
                        rhs=one_sb,
                        start=True,
                        stop=True,
                    )
                nc.vector.tensor_copy(
                    out=rdent[:, 4 * j : 4 * j + 4], in_=pv[j][:, 0:4]
                )
                freed.append(pv[j])
                y_tiles[j] = ypool.tile(
                    [P, 4, D_MODEL], bf16, tag="y_sb", name=f"y{j}"
                )

            def q_piece(c0, n):
                """Prologue Q projection over columns [c0, c0+n) of slot 0.
                Uses the s_ps pool so pieces rotate PSUM banks instead of
                serializing on the single mm bank."""
                qp = s_ps_pool.tile([D_HEAD, n], fp32, tag="s_ps", name=f"qp{c0}")
                for dc in range(DCH):
                    nc.tensor.matmul(
                        qp,
                        lhsT=w_sb[:, dc * 64 : dc * 64 + 64],
                        rhs=xt_sb[:, c0 : c0 + n, dc],
                        start=(dc == 0),
                        stop=(dc == DCH - 1),
                    )
                nc.vector.tensor_copy(out=qt_sb[:, c0 : c0 + n], in_=qp)

            def kv_piece(c0, n):
                kp = s_ps_pool.tile([P, n], fp32, tag="s_ps", name=f"kp{c0}")
                for dc in range(DCH):
                    nc.tensor.matmul(
                        kp,
                        lhsT=w_sb[:, 512 + dc * 128 : 512 + dc * 128 + 128],
                        rhs=xt_sb[:, c0 : c0 + n, dc],
                        start=(dc == 0),
                        stop=(dc == DCH - 1),
                    )
                nc.vector.tensor_copy(out=kvt[:, c0 : c0 + n], in_=kp)

            def q_proj_steps(j):
                """Generator version of q_proj: one matmul per next()."""
                qp = mm_ps.tile([D_HEAD, 512], fp32, tag="mm")
                for dc in range(DCH):
                    nc.tensor.matmul(
                        qp,
                        lhsT=w_sb[:, dc * 64 : dc * 64 + 64],
                        rhs=xt_sb[:, j * 1024 : j * 1024 + 512, dc],
                        start=(dc == 0),
                        stop=(dc == DCH - 1),
                    )
                    yield
                nc.vector.tensor_copy(
                    out=qt_sb[:, j * 512 : (j + 1) * 512], in_=qp
                )
                yield

            def transpose_steps(sc):
                tp = s_ps_pool.tile([P, 4, D_HEAD], bf16, tag="s_ps", name="tp4")
                for t in range(4):
                    kc = sc * 4 + t
                    nc.tensor.transpose(
                        tp[:, t, :],
                        kvt[D_HEAD:P, kc * P : (kc + 1) * P],
                        ident[D_HEAD:P, :],
                    )
                    yield
                nc.vector.tensor_copy(
                    out=vaug[:, sc * 4 : sc * 4 + 4, :D_HEAD], in_=tp
                )
                yield

            # Q projection for slot j runs as filler inside wave j (its
            # first consumers are that wave's chunks)
            qproj_wave = {1: 1, 2: 2, 3: 3}

            # ---- prologue + wave 0: piece-width projections matched to the
            # DMA bite arrival order (x cols 0:128, 128:384, 384:512). The
            # V-transposes for kc 0..1 must be emitted before chunk (0,3)
            # pops PV(0,0), else the vaug dependency is never recorded. ----
            tg0 = transpose_steps(0)
            q_piece(0, 128)
            q_piece(128, 128)
            q_piece(256, 128)
            kv_piece(0, 128)
            kv_piece(128, 256)
            q_piece(384, 128)
            kv_piece(384, 128)
            next(tg0)
            next(tg0)
            emit_chunk(0, 0, 0)
            emit_chunk(0, 1, 0)
            next(tg0)
            next(tg0)
            emit_chunk(0, 2, 0)
            emit_chunk(0, 3, 0)
            for _ in tg0:  # drain: emits the packed vaug copy for kc 0..3
                pass

            # ---- streamed waves ----
            # Wave sc fillers: [qproj (if due), KV proj for sc, spacer,
            # V transposes for sc], popped two per chunk. Wave 0's proj and
            # transposes ran in the prologue.
            for sc in range(1, NWAVE):
                chunks = WAVES[sc]
                oo = list(out_sched[sc])
                nq = 9 if sc in qproj_wave else 0
                # kv projection FIRST: both share the single mm PSUM bank,
                # and the q projection may wait on later-arriving x columns
                # — allocated first it would block the kv chain via WAR
                filler = [
                    kv_proj_mms(sc),
                    iter([None, None]),  # spacer: kvt copy drains
                    transpose_steps(sc),
                ]
                if nq:
                    filler.append(q_proj_steps(qproj_wave[sc]))
                total_fill = nq + 16
                fill_iter = (x for g in filler for x in g)
                pops = [0]

                def pop_fill(upto=None, k=None):
                    tgt = upto if upto is not None else pops[0] + k
                    while pops[0] < min(tgt, total_fill):
                        if next(fill_iter, -1) == -1:
                            pops[0] = total_fill
                            break
                        pops[0] += 1

                # emission-order safety points: a fresh chunk's scores need
                # this wave's kvt copy emitted; a fresh chunk's PV needs its
                # V-transpose emitted
                n = len(chunks)
                for ci, (j, kc) in enumerate(chunks):
                    if kc // 4 == sc:
                        pop_fill(upto=9)
                    if sc in qproj_wave and j == qproj_wave[sc]:
                        pop_fill(upto=total_fill)
                    if len(pending_pv) >= 4:
                        j2, kc2 = pending_pv[0]
                        if kc2 // 4 == sc:
                            pop_fill(upto=total_fill)
                    emit_chunk(j, kc, sc)
                    if oo and ci % 2 == 1:
                        emit_out_op(*oo.pop(0))
                    k = -(-(total_fill - pops[0]) // (n - ci))  # ceil
                    pop_fill(k=min(k, 4))
                pop_fill(upto=total_fill)
                for op in oo:
                    emit_out_op(*op)
                if sc == 2:
                    # pv[0] now belongs to slot-3 accumulation
                    freed.clear()
                fin = [j for j in range(NSLOT) if last_wave[j] == sc and j != 3]
                if fin:
                    while pending_pv:
                        emit_pv(*pending_pv.pop(0))
                    for j in fin:
                        finalize(j)

            # ---- slot-3 tail: per-128-query-tile pipeline. The ot copy and
            # the reciprocal both read the pv PSUM bank directly (no serial
            # copy->recip dependency), then rdent -> out-proj x2 -> scaled
            # copy (DVE half, ACT half) -> per-tile y DMA, so successive
            # tiles overlap across engines. ----
            while pending_pv:
                emit_pv(*pending_pv.pop(0))
            y_tiles[3] = ypool.tile([P, 4, D_MODEL], bf16, tag="y_sb", name="y3")

            def finalize_q(j, t):
                """Per-128-query finalize: reciprocal reads the pv PSUM
                bank directly; the ot copy is one full-width ACT op emitted
                by the caller."""
                c0 = j * 512 + t * P
                nc.vector.reciprocal(
                    rden[:, c0 : c0 + P],
                    pv[j][D_HEAD : D_HEAD + 1, t * P : (t + 1) * P],
                )
                nc.tensor.matmul(
                    pv[j][:, t : t + 1],
                    lhsT=rden[:, c0 : c0 + P],
                    rhs=one_sb,
                    start=True,
                    stop=True,
                )
                nc.vector.tensor_copy(
                    out=rdent[:, 4 * j + t : 4 * j + t + 1],
                    in_=pv[j][:, t : t + 1],
                )

            # emission order keeps each engine FIFO unblocked: the recip
            # chains for tiles 0-2 all precede the first scale op. Each
            # out-op gets its own PSUM bank (score + mm banks are idle by
            # now) so no matmul waits on a previous scale's drain.
            tail_banks = [
                s_ps_pool.tile([P, 512], fp32, tag="s_ps", name=f"tb{k}")
                for k in range(3)
            ] + [
                mm_ps.tile([P, 512], fp32, tag="mm", name=f"tb{k + 3}")
                for k in range(2)
            ]
            nc.vector.tensor_copy(
                out=ot[0:D_HEAD, 1536:2048], in_=pv[3][0:D_HEAD, :]
            )
            nc.vector.reciprocal(
                rden[:, 1536:2048], pv[3][D_HEAD : D_HEAD + 1, :]
            )
            for t in range(4):
                nc.tensor.matmul(
                    pv[3][:, t : t + 1],
                    lhsT=rden[:, 1536 + t * P : 1536 + (t + 1) * P],
                    rhs=one_sb,
                    start=True,
                    stop=True,
                )
            nc.vector.tensor_copy(out=rdent[:, 12:16], in_=pv[3][:, 0:4])
            emit_out_op(3, 0, bank=tail_banks[0])
            emit_out_op(3, 1, bank=tail_banks[1])
            emit_out_op(3, 2, bank=tail_banks[2])
            emit_out_op(3, 3, bank=tail_banks[3])
            emit_out_op(3, 4, bank=tail_banks[4])
            emit_out_op(3, 5, bank=pv[1])
            emit_out_op(3, 6, bank=pv[2])
            # pv[3] frees as soon as the consolidated rdent copy has read
            # cols 0:4 (~right after the last PV), earlier than any scale
            emit_out_op(3, 7, bank=pv[3])

    nc.finalize()
    return nc


def _get_program():
    global _prog
    if _prog is None:
        _prog = _build_program()
    return _prog


def kernel(x, W_q, W_k, W_v, W_o):
    import ml_dtypes
    from concourse.bass_utils import run_bass_kernel_spmd

    bf = ml_dtypes.bfloat16
    nc = _get_program()

    x = np.asarray(x, dtype=np.float32)
    scale = np.float32(1.0 / np.sqrt(D_HEAD))
    wq_s = np.asarray(W_q, dtype=np.float32) * scale
    wkv = np.concatenate(
        [np.asarray(W_k, dtype=np.float32), np.asarray(W_v, dtype=np.float32)],
        axis=1,
    )  # [1024, 128]
    wq_part = wq_s.reshape(DCH, P, 64).transpose(1, 0, 2).reshape(P, 512)
    wkv_part = wkv.reshape(DCH, P, 128).transpose(1, 0, 2).reshape(P, 1024)
    # per-role exp bias for partner-band chunks: -30 kills the whole chunk
    # for role 0 (exp(s - 30) ~ 1e-10), 0 keeps it whole for role 1
    w_hosts = []
    for r in range(2):
        rb = np.full((P, 4), 0.0 if r == 1 else -30.0, dtype=np.float32)
        w_hosts.append(
            np.ascontiguousarray(
                np.concatenate([wq_part, wkv_part, rb], axis=1)
            ).astype(bf)
        )  # [128, 1540]
    wo_host = np.ascontiguousarray(np.asarray(W_o, dtype=np.float32)).astype(bf)

    in_maps = []
    for c in range(NCORES):
        b, r = c // 2, c % 2
        xt_b = x[b].T  # [1024, 4096]
        # permuted key order: position 2k holds this core's query block
        # 2k+r, position 2k+1 holds the partner block 2k+(1-r)
        cols = np.concatenate(
            [
                np.arange(512 * g, 512 * g + 512)
                for k in range(NSLOT)
                for g in (2 * k + r, 2 * k + 1 - r)
            ]
        )
        xt_host = np.ascontiguousarray(
            xt_b[:, cols].reshape(DCH, P, SEQ).transpose(1, 2, 0)
        ).astype(bf)  # [128, 4096, 8]
        in_maps.append(
            {
                "xt": xt_host,
                "w": w_hosts[r],
                "wo": wo_host,
            }
        )

    res = run_bass_kernel_spmd(nc, in_maps, core_ids=list(range(NCORES)))
    out = np.empty((BATCH, SEQ, D_MODEL), dtype=np.float32)
    for c in range(NCORES):
        b, r = c // 2, c % 2
        yv = np.asarray(res.results[c]["y"]).astype(np.float32)
        # y[j, p, t, :] -> query 512*(2j+r) + 128t + p
        yv = yv.transpose(0, 2, 1, 3)  # [j, t, p, m]
        for j in range(NSLOT):
            q0 = 512 * (2 * j + r)
            out[b, q0 : q0 + 512, :] = yv[j].reshape(512, D_MODEL)
    return out



# revision 135
# speedup vs baseline: 1.0683x; 1.0070x over previous
"""Trainium2 Bass kernel for single-head causal attention.

x:[4,4096,1024] f32, W_q/W_k/W_v:[1024,64], W_o:[64,1024].

Sharding: 8 cores = 4 batches x 2 query-stripe roles. Role r of a batch
owns query blocks {2j+r : j=0..3} (512 queries each). Program slot j has
key extent E[j] = (8j+8) 128-key chunks, which exactly covers role 1's
block 2j+1 and over-covers role 0's block 2j by 4 chunks (dead).

All per-core differences (which batch, which stripe, dead chunks) are
carried in the input data; one SPMD program runs on all 8 cores:
- x is shipped as [128, 4096, 8] (d_model-chunk partition, seq, chunk)
  so any column range is a fully contiguous DMA (no sub-512B-element
  descriptor penalty), streamed in arrival-ordered bites.
- The diagonal causal band masks (4 x [128,512]) are generated on the
  otherwise-idle Pool engine with affine_select, not DMAed.
- Partner-block chunks (band >= 4) take no mask op at all: the exp runs
  with a per-partition bias from the w tensor (-30 for role 0 => p ~
  1e-10, 0 for role 1), so role-0's dead chunks vanish from both the PV
  numerator and the denominator row.

bf16 everywhere on the matmul paths (end-to-end rel err ~5e-3 vs the
2e-2 gate). Engines: PE does all matmuls (62.3us busy, the roofline of
this schedule), ACT does exp + the slot-3 ot copies + half its output
scales, DVE does diag masks + kvt/qt copies + finalize + the other
scales. Attention chunks are emitted in waves matched to DMA arrival,
next wave's KV projection interleaved between chunks as PE filler, PV
matmuls lagged 10 chunks behind their scores. Waves are front-loaded
(each chunk sits in the earliest wave whose x data can feed it) with
the KV projection emitted before the Q projection inside each wave's
filler — they share the single mm PSUM bank and Q may wait on
later-arriving x, so allocated first it would block the KV chain via
WAR. Every slot finalize is consolidated into full-width ops (one
[64,512] ot copy, one [1,512] reciprocal, four 1-col rdent transposes,
one [P,4] rdent copy, reading the PV PSUM bank directly) so all eight
out-projection matmuls of a slot unblock at once; tail out-ops get
distinct PSUM banks and the y DMAs go out per 128-query tile to keep
the final HWDGE descriptor-gen chain (625ns each, serialized) short.
"""

import sys

for _p in ("/opt/trn_rl_repo",):
    if _p not in sys.path:
        sys.path.insert(0, _p)

import numpy as np

D_MODEL = 1024
D_HEAD = 64
SEQ = 4096
BATCH = 4
NCORES = 8
NQ = 2048          # queries per core
P = 128
DCH = D_MODEL // P  # 8 contraction chunks
NSLOT = 4           # query slots of 512
E = [8, 16, 24, 32]  # key chunks per slot
NWAVE = 8           # key superchunks of 512

# Attention chunks per wave: matched to DMA arrival order (early waves
# light), per-slot ascending kc, every chunk (j,kc) in wave >= kc//4.
WAVES = [
    [(0, k) for k in range(4)],
    [(0, k) for k in range(4, 8)] + [(1, k) for k in range(8)],
    [(1, k) for k in range(8, 12)] + [(2, k) for k in range(12)],
    [(1, k) for k in range(12, 16)] + [(3, k) for k in range(4)],
    [(2, k) for k in range(12, 16)] + [(3, k) for k in range(4, 12)],
    [(2, k) for k in range(16, 24)],
    [(3, k) for k in range(12, 16)],
    [(3, k) for k in range(16, 32)],
]

_prog = None


def _check_waves():
    seen = {}
    total = 0
    for w, wv in enumerate(WAVES):
        for j, kc in wv:
            assert kc // 4 <= w, (w, j, kc)
            assert seen.get(j, -1) == kc - 1, (j, kc)
            seen[j] = kc
            total += 1
    assert total == sum(E) == 80
    return {j: max(w for w, wv in enumerate(WAVES) if (j, E[j] - 1) in wv)
            for j in range(NSLOT)}


def _build_program():
    import concourse.bacc as bacc
    import concourse.mybir as mybir
    import concourse.tile as tile
    from concourse.masks import make_identity

    fp32 = mybir.dt.float32
    f32r = mybir.dt.float32r
    bf16 = mybir.dt.bfloat16
    nc = bacc.Bacc("TRN2", target_bir_lowering=False, debug=False)

    # w layout: [wq 0:512 | wkv 512:1536 | rbias 1536:1540]
    # rbias: -30 for role 0, 0 for role 1. Partner-band chunks (band >= 4)
    # run exp with this per-partition bias instead of a 0/1 mask multiply:
    # each slot's band region only ever covers its own partner block, which
    # is entirely dead for role 0 (exp(s - 30) ~ 1e-10) and entirely alive
    # for role 1 (bias 0). Replaces the 16 explicit partner-mask DVE ops.
    xt = nc.dram_tensor("xt", [P, SEQ, DCH], bf16, kind="ExternalInput")
    w = nc.dram_tensor("w", [P, DCH * 192 + 4], bf16, kind="ExternalInput")
    wo = nc.dram_tensor("wo", [D_HEAD, D_MODEL], bf16, kind="ExternalInput")
    y = nc.dram_tensor("y", [NSLOT, P, 4, D_MODEL], bf16, kind="ExternalOutput")

    last_wave = _check_waves()
    # output-projection ops (j, i): slot0 -> waves 2,3; slot1 -> 5,6;
    # slot2 -> wave 6 (where DVE is light: slot-3 chunks kc<24 need no
    # masks); slot3 -> tail (wave index NWAVE)
    out_sched = {wi: [] for wi in range(NWAVE + 1)}
    for j, tgt in ((0, (2, 2)), (1, (5, 6)), (2, (6, 6)), (3, (8, 8))):
        for i in range(8):
            out_sched[tgt[i // 4]].append((j, i))

    with tile.TileContext(nc) as tc:
        with (
            tc.tile_pool(name="singles", bufs=1) as singles,
            tc.tile_pool(name="work", bufs=10) as work,
            tc.tile_pool(name="ypool", bufs=2) as ypool,
            tc.tile_pool(name="mm_ps", bufs=2, space="PSUM") as mm_ps,
            tc.tile_pool(name="s_ps", bufs=3, space="PSUM") as s_ps_pool,
            tc.tile_pool(name="pv_ps", bufs=1, space="PSUM") as pv_pool,
        ):
            # ---- persistent SBUF ----
            w_sb = singles.tile([P, DCH * 192 + 4], bf16, tag="w_sb")
            xt_sb = singles.tile([P, SEQ, DCH], bf16, tag="xt_sb")
            msk_sb = singles.tile([P, 4, 512], bf16, tag="msk_sb")
            wo_sb = singles.tile([D_HEAD, D_MODEL], bf16, tag="wo_sb")
            kvt = singles.tile([P, SEQ], bf16, tag="kvt")  # 0:64 K^T, 64:128 V^T
            qt_sb = singles.tile([D_HEAD, NQ], bf16, tag="qt_sb")
            vaug = singles.tile([P, 32, D_HEAD + 1], bf16, tag="vaug")
            ot = singles.tile([D_HEAD + 1, NQ], bf16, tag="ot")
            rden = singles.tile([1, NQ], fp32, tag="rden")
            rbias = singles.tile([P, 1], fp32, tag="rbias")
            rdent = singles.tile([P, 16], fp32, tag="rdent")
            ident = singles.tile([P, D_HEAD], bf16, tag="ident")
            one_sb = singles.tile([1, 1], fp32, tag="one_sb")

            # ---- input DMAs (SP queue, ordered by first use) ----
            def ld_x(dst, src, s0):
                nc.sync.dma_start(
                    out=dst[:, s0 : s0 + 512, :], in_=src[:, s0 : s0 + 512, :]
                )

            def ld_xh(dst, src, s0, n=512):
                nc.sync.dma_start(
                    out=dst[:, s0 : s0 + n, :], in_=src[:, s0 : s0 + n, :]
                )

            nc.sync.dma_start(out=w_sb[:, 0:512], in_=w[:, 0:512])
            ld_xh(xt_sb, xt, 0, 128)
            ld_xh(xt_sb, xt, 128, 128)
            ld_xh(xt_sb, xt, 256, 128)
            nc.sync.dma_start(out=w_sb[:, 512:1536], in_=w[:, 512:1536])
            ld_xh(xt_sb, xt, 384, 128)
            nc.sync.dma_start(out=w_sb[:, 1536:1540], in_=w[:, 1536:1540])
            ld_x(xt_sb, xt, 512)       # chunk 1: wave-1 fresh keys
            ld_x(xt_sb, xt, 1024)      # chunk 2: slot-1 queries
            ld_x(xt_sb, xt, 2048)      # chunk 4: slot-2 queries
            nc.sync.dma_start(out=wo_sb, in_=wo[:, :])
            ld_x(xt_sb, xt, 1536)      # chunk 3
            ld_x(xt_sb, xt, 3072)      # chunk 6: slot-3 queries
            ld_x(xt_sb, xt, 2560)      # chunk 5
            ld_x(xt_sb, xt, 3584)      # chunk 7

            nc.vector.memset(one_sb, 1.0)
            nc.vector.memset(vaug[:, :, D_HEAD : D_HEAD + 1], 1.0)
            make_identity(nc, ident[D_HEAD:P, :])
            # fp32 per-partition exp bias (see w layout note)
            nc.vector.tensor_copy(out=rbias, in_=w_sb[:, 1536:1537])
            # causal band masks generated on the idle Pool engine:
            # msk_sb[p, c, jq] = 1 if 128c + p <= jq else 0, per band c
            for c in range(4):
                nc.gpsimd.memset(msk_sb[:, c, :], 1.0)
                nc.gpsimd.affine_select(
                    out=msk_sb[:, c, :],
                    in_=msk_sb[:, c, :],
                    compare_op=mybir.AluOpType.is_ge,
                    fill=0.0,
                    base=-128 * c,
                    channel_multiplier=-1,
                    pattern=[[1, 512]],
                )

            def kv_proj_mms(sc):
                """Generator: one KV-projection matmul per next() call."""
                kp = mm_ps.tile([P, 512], fp32, tag="mm")
                for dc in range(DCH):
                    nc.tensor.matmul(
                        kp,
                        lhsT=w_sb[:, 512 + dc * 128 : 512 + dc * 128 + 128],
                        rhs=xt_sb[:, sc * 512 : (sc + 1) * 512, dc],
                        start=(dc == 0),
                        stop=(dc == DCH - 1),
                    )
                    yield
                nc.vector.tensor_copy(
                    out=kvt[:, sc * 512 : (sc + 1) * 512], in_=kp
                )
                yield

            def transposes(sc):
                for t in range(4):  # V^T 128-col blocks -> natural V chunks
                    kc = sc * 4 + t
                    tp = s_ps_pool.tile([P, D_HEAD], bf16, tag="s_ps")
                    nc.tensor.transpose(
                        tp,
                        kvt[D_HEAD:P, kc * P : (kc + 1) * P],
                        ident[D_HEAD:P, :],
                    )
                    nc.vector.tensor_copy(out=vaug[:, kc, :D_HEAD], in_=tp)

            # PV accumulators: full-bank tiles, PV uses rows 0:65.
            # Slot 3 shares slot 0's bank: slot-0 accumulation ends in wave
            # 1 and its out-ops are confined to wave 2, while slot-3
            # accumulation starts in wave 3 (start=True clears the bank).
            pv = [
                pv_pool.tile([P, 512], fp32, tag=f"pv{g}", name=f"pv{g}")
                for g in range(3)
            ]
            pv.append(pv[0])
            freed = []  # pv banks released by finalized slots
            y_tiles = {}
            ncopy = [0]
            pending_pv = []

            def emit_pv(j, kc):
                nc.tensor.matmul(
                    pv[j][0 : D_HEAD + 1, :],
                    lhsT=vaug[:, kc, :],
                    rhs=pending_pv_pt.pop((j, kc)),
                    start=(kc == 0),
                    stop=(kc == E[j] - 1),
                    skip_group_check=True,
                )

            pending_pv_pt = {}

            def emit_chunk(j, kc, wv=0):
                sps = s_ps_pool.tile([P, 512], fp32, tag="s_ps")
                nc.tensor.matmul(
                    sps,
                    lhsT=kvt[0:D_HEAD, kc * P : (kc + 1) * P],
                    rhs=qt_sb[:, j * 512 : (j + 1) * 512],
                    start=True,
                    stop=True,
                )
                p_t = work.tile([P, 512], bf16, tag="p_t")
                band = kc - (E[j] - 8)
                if band >= 4:
                    # partner block: role-0 kills the whole chunk via the
                    # exp bias (exp(s - 30) ~ 1e-10); role 1 keeps it whole
                    nc.scalar.activation(
                        p_t, sps, mybir.ActivationFunctionType.Exp,
                        bias=rbias[:, 0:1],
                    )
                else:
                    nc.scalar.activation(
                        p_t, sps, mybir.ActivationFunctionType.Exp
                    )
                if 0 <= band < 4:
                    # diagonal band: per-query causal step mask
                    nc.vector.tensor_tensor(
                        p_t, p_t, msk_sb[:, band, :], mybir.AluOpType.mult
                    )
                pending_pv_pt[(j, kc)] = p_t
                pending_pv.append((j, kc))
                if len(pending_pv) > 10:
                    emit_pv(*pending_pv.pop(0))

            def emit_out_op(j, i, bank=None):
                """One output-projection matmul + scaled PSUM->SBUF copy."""
                t, no = i // 2, i % 2
                if bank is None:
                    bank = freed[ncopy[0] % len(freed)]
                    ncopy[0] += 1
                q0 = j * 512 + t * P
                nc.tensor.matmul(
                    bank,
                    lhsT=ot[0:D_HEAD, q0 : q0 + P],
                    rhs=wo_sb[:, no * 512 : (no + 1) * 512],
                    start=True,
                    stop=True,
                )
                if j == 3 and i % 2 == 0:
                    nc.scalar.mul(
                        y_tiles[j][:, t, no * 512 : (no + 1) * 512],
                        bank,
                        rdent[:, 4 * j + t : 4 * j + t + 1],
                    )
                else:
                    nc.vector.tensor_scalar_mul(
                        y_tiles[j][:, t, no * 512 : (no + 1) * 512],
                        bank,
                        rdent[:, 4 * j + t : 4 * j + t + 1],
                    )
                if j == 3:
                    if i % 2 == 1:
                        # per-tile DMA: few enough that HWDGE desc-gen
                        # (625ns each, serialized) stays off the tail path
                        nc.sync.dma_start(
                            out=y[j][:, t : t + 1, :],
                            in_=y_tiles[j][:, t : t + 1, :],
                        )
                elif i == 3:
                    nc.sync.dma_start(
                        out=y[j][:, 0:2, :], in_=y_tiles[j][:, 0:2, :]
                    )
                elif i == 7:
                    nc.sync.dma_start(
                        out=y[j][:, 2:4, :], in_=y_tiles[j][:, 2:4, :]
                    )

            def finalize_half(j, h):
                """Half of slot-j finalize: O^T + den, 1/den, rdent cols."""
                c0 = j * 512 + h * 256
                c1 = c0 + 256
                nc.vector.tensor_copy(
                    out=ot[:, c0:c1], in_=pv[j][0 : D_HEAD + 1, h * 256 : h * 256 + 256]
                )
                nc.vector.reciprocal(
                    rden[:, c0:c1], ot[D_HEAD : D_HEAD + 1, c0:c1]
                )
                for t in (2 * h, 2 * h + 1):
                    nc.tensor.matmul(
                        pv[j][:, t : t + 1],
                        lhsT=rden[:, j * 512 + t * P : j * 512 + (t + 1) * P],
                        rhs=one_sb,
                        start=True,
                        stop=True,
                    )
                nc.vector.tensor_copy(
                    out=rdent[:, 4 * j + 2 * h : 4 * j + 2 * h + 2],
                    in_=pv[j][:, 2 * h : 2 * h + 2],
                )

            def finalize(j):
                # consolidated (same shape as the slot-3 tail): full-width
                # ot copy + reciprocal reading the pv PSUM bank directly,
                # then the four rdent transposes and one rdent copy
                nc.vector.tensor_copy(
                    out=ot[0:D_HEAD, j * 512 : (j + 1) * 512],
                    in_=pv[j][0:D_HEAD, :],
                )
                nc.vector.reciprocal(
                    rden[:, j * 512 : (j + 1) * 512],
                    pv[j][D_HEAD : D_HEAD + 1, :],
                )
                for t in range(4):
                    nc.tensor.matmul(
                        pv[j][:, t : t + 1],
                        lhsT=rden[:, j * 512 + t * P : j * 512 + (t + 1) * P],
                        rhs=one_sb,
                        start=True,
                        stop=True,
                    )
                nc.vector.tensor_copy(
                    out=rdent[:, 4 * j : 4 * j + 4], in_=pv[j][:, 0:4]
                )
                freed.append(pv[j])
                y_tiles[j] = ypool.tile(
                    [P, 4, D_MODEL], bf16, tag="y_sb", name=f"y{j}"
                )

            def q_piece(c0, n):
                """Prologue Q projection over columns [c0, c0+n) of slot 0.
                Uses the s_ps pool so pieces rotate PSUM banks instead of
                serializing on the single mm bank."""
                qp = s_ps_pool.tile([D_HEAD, n], fp32, tag="s_ps", name=f"qp{c0}")
                for dc in range(DCH):
                    nc.tensor.matmul(
                        qp,
                        lhsT=w_sb[:, dc * 64 : dc * 64 + 64],
                        rhs=xt_sb[:, c0 : c0 + n, dc],
                        start=(dc == 0),
                        stop=(dc == DCH - 1),
                    )
                nc.vector.tensor_copy(out=qt_sb[:, c0 : c0 + n], in_=qp)

            def kv_piece(c0, n):
                kp = s_ps_pool.tile([P, n], fp32, tag="s_ps", name=f"kp{c0}")
                for dc in range(DCH):
                    nc.tensor.matmul(
                        kp,
                        lhsT=w_sb[:, 512 + dc * 128 : 512 + dc * 128 + 128],
                        rhs=xt_sb[:, c0 : c0 + n, dc],
                        start=(dc == 0),
                        stop=(dc == DCH - 1),
                    )
                nc.vector.tensor_copy(out=kvt[:, c0 : c0 + n], in_=kp)

            def q_proj_steps(j):
                """Generator version of q_proj: one matmul per next()."""
                qp = mm_ps.tile([D_HEAD, 512], fp32, tag="mm")
                for dc in range(DCH):
                    nc.tensor.matmul(
                        qp,
                        lhsT=w_sb[:, dc * 64 : dc * 64 + 64],
                        rhs=xt_sb[:, j * 1024 : j * 1024 + 512, dc],
                        start=(dc == 0),
                        stop=(dc == DCH - 1),
                    )
                    yield
                nc.vector.tensor_copy(
                    out=qt_sb[:, j * 512 : (j + 1) * 512], in_=qp
                )
                yield

            def transpose_steps(sc):
                tp = s_ps_pool.tile([P, 4, D_HEAD], bf16, tag="s_ps", name="tp4")
                for t in range(4):
                    kc = sc * 4 + t
                    nc.tensor.transpose(
                        tp[:, t, :],
                        kvt[D_HEAD:P, kc * P : (kc + 1) * P],
                        ident[D_HEAD:P, :],
                    )
                    yield
                nc.vector.tensor_copy(
                    out=vaug[:, sc * 4 : sc * 4 + 4, :D_HEAD], in_=tp
                )
                yield

            # Q projection for slot j runs as filler inside wave j (its
            # first consumers are that wave's chunks)
            qproj_wave = {1: 1, 2: 2, 3: 3}

            # ---- prologue + wave 0: piece-width projections matched to the
            # DMA bite arrival order (x cols 0:128, 128:384, 384:512). The
            # V-transposes for kc 0..1 must be emitted before chunk (0,3)
            # pops PV(0,0), else the vaug dependency is never recorded. ----
            tg0 = transpose_steps(0)
            q_piece(0, 128)
            q_piece(128, 128)
            q_piece(256, 128)
            kv_piece(0, 128)
            kv_piece(128, 256)
            q_piece(384, 128)
            kv_piece(384, 128)
            next(tg0)
            next(tg0)
            emit_chunk(0, 0, 0)
            emit_chunk(0, 1, 0)
            next(tg0)
            next(tg0)
            emit_chunk(0, 2, 0)
            emit_chunk(0, 3, 0)
            for _ in tg0:  # drain: emits the packed vaug copy for kc 0..3
                pass

            # ---- streamed waves ----
            # Wave sc fillers: [qproj (if due), KV proj for sc, spacer,
            # V transposes for sc], popped two per chunk. Wave 0's proj and
            # transposes ran in the prologue.
            for sc in range(1, NWAVE):
                chunks = WAVES[sc]
                oo = list(out_sched[sc])
                nq = 9 if sc in qproj_wave else 0
                # kv projection FIRST: both share the single mm PSUM bank,
                # and the q projection may wait on later-arriving x columns
                # — allocated first it would block the kv chain via WAR
                filler = [
                    kv_proj_mms(sc),
                    iter([None, None]),  # spacer: kvt copy drains
                    transpose_steps(sc),
                ]
                if nq:
                    filler.append(q_proj_steps(qproj_wave[sc]))
                total_fill = nq + 16
                fill_iter = (x for g in filler for x in g)
                pops = [0]

                def pop_fill(upto=None, k=None):
                    tgt = upto if upto is not None else pops[0] + k
                    while pops[0] < min(tgt, total_fill):
                        if next(fill_iter, -1) == -1:
                            pops[0] = total_fill
                            break
                        pops[0] += 1

                # emission-order safety points: a fresh chunk's scores need
                # this wave's kvt copy emitted; a fresh chunk's PV needs its
                # V-transpose emitted
                n = len(chunks)
                for ci, (j, kc) in enumerate(chunks):
                    if kc // 4 == sc:
                        pop_fill(upto=9)
                    if sc in qproj_wave and j == qproj_wave[sc]:
                        pop_fill(upto=total_fill)
                    if len(pending_pv) >= 4:
                        j2, kc2 = pending_pv[0]
                        if kc2 // 4 == sc:
                            pop_fill(upto=total_fill)
                    emit_chunk(j, kc, sc)
                    if oo:
                        emit_out_op(*oo.pop(0))
                    k = -(-(total_fill - pops[0]) // (n - ci))  # ceil
                    pop_fill(k=min(k, 4))
                pop_fill(upto=total_fill)
                for op in oo:
                    emit_out_op(*op)
                if sc == 2:
                    # pv[0] now belongs to slot-3 accumulation
                    freed.clear()
                fin = [j for j in range(NSLOT) if last_wave[j] == sc and j != 3]
                if fin:
                    while pending_pv:
                        emit_pv(*pending_pv.pop(0))
                    for j in fin:
                        finalize(j)

            # ---- slot-3 tail: per-128-query-tile pipeline. The ot copy and
            # the reciprocal both read the pv PSUM bank directly (no serial
            # copy->recip dependency), then rdent -> out-proj x2 -> scaled
            # copy (DVE half, ACT half) -> per-tile y DMA, so successive
            # tiles overlap across engines. ----
            while pending_pv:
                emit_pv(*pending_pv.pop(0))
            y_tiles[3] = ypool.tile([P, 4, D_MODEL], bf16, tag="y_sb", name="y3")

            def finalize_q(j, t):
                """Per-128-query finalize: reciprocal reads the pv PSUM
                bank directly; the ot copy is one full-width ACT op emitted
                by the caller."""
                c0 = j * 512 + t * P
                nc.vector.reciprocal(
                    rden[:, c0 : c0 + P],
                    pv[j][D_HEAD : D_HEAD + 1, t * P : (t + 1) * P],
                )
                nc.tensor.matmul(
                    pv[j][:, t : t + 1],
                    lhsT=rden[:, c0 : c0 + P],
                    rhs=one_sb,
                    start=True,
                    stop=True,
                )
                nc.vector.tensor_copy(
                    out=rdent[:, 4 * j + t : 4 * j + t + 1],
                    in_=pv[j][:, t : t + 1],
                )

            # emission order keeps each engine FIFO unblocked: the recip
            # chains for tiles 0-2 all precede the first scale op. Each
            # out-op gets its own PSUM bank (score + mm banks are idle by
            # now) so no matmul waits on a previous scale's drain.
            tail_banks = [
                s_ps_pool.tile([P, 512], fp32, tag="s_ps", name=f"tb{k}")
                for k in range(3)
            ] + [
                mm_ps.tile([P, 512], fp32, tag="mm", name=f"tb{k + 3}")
                for k in range(2)
            ]
            nc.vector.tensor_copy(
                out=ot[0:D_HEAD, 1536:2048], in_=pv[3][0:D_HEAD, :]
            )
            nc.vector.reciprocal(
                rden[:, 1536:2048], pv[3][D_HEAD : D_HEAD + 1, :]
            )
            for t in range(4):
                nc.tensor.matmul(
                    pv[3][:, t : t + 1],
                    lhsT=rden[:, 1536 + t * P : 1536 + (t + 1) * P],
                    rhs=one_sb,
                    start=True,
                    stop=True,
                )
            nc.vector.tensor_copy(out=rdent[:, 12:16], in_=pv[3][:, 0:4])
            emit_out_op(3, 0, bank=tail_banks[0])
            emit_out_op(3, 1, bank=tail_banks[1])
            emit_out_op(3, 2, bank=tail_banks[2])
            emit_out_op(3, 3, bank=tail_banks[3])
            emit_out_op(3, 4, bank=tail_banks[4])
            emit_out_op(3, 5, bank=pv[1])
            emit_out_op(3, 6, bank=pv[2])
            # pv[3] frees as soon as the consolidated rdent copy has read
            # cols 0:4 (~right after the last PV), earlier than any scale
            emit_out_op(3, 7, bank=pv[3])

    nc.finalize()
    return nc


def _get_program():
    global _prog
    if _prog is None:
        _prog = _build_program()
    return _prog


def kernel(x, W_q, W_k, W_v, W_o):
    import ml_dtypes
    from concourse.bass_utils import run_bass_kernel_spmd

    bf = ml_dtypes.bfloat16
    nc = _get_program()

    x = np.asarray(x, dtype=np.float32)
    scale = np.float32(1.0 / np.sqrt(D_HEAD))
    wq_s = np.asarray(W_q, dtype=np.float32) * scale
    wkv = np.concatenate(
        [np.asarray(W_k, dtype=np.float32), np.asarray(W_v, dtype=np.float32)],
        axis=1,
    )  # [1024, 128]
    wq_part = wq_s.reshape(DCH, P, 64).transpose(1, 0, 2).reshape(P, 512)
    wkv_part = wkv.reshape(DCH, P, 128).transpose(1, 0, 2).reshape(P, 1024)
    # per-role exp bias for partner-band chunks: -30 kills the whole chunk
    # for role 0 (exp(s - 30) ~ 1e-10), 0 keeps it whole for role 1
    w_hosts = []
    for r in range(2):
        rb = np.full((P, 4), 0.0 if r == 1 else -30.0, dtype=np.float32)
        w_hosts.append(
            np.ascontiguousarray(
                np.concatenate([wq_part, wkv_part, rb], axis=1)
            ).astype(bf)
        )  # [128, 1540]
    wo_host = np.ascontiguousarray(np.asarray(W_o, dtype=np.float32)).astype(bf)

    in_maps = []
    for c in range(NCORES):
        b, r = c // 2, c % 2
        xt_b = x[b].T  # [1024, 4096]
        # permuted key order: position 2k holds this core's query block
        # 2k+r, position 2k+1 holds the partner block 2k+(1-r)
        cols = np.concatenate(
            [
                np.arange(512 * g, 512 * g + 512)
                for k in range(NSLOT)
                for g in (2 * k + r, 2 * k + 1 - r)
            ]
        )
        xt_host = np.ascontiguousarray(
            xt_b[:, cols].reshape(DCH, P, SEQ).transpose(1, 2, 0)
        ).astype(bf)  # [128, 4096, 8]
        in_maps.append(
            {
                "xt": xt_host,
                "w": w_hosts[r],
                "wo": wo_host,
            }
        )

    res = run_bass_kernel_spmd(nc, in_maps, core_ids=list(range(NCORES)))
    out = np.empty((BATCH, SEQ, D_MODEL), dtype=np.float32)
    for c in range(NCORES):
        b, r = c // 2, c % 2
        yv = np.asarray(res.results[c]["y"]).astype(np.float32)
        # y[j, p, t, :] -> query 512*(2j+r) + 128t + p
        yv = yv.transpose(0, 2, 1, 3)  # [j, t, p, m]
        for j in range(NSLOT):
            q0 = 512 * (2 * j + r)
            out[b, q0 : q0 + 512, :] = yv[j].reshape(512, D_MODEL)
    return out



# revision 139
# speedup vs baseline: 1.0703x; 1.0018x over previous
"""Trainium2 Bass kernel for single-head causal attention.

x:[4,4096,1024] f32, W_q/W_k/W_v:[1024,64], W_o:[64,1024].

Sharding: 8 cores = 4 batches x 2 query-stripe roles. Role r of a batch
owns query blocks {2j+r : j=0..3} (512 queries each). Program slot j has
key extent E[j] = (8j+8) 128-key chunks, which exactly covers role 1's
block 2j+1 and over-covers role 0's block 2j by 4 chunks (dead).

All per-core differences (which batch, which stripe, dead chunks) are
carried in the input data; one SPMD program runs on all 8 cores:
- x is shipped as [128, 4096, 8] (d_model-chunk partition, seq, chunk)
  so any column range is a fully contiguous DMA (no sub-512B-element
  descriptor penalty), streamed in arrival-ordered bites.
- The diagonal causal band masks (4 x [128,512]) are generated on the
  otherwise-idle Pool engine with affine_select, not DMAed.
- Partner-block chunks (band >= 4) take no mask op at all: the exp runs
  with a per-partition bias from the w tensor (-30 for role 0 => p ~
  1e-10, 0 for role 1), so role-0's dead chunks vanish from both the PV
  numerator and the denominator row.

bf16 everywhere on the matmul paths (end-to-end rel err ~5e-3 vs the
2e-2 gate). Engines: PE does all matmuls (62.3us busy, the roofline of
this schedule), ACT does exp + the slot-3 ot copies + half its output
scales, DVE does diag masks + kvt/qt copies + finalize + the other
scales. Attention chunks are emitted in waves matched to DMA arrival,
next wave's KV projection interleaved between chunks as PE filler, PV
matmuls lagged 10 chunks behind their scores. Waves are front-loaded
(each chunk sits in the earliest wave whose x data can feed it) with
the KV projection emitted before the Q projection inside each wave's
filler — they share the single mm PSUM bank and Q may wait on
later-arriving x, so allocated first it would block the KV chain via
WAR. Every slot finalize is consolidated into full-width ops (one
[64,512] ot copy, one [1,512] reciprocal, four 1-col rdent transposes,
one [P,4] rdent copy, reading the PV PSUM bank directly) so all eight
out-projection matmuls of a slot unblock at once; tail out-ops get
distinct PSUM banks and the y DMAs go out per 128-query tile to keep
the final HWDGE descriptor-gen chain (625ns each, serialized) short.
"""

import sys

for _p in ("/opt/trn_rl_repo",):
    if _p not in sys.path:
        sys.path.insert(0, _p)

import numpy as np

D_MODEL = 1024
D_HEAD = 64
SEQ = 4096
BATCH = 4
NCORES = 8
NQ = 2048          # queries per core
P = 128
DCH = D_MODEL // P  # 8 contraction chunks
NSLOT = 4           # query slots of 512
E = [8, 16, 24, 32]  # key chunks per slot
NWAVE = 8           # key superchunks of 512

# Attention chunks per wave: matched to DMA arrival order (early waves
# light), per-slot ascending kc, every chunk (j,kc) in wave >= kc//4.
WAVES = [
    [(0, k) for k in range(4)],
    [(0, k) for k in range(4, 8)] + [(1, k) for k in range(8)],
    [(1, k) for k in range(8, 12)] + [(2, k) for k in range(12)],
    [(1, k) for k in range(12, 16)] + [(3, k) for k in range(4)],
    [(2, k) for k in range(12, 16)] + [(3, k) for k in range(4, 12)],
    [(2, k) for k in range(16, 24)],
    [(3, k) for k in range(12, 16)],
    [(3, k) for k in range(16, 32)],
]

_prog = None


def _check_waves():
    seen = {}
    total = 0
    for w, wv in enumerate(WAVES):
        for j, kc in wv:
            assert kc // 4 <= w, (w, j, kc)
            assert seen.get(j, -1) == kc - 1, (j, kc)
            seen[j] = kc
            total += 1
    assert total == sum(E) == 80
    return {j: max(w for w, wv in enumerate(WAVES) if (j, E[j] - 1) in wv)
            for j in range(NSLOT)}


def _build_program():
    import concourse.bacc as bacc
    import concourse.mybir as mybir
    import concourse.tile as tile
    from concourse.masks import make_identity

    fp32 = mybir.dt.float32
    f32r = mybir.dt.float32r
    bf16 = mybir.dt.bfloat16
    nc = bacc.Bacc("TRN2", target_bir_lowering=False, debug=False)

    # w layout: [wq 0:512 | wkv 512:1536 | rbias 1536:1540]
    # rbias: -30 for role 0, 0 for role 1. Partner-band chunks (band >= 4)
    # run exp with this per-partition bias instead of a 0/1 mask multiply:
    # each slot's band region only ever covers its own partner block, which
    # is entirely dead for role 0 (exp(s - 30) ~ 1e-10) and entirely alive
    # for role 1 (bias 0). Replaces the 16 explicit partner-mask DVE ops.
    xt = nc.dram_tensor("xt", [P, SEQ, DCH], bf16, kind="ExternalInput")
    w = nc.dram_tensor("w", [P, DCH * 192 + 4], bf16, kind="ExternalInput")
    wo = nc.dram_tensor("wo", [D_HEAD, D_MODEL], bf16, kind="ExternalInput")
    y = nc.dram_tensor("y", [NSLOT, P, 4, D_MODEL], bf16, kind="ExternalOutput")

    last_wave = _check_waves()
    # output-projection ops (j, i): slot0 -> waves 2,3; slot1 -> 5,6;
    # slot2 -> wave 6 (where DVE is light: slot-3 chunks kc<24 need no
    # masks); slot3 -> tail (wave index NWAVE)
    out_sched = {wi: [] for wi in range(NWAVE + 1)}
    for j, tgt in ((0, (2, 2)), (1, (5, 6)), (2, (6, 6)), (3, (8, 8))):
        for i in range(8):
            out_sched[tgt[i // 4]].append((j, i))

    with tile.TileContext(nc) as tc:
        with (
            tc.tile_pool(name="singles", bufs=1) as singles,
            tc.tile_pool(name="work", bufs=10) as work,
            tc.tile_pool(name="ypool", bufs=2) as ypool,
            tc.tile_pool(name="mm_ps", bufs=2, space="PSUM") as mm_ps,
            tc.tile_pool(name="s_ps", bufs=3, space="PSUM") as s_ps_pool,
            tc.tile_pool(name="pv_ps", bufs=1, space="PSUM") as pv_pool,
        ):
            # ---- persistent SBUF ----
            w_sb = singles.tile([P, DCH * 192 + 4], bf16, tag="w_sb")
            xt_sb = singles.tile([P, SEQ, DCH], bf16, tag="xt_sb")
            msk_sb = singles.tile([P, 4, 512], bf16, tag="msk_sb")
            wo_sb = singles.tile([D_HEAD, D_MODEL], bf16, tag="wo_sb")
            kvt = singles.tile([P, SEQ], bf16, tag="kvt")  # 0:64 K^T, 64:128 V^T
            qt_sb = singles.tile([D_HEAD, NQ], bf16, tag="qt_sb")
            vaug = singles.tile([P, 32, D_HEAD + 1], bf16, tag="vaug")
            ot = singles.tile([D_HEAD + 1, NQ], bf16, tag="ot")
            rden = singles.tile([1, NQ], fp32, tag="rden")
            rbias = singles.tile([P, 1], fp32, tag="rbias")
            rdent = singles.tile([P, 16], fp32, tag="rdent")
            ident = singles.tile([P, D_HEAD], bf16, tag="ident")
            one_sb = singles.tile([1, 1], fp32, tag="one_sb")

            # ---- input DMAs (SP queue, ordered by first use) ----
            def ld_x(dst, src, s0):
                nc.sync.dma_start(
                    out=dst[:, s0 : s0 + 512, :], in_=src[:, s0 : s0 + 512, :]
                )

            def ld_xh(dst, src, s0, n=512):
                nc.sync.dma_start(
                    out=dst[:, s0 : s0 + n, :], in_=src[:, s0 : s0 + n, :]
                )

            nc.sync.dma_start(out=w_sb[:, 0:512], in_=w[:, 0:512])
            ld_xh(xt_sb, xt, 0, 128)
            ld_xh(xt_sb, xt, 128, 128)
            ld_xh(xt_sb, xt, 256, 128)
            nc.sync.dma_start(out=w_sb[:, 512:1536], in_=w[:, 512:1536])
            ld_xh(xt_sb, xt, 384, 128)
            nc.sync.dma_start(out=w_sb[:, 1536:1540], in_=w[:, 1536:1540])
            ld_x(xt_sb, xt, 512)       # chunk 1: wave-1 fresh keys
            ld_x(xt_sb, xt, 1024)      # chunk 2: slot-1 queries
            ld_x(xt_sb, xt, 2048)      # chunk 4: slot-2 queries
            nc.sync.dma_start(out=wo_sb, in_=wo[:, :])
            ld_x(xt_sb, xt, 1536)      # chunk 3
            ld_x(xt_sb, xt, 3072)      # chunk 6: slot-3 queries
            ld_x(xt_sb, xt, 2560)      # chunk 5
            ld_x(xt_sb, xt, 3584)      # chunk 7

            nc.vector.memset(one_sb, 1.0)
            nc.vector.memset(vaug[:, :, D_HEAD : D_HEAD + 1], 1.0)
            make_identity(nc, ident[D_HEAD:P, :])
            # fp32 per-partition exp bias (see w layout note)
            nc.vector.tensor_copy(out=rbias, in_=w_sb[:, 1536:1537])
            # causal band masks generated on the idle Pool engine:
            # msk_sb[p, c, jq] = 1 if 128c + p <= jq else 0, per band c
            for c in range(4):
                nc.gpsimd.memset(msk_sb[:, c, :], 1.0)
                nc.gpsimd.affine_select(
                    out=msk_sb[:, c, :],
                    in_=msk_sb[:, c, :],
                    compare_op=mybir.AluOpType.is_ge,
                    fill=0.0,
                    base=-128 * c,
                    channel_multiplier=-1,
                    pattern=[[1, 512]],
                )

            def kv_proj_mms(sc):
                """Generator: one KV-projection matmul per next() call."""
                kp = mm_ps.tile([P, 512], fp32, tag="mm")
                for dc in range(DCH):
                    nc.tensor.matmul(
                        kp,
                        lhsT=w_sb[:, 512 + dc * 128 : 512 + dc * 128 + 128],
                        rhs=xt_sb[:, sc * 512 : (sc + 1) * 512, dc],
                        start=(dc == 0),
                        stop=(dc == DCH - 1),
                    )
                    yield
                nc.vector.tensor_copy(
                    out=kvt[:, sc * 512 : (sc + 1) * 512], in_=kp
                )
                yield

            def transposes(sc):
                for t in range(4):  # V^T 128-col blocks -> natural V chunks
                    kc = sc * 4 + t
                    tp = s_ps_pool.tile([P, D_HEAD], bf16, tag="s_ps")
                    nc.tensor.transpose(
                        tp,
                        kvt[D_HEAD:P, kc * P : (kc + 1) * P],
                        ident[D_HEAD:P, :],
                    )
                    nc.vector.tensor_copy(out=vaug[:, kc, :D_HEAD], in_=tp)

            # PV accumulators: full-bank tiles, PV uses rows 0:65.
            # Slot 3 shares slot 0's bank: slot-0 accumulation ends in wave
            # 1 and its out-ops are confined to wave 2, while slot-3
            # accumulation starts in wave 3 (start=True clears the bank).
            pv = [
                pv_pool.tile([P, 512], fp32, tag=f"pv{g}", name=f"pv{g}")
                for g in range(3)
            ]
            pv.append(pv[0])
            freed = []  # pv banks released by finalized slots
            y_tiles = {}
            ncopy = [0]
            pending_pv = []

            def emit_pv(j, kc):
                nc.tensor.matmul(
                    pv[j][0 : D_HEAD + 1, :],
                    lhsT=vaug[:, kc, :],
                    rhs=pending_pv_pt.pop((j, kc)),
                    start=(kc == 0),
                    stop=(kc == E[j] - 1),
                    skip_group_check=True,
                )

            pending_pv_pt = {}

            def emit_chunk(j, kc, wv=0):
                sps = s_ps_pool.tile([P, 512], fp32, tag="s_ps")
                nc.tensor.matmul(
                    sps,
                    lhsT=kvt[0:D_HEAD, kc * P : (kc + 1) * P],
                    rhs=qt_sb[:, j * 512 : (j + 1) * 512],
                    start=True,
                    stop=True,
                )
                p_t = work.tile([P, 512], bf16, tag="p_t")
                band = kc - (E[j] - 8)
                if band >= 4:
                    # partner block: role-0 kills the whole chunk via the
                    # exp bias (exp(s - 30) ~ 1e-10); role 1 keeps it whole
                    nc.scalar.activation(
                        p_t, sps, mybir.ActivationFunctionType.Exp,
                        bias=rbias[:, 0:1],
                    )
                else:
                    nc.scalar.activation(
                        p_t, sps, mybir.ActivationFunctionType.Exp
                    )
                if 0 <= band < 4:
                    # diagonal band: per-query causal step mask
                    nc.vector.tensor_tensor(
                        p_t, p_t, msk_sb[:, band, :], mybir.AluOpType.mult
                    )
                pending_pv_pt[(j, kc)] = p_t
                pending_pv.append((j, kc))
                if len(pending_pv) > 10:
                    emit_pv(*pending_pv.pop(0))

            def emit_out_op(j, i, bank=None):
                """One output-projection matmul + scaled PSUM->SBUF copy."""
                t, no = i // 2, i % 2
                if bank is None:
                    bank = freed[ncopy[0] % len(freed)]
                    ncopy[0] += 1
                q0 = j * 512 + t * P
                nc.tensor.matmul(
                    bank,
                    lhsT=ot[0:D_HEAD, q0 : q0 + P],
                    rhs=wo_sb[:, no * 512 : (no + 1) * 512],
                    start=True,
                    stop=True,
                )
                if j == 3 and i % 2 == 0:
                    nc.scalar.mul(
                        y_tiles[j][:, t, no * 512 : (no + 1) * 512],
                        bank,
                        rdent[:, 4 * j + t : 4 * j + t + 1],
                    )
                else:
                    nc.vector.tensor_scalar_mul(
                        y_tiles[j][:, t, no * 512 : (no + 1) * 512],
                        bank,
                        rdent[:, 4 * j + t : 4 * j + t + 1],
                    )
                if j == 3:
                    if i % 2 == 1:
                        # per-tile DMA: few enough that HWDGE desc-gen
                        # (625ns each, serialized) stays off the tail path
                        nc.sync.dma_start(
                            out=y[j][:, t : t + 1, :],
                            in_=y_tiles[j][:, t : t + 1, :],
                        )
                elif i == 3:
                    nc.sync.dma_start(
                        out=y[j][:, 0:2, :], in_=y_tiles[j][:, 0:2, :]
                    )
                elif i == 7:
                    nc.sync.dma_start(
                        out=y[j][:, 2:4, :], in_=y_tiles[j][:, 2:4, :]
                    )

            def finalize_half(j, h):
                """Half of slot-j finalize: O^T + den, 1/den, rdent cols."""
                c0 = j * 512 + h * 256
                c1 = c0 + 256
                nc.vector.tensor_copy(
                    out=ot[:, c0:c1], in_=pv[j][0 : D_HEAD + 1, h * 256 : h * 256 + 256]
                )
                nc.vector.reciprocal(
                    rden[:, c0:c1], ot[D_HEAD : D_HEAD + 1, c0:c1]
                )
                for t in (2 * h, 2 * h + 1):
                    nc.tensor.matmul(
                        pv[j][:, t : t + 1],
                        lhsT=rden[:, j * 512 + t * P : j * 512 + (t + 1) * P],
                        rhs=one_sb,
                        start=True,
                        stop=True,
                    )
                nc.vector.tensor_copy(
                    out=rdent[:, 4 * j + 2 * h : 4 * j + 2 * h + 2],
                    in_=pv[j][:, 2 * h : 2 * h + 2],
                )

            def finalize(j):
                # consolidated (same shape as the slot-3 tail): full-width
                # ot copy + reciprocal reading the pv PSUM bank directly,
                # then the four rdent transposes and one rdent copy
                nc.vector.tensor_copy(
                    out=ot[0:D_HEAD, j * 512 : (j + 1) * 512],
                    in_=pv[j][0:D_HEAD, :],
                )
                nc.vector.reciprocal(
                    rden[:, j * 512 : (j + 1) * 512],
                    pv[j][D_HEAD : D_HEAD + 1, :],
                )
                for t in range(4):
                    nc.tensor.matmul(
                        pv[j][:, t : t + 1],
                        lhsT=rden[:, j * 512 + t * P : j * 512 + (t + 1) * P],
                        rhs=one_sb,
                        start=True,
                        stop=True,
                    )
                nc.vector.tensor_copy(
                    out=rdent[:, 4 * j : 4 * j + 4], in_=pv[j][:, 0:4]
                )
                freed.append(pv[j])
                y_tiles[j] = ypool.tile(
                    [P, 4, D_MODEL], bf16, tag="y_sb", name=f"y{j}"
                )

            def q_piece(c0, n):
                """Prologue Q projection over columns [c0, c0+n) of slot 0.
                Uses the s_ps pool so pieces rotate PSUM banks instead of
                serializing on the single mm bank."""
                qp = s_ps_pool.tile([D_HEAD, n], fp32, tag="s_ps", name=f"qp{c0}")
                for dc in range(DCH):
                    nc.tensor.matmul(
                        qp,
                        lhsT=w_sb[:, dc * 64 : dc * 64 + 64],
                        rhs=xt_sb[:, c0 : c0 + n, dc],
                        start=(dc == 0),
                        stop=(dc == DCH - 1),
                    )
                nc.vector.tensor_copy(out=qt_sb[:, c0 : c0 + n], in_=qp)

            def kv_piece(c0, n):
                kp = s_ps_pool.tile([P, n], fp32, tag="s_ps", name=f"kp{c0}")
                for dc in range(DCH):
                    nc.tensor.matmul(
                        kp,
                        lhsT=w_sb[:, 512 + dc * 128 : 512 + dc * 128 + 128],
                        rhs=xt_sb[:, c0 : c0 + n, dc],
                        start=(dc == 0),
                        stop=(dc == DCH - 1),
                    )
                nc.vector.tensor_copy(out=kvt[:, c0 : c0 + n], in_=kp)

            def q_proj_steps(j):
                """Generator version of q_proj: one matmul per next()."""
                qp = mm_ps.tile([D_HEAD, 512], fp32, tag="mm")
                for dc in range(DCH):
                    nc.tensor.matmul(
                        qp,
                        lhsT=w_sb[:, dc * 64 : dc * 64 + 64],
                        rhs=xt_sb[:, j * 1024 : j * 1024 + 512, dc],
                        start=(dc == 0),
                        stop=(dc == DCH - 1),
                    )
                    yield
                nc.vector.tensor_copy(
                    out=qt_sb[:, j * 512 : (j + 1) * 512], in_=qp
                )
                yield

            def transpose_steps(sc):
                tp = s_ps_pool.tile([P, 4, D_HEAD], bf16, tag="s_ps", name="tp4")
                for t in range(4):
                    kc = sc * 4 + t
                    nc.tensor.transpose(
                        tp[:, t, :],
                        kvt[D_HEAD:P, kc * P : (kc + 1) * P],
                        ident[D_HEAD:P, :],
                    )
                    yield
                nc.vector.tensor_copy(
                    out=vaug[:, sc * 4 : sc * 4 + 4, :D_HEAD], in_=tp
                )
                yield

            # Q projection for slot j runs as filler inside wave j (its
            # first consumers are that wave's chunks)
            qproj_wave = {1: 1, 2: 2, 3: 3}

            # ---- prologue + wave 0: piece-width projections matched to the
            # DMA bite arrival order (x cols 0:128, 128:384, 384:512). The
            # V-transposes for kc 0..1 must be emitted before chunk (0,3)
            # pops PV(0,0), else the vaug dependency is never recorded. ----
            tg0 = transpose_steps(0)
            q_piece(0, 128)
            q_piece(128, 128)
            q_piece(256, 128)
            kv_piece(0, 128)
            kv_piece(128, 256)
            q_piece(384, 128)
            kv_piece(384, 128)
            next(tg0)
            next(tg0)
            emit_chunk(0, 0, 0)
            emit_chunk(0, 1, 0)
            next(tg0)
            next(tg0)
            emit_chunk(0, 2, 0)
            emit_chunk(0, 3, 0)
            for _ in tg0:  # drain: emits the packed vaug copy for kc 0..3
                pass

            # ---- streamed waves ----
            # Wave sc fillers: [qproj (if due), KV proj for sc, spacer,
            # V transposes for sc], popped two per chunk. Wave 0's proj and
            # transposes ran in the prologue.
            for sc in range(1, NWAVE):
                chunks = WAVES[sc]
                oo = list(out_sched[sc])
                nq = 9 if sc in qproj_wave else 0
                # kv projection FIRST: both share the single mm PSUM bank,
                # and the q projection may wait on later-arriving x columns
                # — allocated first it would block the kv chain via WAR
                filler = [
                    kv_proj_mms(sc),
                    iter([None, None]),  # spacer: kvt copy drains
                    transpose_steps(sc),
                ]
                if nq:
                    filler.append(q_proj_steps(qproj_wave[sc]))
                total_fill = nq + 16
                fill_iter = (x for g in filler for x in g)
                pops = [0]

                def pop_fill(upto=None, k=None):
                    tgt = upto if upto is not None else pops[0] + k
                    while pops[0] < min(tgt, total_fill):
                        if next(fill_iter, -1) == -1:
                            pops[0] = total_fill
                            break
                        pops[0] += 1

                # emission-order safety points: a fresh chunk's scores need
                # this wave's kvt copy emitted; a fresh chunk's PV needs its
                # V-transpose emitted
                n = len(chunks)
                for ci, (j, kc) in enumerate(chunks):
                    if kc // 4 == sc:
                        pop_fill(upto=9)
                    if sc in qproj_wave and j == qproj_wave[sc]:
                        pop_fill(upto=total_fill)
                    if len(pending_pv) >= 4:
                        j2, kc2 = pending_pv[0]
                        if kc2 // 4 == sc:
                            pop_fill(upto=total_fill)
                    emit_chunk(j, kc, sc)
                    if oo:
                        emit_out_op(*oo.pop(0))
                    k = -(-(total_fill - pops[0]) // (n - ci))  # ceil
                    pop_fill(k=min(k, 3))
                pop_fill(upto=total_fill)
                for op in oo:
                    emit_out_op(*op)
                if sc == 2:
                    # pv[0] now belongs to slot-3 accumulation
                    freed.clear()
                fin = [j for j in range(NSLOT) if last_wave[j] == sc and j != 3]
                if fin:
                    while pending_pv:
                        emit_pv(*pending_pv.pop(0))
                    for j in fin:
                        finalize(j)

            # ---- slot-3 tail: per-128-query-tile pipeline. The ot copy and
            # the reciprocal both read the pv PSUM bank directly (no serial
            # copy->recip dependency), then rdent -> out-proj x2 -> scaled
            # copy (DVE half, ACT half) -> per-tile y DMA, so successive
            # tiles overlap across engines. ----
            while pending_pv:
                emit_pv(*pending_pv.pop(0))
            y_tiles[3] = ypool.tile([P, 4, D_MODEL], bf16, tag="y_sb", name="y3")

            def finalize_q(j, t):
                """Per-128-query finalize: reciprocal reads the pv PSUM
                bank directly; the ot copy is one full-width ACT op emitted
                by the caller."""
                c0 = j * 512 + t * P
                nc.vector.reciprocal(
                    rden[:, c0 : c0 + P],
                    pv[j][D_HEAD : D_HEAD + 1, t * P : (t + 1) * P],
                )
                nc.tensor.matmul(
                    pv[j][:, t : t + 1],
                    lhsT=rden[:, c0 : c0 + P],
                    rhs=one_sb,
                    start=True,
                    stop=True,
                )
                nc.vector.tensor_copy(
                    out=rdent[:, 4 * j + t : 4 * j + t + 1],
                    in_=pv[j][:, t : t + 1],
                )

            # emission order keeps each engine FIFO unblocked: the recip
            # chains for tiles 0-2 all precede the first scale op. Each
            # out-op gets its own PSUM bank (score + mm banks are idle by
            # now) so no matmul waits on a previous scale's drain.
            tail_banks = [
                s_ps_pool.tile([P, 512], fp32, tag="s_ps", name=f"tb{k}")
                for k in range(3)
            ] + [
                mm_ps.tile([P, 512], fp32, tag="mm", name=f"tb{k + 3}")
                for k in range(2)
            ]
            nc.vector.tensor_copy(
                out=ot[0:D_HEAD, 1536:2048], in_=pv[3][0:D_HEAD, :]
            )
            nc.vector.reciprocal(
                rden[:, 1536:2048], pv[3][D_HEAD : D_HEAD + 1, :]
            )
            for t in range(4):
                nc.tensor.matmul(
                    pv[3][:, t : t + 1],
                    lhsT=rden[:, 1536 + t * P : 1536 + (t + 1) * P],
                    rhs=one_sb,
                    start=True,
                    stop=True,
                )
            nc.vector.tensor_copy(out=rdent[:, 12:16], in_=pv[3][:, 0:4])
            emit_out_op(3, 0, bank=tail_banks[0])
            emit_out_op(3, 1, bank=tail_banks[1])
            emit_out_op(3, 2, bank=tail_banks[2])
            emit_out_op(3, 3, bank=tail_banks[3])
            emit_out_op(3, 4, bank=tail_banks[4])
            emit_out_op(3, 5, bank=pv[1])
            emit_out_op(3, 6, bank=pv[2])
            # pv[3] frees as soon as the consolidated rdent copy has read
            # cols 0:4 (~right after the last PV), earlier than any scale
            emit_out_op(3, 7, bank=pv[3])

    nc.finalize()
    return nc


def _get_program():
    global _prog
    if _prog is None:
        _prog = _build_program()
    return _prog


def kernel(x, W_q, W_k, W_v, W_o):
    import ml_dtypes
    from concourse.bass_utils import run_bass_kernel_spmd

    bf = ml_dtypes.bfloat16
    nc = _get_program()

    x = np.asarray(x, dtype=np.float32)
    scale = np.float32(1.0 / np.sqrt(D_HEAD))
    wq_s = np.asarray(W_q, dtype=np.float32) * scale
    wkv = np.concatenate(
        [np.asarray(W_k, dtype=np.float32), np.asarray(W_v, dtype=np.float32)],
        axis=1,
    )  # [1024, 128]
    wq_part = wq_s.reshape(DCH, P, 64).transpose(1, 0, 2).reshape(P, 512)
    wkv_part = wkv.reshape(DCH, P, 128).transpose(1, 0, 2).reshape(P, 1024)
    # per-role exp bias for partner-band chunks: -30 kills the whole chunk
    # for role 0 (exp(s - 30) ~ 1e-10), 0 keeps it whole for role 1
    w_hosts = []
    for r in range(2):
        rb = np.full((P, 4), 0.0 if r == 1 else -30.0, dtype=np.float32)
        w_hosts.append(
            np.ascontiguousarray(
                np.concatenate([wq_part, wkv_part, rb], axis=1)
            ).astype(bf)
        )  # [128, 1540]
    wo_host = np.ascontiguousarray(np.asarray(W_o, dtype=np.float32)).astype(bf)

    in_maps = []
    for c in range(NCORES):
        b, r = c // 2, c % 2
        xt_b = x[b].T  # [1024, 4096]
        # permuted key order: position 2k holds this core's query block
        # 2k+r, position 2k+1 holds the partner block 2k+(1-r)
        cols = np.concatenate(
            [
                np.arange(512 * g, 512 * g + 512)
                for k in range(NSLOT)
                for g in (2 * k + r, 2 * k + 1 - r)
            ]
        )
        xt_host = np.ascontiguousarray(
            xt_b[:, cols].reshape(DCH, P, SEQ).transpose(1, 2, 0)
        ).astype(bf)  # [128, 4096, 8]
        in_maps.append(
            {
                "xt": xt_host,
                "w": w_hosts[r],
                "wo": wo_host,
            }
        )

    res = run_bass_kernel_spmd(nc, in_maps, core_ids=list(range(NCORES)))
    out = np.empty((BATCH, SEQ, D_MODEL), dtype=np.float32)
    for c in range(NCORES):
        b, r = c // 2, c % 2
        yv = np.asarray(res.results[c]["y"]).astype(np.float32)
        # y[j, p, t, :] -> query 512*(2j+r) + 128t + p
        yv = yv.transpose(0, 2, 1, 3)  # [j, t, p, m]
        for j in range(NSLOT):
            q0 = 512 * (2 * j + r)
            out[b, q0 : q0 + 512, :] = yv[j].reshape(512, D_MODEL)
    return out



# revision 142
# speedup vs baseline: 1.0710x; 1.0007x over previous
"""Trainium2 Bass kernel for single-head causal attention.

x:[4,4096,1024] f32, W_q/W_k/W_v:[1024,64], W_o:[64,1024].

Sharding: 8 cores = 4 batches x 2 query-stripe roles. Role r of a batch
owns query blocks {2j+r : j=0..3} (512 queries each). Program slot j has
key extent E[j] = (8j+8) 128-key chunks, which exactly covers role 1's
block 2j+1 and over-covers role 0's block 2j by 4 chunks (dead).

All per-core differences (which batch, which stripe, dead chunks) are
carried in the input data; one SPMD program runs on all 8 cores:
- x is shipped as [128, 4096, 8] (d_model-chunk partition, seq, chunk)
  so any column range is a fully contiguous DMA (no sub-512B-element
  descriptor penalty), streamed in arrival-ordered bites.
- The diagonal causal band masks (4 x [128,512]) are generated on the
  otherwise-idle Pool engine with affine_select, not DMAed.
- Partner-block chunks (band >= 4) take no mask op at all: the exp runs
  with a per-partition bias from the w tensor (-30 for role 0 => p ~
  1e-10, 0 for role 1), so role-0's dead chunks vanish from both the PV
  numerator and the denominator row.

bf16 everywhere on the matmul paths (end-to-end rel err ~5e-3 vs the
2e-2 gate). Engines: PE does all matmuls (62.3us busy, the roofline of
this schedule), ACT does exp + the slot-3 ot copies + half its output
scales, DVE does diag masks + kvt/qt copies + finalize + the other
scales. Attention chunks are emitted in waves matched to DMA arrival,
next wave's KV projection interleaved between chunks as PE filler, PV
matmuls lagged 10 chunks behind their scores. Waves are front-loaded
(each chunk sits in the earliest wave whose x data can feed it) with
the KV projection emitted before the Q projection inside each wave's
filler — they share the single mm PSUM bank and Q may wait on
later-arriving x, so allocated first it would block the KV chain via
WAR. Every slot finalize is consolidated into full-width ops (one
[64,512] ot copy, one [1,512] reciprocal, four 1-col rdent transposes,
one [P,4] rdent copy, reading the PV PSUM bank directly) so all eight
out-projection matmuls of a slot unblock at once; tail out-ops get
distinct PSUM banks and the y DMAs go out per 128-query tile to keep
the final HWDGE descriptor-gen chain (625ns each, serialized) short.
"""

import sys

for _p in ("/opt/trn_rl_repo",):
    if _p not in sys.path:
        sys.path.insert(0, _p)

import numpy as np

D_MODEL = 1024
D_HEAD = 64
SEQ = 4096
BATCH = 4
NCORES = 8
NQ = 2048          # queries per core
P = 128
DCH = D_MODEL // P  # 8 contraction chunks
NSLOT = 4           # query slots of 512
E = [8, 16, 24, 32]  # key chunks per slot
NWAVE = 8           # key superchunks of 512

# Attention chunks per wave: matched to DMA arrival order (early waves
# light), per-slot ascending kc, every chunk (j,kc) in wave >= kc//4.
WAVES = [
    [(0, k) for k in range(4)],
    [(0, k) for k in range(4, 8)] + [(1, k) for k in range(8)],
    [(1, k) for k in range(8, 12)] + [(2, k) for k in range(12)],
    [(1, k) for k in range(12, 16)] + [(3, k) for k in range(4)],
    [(2, k) for k in range(12, 16)] + [(3, k) for k in range(4, 12)],
    [(2, k) for k in range(16, 24)],
    [(3, k) for k in range(12, 16)],
    [(3, k) for k in range(16, 32)],
]

_prog = None


def _check_waves():
    seen = {}
    total = 0
    for w, wv in enumerate(WAVES):
        for j, kc in wv:
            assert kc // 4 <= w, (w, j, kc)
            assert seen.get(j, -1) == kc - 1, (j, kc)
            seen[j] = kc
            total += 1
    assert total == sum(E) == 80
    return {j: max(w for w, wv in enumerate(WAVES) if (j, E[j] - 1) in wv)
            for j in range(NSLOT)}


def _build_program():
    import concourse.bacc as bacc
    import concourse.mybir as mybir
    import concourse.tile as tile
    from concourse.masks import make_identity

    fp32 = mybir.dt.float32
    f32r = mybir.dt.float32r
    bf16 = mybir.dt.bfloat16
    nc = bacc.Bacc("TRN2", target_bir_lowering=False, debug=False)

    # w layout: [wq 0:512 | wkv 512:1536 | rbias 1536:1540]
    # rbias: -30 for role 0, 0 for role 1. Partner-band chunks (band >= 4)
    # run exp with this per-partition bias instead of a 0/1 mask multiply:
    # each slot's band region only ever covers its own partner block, which
    # is entirely dead for role 0 (exp(s - 30) ~ 1e-10) and entirely alive
    # for role 1 (bias 0). Replaces the 16 explicit partner-mask DVE ops.
    xt = nc.dram_tensor("xt", [P, SEQ, DCH], bf16, kind="ExternalInput")
    w = nc.dram_tensor("w", [P, DCH * 192 + 4], bf16, kind="ExternalInput")
    wo = nc.dram_tensor("wo", [D_HEAD, D_MODEL], bf16, kind="ExternalInput")
    y = nc.dram_tensor("y", [NSLOT, P, 4, D_MODEL], bf16, kind="ExternalOutput")

    last_wave = _check_waves()
    # output-projection ops (j, i): slot0 -> waves 2,3; slot1 -> 5,6;
    # slot2 -> wave 6 (where DVE is light: slot-3 chunks kc<24 need no
    # masks); slot3 -> tail (wave index NWAVE)
    out_sched = {wi: [] for wi in range(NWAVE + 1)}
    for j, tgt in ((0, (2, 2)), (1, (5, 6)), (2, (6, 6)), (3, (8, 8))):
        for i in range(8):
            out_sched[tgt[i // 4]].append((j, i))

    with tile.TileContext(nc) as tc:
        with (
            tc.tile_pool(name="singles", bufs=1) as singles,
            tc.tile_pool(name="work", bufs=10) as work,
            tc.tile_pool(name="ypool", bufs=2) as ypool,
            tc.tile_pool(name="mm_ps", bufs=2, space="PSUM") as mm_ps,
            tc.tile_pool(name="s_ps", bufs=3, space="PSUM") as s_ps_pool,
            tc.tile_pool(name="pv_ps", bufs=1, space="PSUM") as pv_pool,
        ):
            # ---- persistent SBUF ----
            w_sb = singles.tile([P, DCH * 192 + 4], bf16, tag="w_sb")
            xt_sb = singles.tile([P, SEQ, DCH], bf16, tag="xt_sb")
            msk_sb = singles.tile([P, 4, 512], bf16, tag="msk_sb")
            wo_sb = singles.tile([D_HEAD, D_MODEL], bf16, tag="wo_sb")
            kvt = singles.tile([P, SEQ], bf16, tag="kvt")  # 0:64 K^T, 64:128 V^T
            qt_sb = singles.tile([D_HEAD, NQ], bf16, tag="qt_sb")
            vaug = singles.tile([P, 32, D_HEAD + 1], bf16, tag="vaug")
            ot = singles.tile([D_HEAD + 1, NQ], bf16, tag="ot")
            rden = singles.tile([1, NQ], fp32, tag="rden")
            rbias = singles.tile([P, 1], fp32, tag="rbias")
            rdent = singles.tile([P, 16], fp32, tag="rdent")
            ident = singles.tile([P, D_HEAD], bf16, tag="ident")
            one_sb = singles.tile([1, 1], fp32, tag="one_sb")

            # ---- input DMAs (SP queue, ordered by first use) ----
            def ld_x(dst, src, s0):
                nc.sync.dma_start(
                    out=dst[:, s0 : s0 + 512, :], in_=src[:, s0 : s0 + 512, :]
                )

            def ld_xh(dst, src, s0, n=512):
                nc.sync.dma_start(
                    out=dst[:, s0 : s0 + n, :], in_=src[:, s0 : s0 + n, :]
                )

            nc.sync.dma_start(out=w_sb[:, 0:512], in_=w[:, 0:512])
            ld_xh(xt_sb, xt, 0, 128)
            ld_xh(xt_sb, xt, 128, 128)
            ld_xh(xt_sb, xt, 256, 128)
            nc.sync.dma_start(out=w_sb[:, 512:1536], in_=w[:, 512:1536])
            ld_xh(xt_sb, xt, 384, 128)
            nc.sync.dma_start(out=w_sb[:, 1536:1540], in_=w[:, 1536:1540])
            ld_x(xt_sb, xt, 512)       # chunk 1: wave-1 fresh keys
            ld_x(xt_sb, xt, 1024)      # chunk 2: slot-1 queries
            ld_x(xt_sb, xt, 2048)      # chunk 4: slot-2 queries
            nc.sync.dma_start(out=wo_sb, in_=wo[:, :])
            ld_x(xt_sb, xt, 1536)      # chunk 3
            ld_x(xt_sb, xt, 3072)      # chunk 6: slot-3 queries
            ld_x(xt_sb, xt, 2560)      # chunk 5
            ld_x(xt_sb, xt, 3584)      # chunk 7

            nc.vector.memset(one_sb, 1.0)
            nc.vector.memset(vaug[:, :, D_HEAD : D_HEAD + 1], 1.0)
            make_identity(nc, ident[D_HEAD:P, :])
            # fp32 per-partition exp bias (see w layout note)
            nc.vector.tensor_copy(out=rbias, in_=w_sb[:, 1536:1537])
            # causal band masks generated on the idle Pool engine:
            # msk_sb[p, c, jq] = 1 if 128c + p <= jq else 0, per band c
            for c in range(4):
                nc.gpsimd.memset(msk_sb[:, c, :], 1.0)
                nc.gpsimd.affine_select(
                    out=msk_sb[:, c, :],
                    in_=msk_sb[:, c, :],
                    compare_op=mybir.AluOpType.is_ge,
                    fill=0.0,
                    base=-128 * c,
                    channel_multiplier=-1,
                    pattern=[[1, 512]],
                )

            def kv_proj_mms(sc):
                """Generator: one KV-projection matmul per next() call."""
                kp = mm_ps.tile([P, 512], fp32, tag="mm")
                for dc in range(DCH):
                    nc.tensor.matmul(
                        kp,
                        lhsT=w_sb[:, 512 + dc * 128 : 512 + dc * 128 + 128],
                        rhs=xt_sb[:, sc * 512 : (sc + 1) * 512, dc],
                        start=(dc == 0),
                        stop=(dc == DCH - 1),
                    )
                    yield
                nc.vector.tensor_copy(
                    out=kvt[:, sc * 512 : (sc + 1) * 512], in_=kp
                )
                yield

            def transposes(sc):
                for t in range(4):  # V^T 128-col blocks -> natural V chunks
                    kc = sc * 4 + t
                    tp = s_ps_pool.tile([P, D_HEAD], bf16, tag="s_ps")
                    nc.tensor.transpose(
                        tp,
                        kvt[D_HEAD:P, kc * P : (kc + 1) * P],
                        ident[D_HEAD:P, :],
                    )
                    nc.vector.tensor_copy(out=vaug[:, kc, :D_HEAD], in_=tp)

            # PV accumulators: full-bank tiles, PV uses rows 0:65.
            # Slot 3 shares slot 0's bank: slot-0 accumulation ends in wave
            # 1 and its out-ops are confined to wave 2, while slot-3
            # accumulation starts in wave 3 (start=True clears the bank).
            pv = [
                pv_pool.tile([P, 512], fp32, tag=f"pv{g}", name=f"pv{g}")
                for g in range(3)
            ]
            pv.append(pv[0])
            freed = []  # pv banks released by finalized slots
            y_tiles = {}
            ncopy = [0]
            pending_pv = []

            def emit_pv(j, kc):
                nc.tensor.matmul(
                    pv[j][0 : D_HEAD + 1, :],
                    lhsT=vaug[:, kc, :],
                    rhs=pending_pv_pt.pop((j, kc)),
                    start=(kc == 0),
                    stop=(kc == E[j] - 1),
                    skip_group_check=True,
                )

            pending_pv_pt = {}

            def emit_chunk(j, kc, wv=0):
                sps = s_ps_pool.tile([P, 512], fp32, tag="s_ps")
                nc.tensor.matmul(
                    sps,
                    lhsT=kvt[0:D_HEAD, kc * P : (kc + 1) * P],
                    rhs=qt_sb[:, j * 512 : (j + 1) * 512],
                    start=True,
                    stop=True,
                )
                p_t = work.tile([P, 512], bf16, tag="p_t")
                band = kc - (E[j] - 8)
                if band >= 4:
                    # partner block: role-0 kills the whole chunk via the
                    # exp bias (exp(s - 30) ~ 1e-10); role 1 keeps it whole
                    nc.scalar.activation(
                        p_t, sps, mybir.ActivationFunctionType.Exp,
                        bias=rbias[:, 0:1],
                    )
                else:
                    nc.scalar.activation(
                        p_t, sps, mybir.ActivationFunctionType.Exp
                    )
                if 0 <= band < 4:
                    # diagonal band: per-query causal step mask
                    nc.vector.tensor_tensor(
                        p_t, p_t, msk_sb[:, band, :], mybir.AluOpType.mult
                    )
                pending_pv_pt[(j, kc)] = p_t
                pending_pv.append((j, kc))
                if len(pending_pv) > 9:
                    emit_pv(*pending_pv.pop(0))

            def emit_out_op(j, i, bank=None):
                """One output-projection matmul + scaled PSUM->SBUF copy."""
                t, no = i // 2, i % 2
                if bank is None:
                    bank = freed[ncopy[0] % len(freed)]
                    ncopy[0] += 1
                q0 = j * 512 + t * P
                nc.tensor.matmul(
                    bank,
                    lhsT=ot[0:D_HEAD, q0 : q0 + P],
                    rhs=wo_sb[:, no * 512 : (no + 1) * 512],
                    start=True,
                    stop=True,
                )
                if j == 3 and i % 2 == 0:
                    nc.scalar.mul(
                        y_tiles[j][:, t, no * 512 : (no + 1) * 512],
                        bank,
                        rdent[:, 4 * j + t : 4 * j + t + 1],
                    )
                else:
                    nc.vector.tensor_scalar_mul(
                        y_tiles[j][:, t, no * 512 : (no + 1) * 512],
                        bank,
                        rdent[:, 4 * j + t : 4 * j + t + 1],
                    )
                if j == 3:
                    if i % 2 == 1:
                        # per-tile DMA: few enough that HWDGE desc-gen
                        # (625ns each, serialized) stays off the tail path
                        nc.sync.dma_start(
                            out=y[j][:, t : t + 1, :],
                            in_=y_tiles[j][:, t : t + 1, :],
                        )
                elif i == 3:
                    nc.sync.dma_start(
                        out=y[j][:, 0:2, :], in_=y_tiles[j][:, 0:2, :]
                    )
                elif i == 7:
                    nc.sync.dma_start(
                        out=y[j][:, 2:4, :], in_=y_tiles[j][:, 2:4, :]
                    )

            def finalize_half(j, h):
                """Half of slot-j finalize: O^T + den, 1/den, rdent cols."""
                c0 = j * 512 + h * 256
                c1 = c0 + 256
                nc.vector.tensor_copy(
                    out=ot[:, c0:c1], in_=pv[j][0 : D_HEAD + 1, h * 256 : h * 256 + 256]
                )
                nc.vector.reciprocal(
                    rden[:, c0:c1], ot[D_HEAD : D_HEAD + 1, c0:c1]
                )
                for t in (2 * h, 2 * h + 1):
                    nc.tensor.matmul(
                        pv[j][:, t : t + 1],
                        lhsT=rden[:, j * 512 + t * P : j * 512 + (t + 1) * P],
                        rhs=one_sb,
                        start=True,
                        stop=True,
                    )
                nc.vector.tensor_copy(
                    out=rdent[:, 4 * j + 2 * h : 4 * j + 2 * h + 2],
                    in_=pv[j][:, 2 * h : 2 * h + 2],
                )

            def finalize(j):
                # consolidated (same shape as the slot-3 tail): full-width
                # ot copy + reciprocal reading the pv PSUM bank directly,
                # then the four rdent transposes and one rdent copy
                nc.vector.tensor_copy(
                    out=ot[0:D_HEAD, j * 512 : (j + 1) * 512],
                    in_=pv[j][0:D_HEAD, :],
                )
                nc.vector.reciprocal(
                    rden[:, j * 512 : (j + 1) * 512],
                    pv[j][D_HEAD : D_HEAD + 1, :],
                )
                for t in range(4):
                    nc.tensor.matmul(
                        pv[j][:, t : t + 1],
                        lhsT=rden[:, j * 512 + t * P : j * 512 + (t + 1) * P],
                        rhs=one_sb,
                        start=True,
                        stop=True,
                    )
                nc.vector.tensor_copy(
                    out=rdent[:, 4 * j : 4 * j + 4], in_=pv[j][:, 0:4]
                )
                freed.append(pv[j])
                y_tiles[j] = ypool.tile(
                    [P, 4, D_MODEL], bf16, tag="y_sb", name=f"y{j}"
                )

            def q_piece(c0, n):
                """Prologue Q projection over columns [c0, c0+n) of slot 0.
                Uses the s_ps pool so pieces rotate PSUM banks instead of
                serializing on the single mm bank."""
                qp = s_ps_pool.tile([D_HEAD, n], fp32, tag="s_ps", name=f"qp{c0}")
                for dc in range(DCH):
                    nc.tensor.matmul(
                        qp,
                        lhsT=w_sb[:, dc * 64 : dc * 64 + 64],
                        rhs=xt_sb[:, c0 : c0 + n, dc],
                        start=(dc == 0),
                        stop=(dc == DCH - 1),
                    )
                nc.vector.tensor_copy(out=qt_sb[:, c0 : c0 + n], in_=qp)

            def kv_piece(c0, n):
                kp = s_ps_pool.tile([P, n], fp32, tag="s_ps", name=f"kp{c0}")
                for dc in range(DCH):
                    nc.tensor.matmul(
                        kp,
                        lhsT=w_sb[:, 512 + dc * 128 : 512 + dc * 128 + 128],
                        rhs=xt_sb[:, c0 : c0 + n, dc],
                        start=(dc == 0),
                        stop=(dc == DCH - 1),
                    )
                nc.vector.tensor_copy(out=kvt[:, c0 : c0 + n], in_=kp)

            def q_proj_steps(j):
                """Generator version of q_proj: one matmul per next()."""
                qp = mm_ps.tile([D_HEAD, 512], fp32, tag="mm")
                for dc in range(DCH):
                    nc.tensor.matmul(
                        qp,
                        lhsT=w_sb[:, dc * 64 : dc * 64 + 64],
                        rhs=xt_sb[:, j * 1024 : j * 1024 + 512, dc],
                        start=(dc == 0),
                        stop=(dc == DCH - 1),
                    )
                    yield
                nc.vector.tensor_copy(
                    out=qt_sb[:, j * 512 : (j + 1) * 512], in_=qp
                )
                yield

            def transpose_steps(sc):
                tp = s_ps_pool.tile([P, 4, D_HEAD], bf16, tag="s_ps", name="tp4")
                for t in range(4):
                    kc = sc * 4 + t
                    nc.tensor.transpose(
                        tp[:, t, :],
                        kvt[D_HEAD:P, kc * P : (kc + 1) * P],
                        ident[D_HEAD:P, :],
                    )
                    yield
                nc.vector.tensor_copy(
                    out=vaug[:, sc * 4 : sc * 4 + 4, :D_HEAD], in_=tp
                )
                yield

            # Q projection for slot j runs as filler inside wave j (its
            # first consumers are that wave's chunks)
            qproj_wave = {1: 1, 2: 2, 3: 3}

            # ---- prologue + wave 0: piece-width projections matched to the
            # DMA bite arrival order (x cols 0:128, 128:384, 384:512). The
            # V-transposes for kc 0..1 must be emitted before chunk (0,3)
            # pops PV(0,0), else the vaug dependency is never recorded. ----
            tg0 = transpose_steps(0)
            q_piece(0, 128)
            q_piece(128, 128)
            q_piece(256, 128)
            kv_piece(0, 128)
            kv_piece(128, 256)
            q_piece(384, 128)
            kv_piece(384, 128)
            next(tg0)
            next(tg0)
            emit_chunk(0, 0, 0)
            emit_chunk(0, 1, 0)
            next(tg0)
            next(tg0)
            emit_chunk(0, 2, 0)
            emit_chunk(0, 3, 0)
            for _ in tg0:  # drain: emits the packed vaug copy for kc 0..3
                pass

            # ---- streamed waves ----
            # Wave sc fillers: [qproj (if due), KV proj for sc, spacer,
            # V transposes for sc], popped two per chunk. Wave 0's proj and
            # transposes ran in the prologue.
            for sc in range(1, NWAVE):
                chunks = WAVES[sc]
                oo = list(out_sched[sc])
                nq = 9 if sc in qproj_wave else 0
                # kv projection FIRST: both share the single mm PSUM bank,
                # and the q projection may wait on later-arriving x columns
                # — allocated first it would block the kv chain via WAR
                filler = [
                    kv_proj_mms(sc),
                    iter([None, None]),  # spacer: kvt copy drains
                    transpose_steps(sc),
                ]
                if nq:
                    filler.append(q_proj_steps(qproj_wave[sc]))
                total_fill = nq + 16
                fill_iter = (x for g in filler for x in g)
                pops = [0]

                def pop_fill(upto=None, k=None):
                    tgt = upto if upto is not None else pops[0] + k
                    while pops[0] < min(tgt, total_fill):
                        if next(fill_iter, -1) == -1:
                            pops[0] = total_fill
                            break
                        pops[0] += 1

                # emission-order safety points: a fresh chunk's scores need
                # this wave's kvt copy emitted; a fresh chunk's PV needs its
                # V-transpose emitted
                n = len(chunks)
                for ci, (j, kc) in enumerate(chunks):
                    if kc // 4 == sc:
                        pop_fill(upto=9)
                    if sc in qproj_wave and j == qproj_wave[sc]:
                        pop_fill(upto=total_fill)
                    if len(pending_pv) >= 4:
                        j2, kc2 = pending_pv[0]
                        if kc2 // 4 == sc:
                            pop_fill(upto=total_fill)
                    emit_chunk(j, kc, sc)
                    if oo:
                        emit_out_op(*oo.pop(0))
                    k = -(-(total_fill - pops[0]) // (n - ci))  # ceil
                    pop_fill(k=min(k, 3))
                pop_fill(upto=total_fill)
                for op in oo:
                    emit_out_op(*op)
                if sc == 2:
                    # pv[0] now belongs to slot-3 accumulation
                    freed.clear()
                fin = [j for j in range(NSLOT) if last_wave[j] == sc and j != 3]
                if fin:
                    while pending_pv:
                        emit_pv(*pending_pv.pop(0))
                    for j in fin:
                        finalize(j)

            # ---- slot-3 tail: per-128-query-tile pipeline. The ot copy and
            # the reciprocal both read the pv PSUM bank directly (no serial
            # copy->recip dependency), then rdent -> out-proj x2 -> scaled
            # copy (DVE half, ACT half) -> per-tile y DMA, so successive
            # tiles overlap across engines. ----
            while pending_pv:
                emit_pv(*pending_pv.pop(0))
            y_tiles[3] = ypool.tile([P, 4, D_MODEL], bf16, tag="y_sb", name="y3")

            def finalize_q(j, t):
                """Per-128-query finalize: reciprocal reads the pv PSUM
                bank directly; the ot copy is one full-width ACT op emitted
                by the caller."""
                c0 = j * 512 + t * P
                nc.vector.reciprocal(
                    rden[:, c0 : c0 + P],
                    pv[j][D_HEAD : D_HEAD + 1, t * P : (t + 1) * P],
                )
                nc.tensor.matmul(
                    pv[j][:, t : t + 1],
                    lhsT=rden[:, c0 : c0 + P],
                    rhs=one_sb,
                    start=True,
                    stop=True,
                )
                nc.vector.tensor_copy(
                    out=rdent[:, 4 * j + t : 4 * j + t + 1],
                    in_=pv[j][:, t : t + 1],
                )

            # emission order keeps each engine FIFO unblocked: the recip
            # chains for tiles 0-2 all precede the first scale op. Each
            # out-op gets its own PSUM bank (score + mm banks are idle by
            # now) so no matmul waits on a previous scale's drain.
            tail_banks = [
                s_ps_pool.tile([P, 512], fp32, tag="s_ps", name=f"tb{k}")
                for k in range(3)
            ] + [
                mm_ps.tile([P, 512], fp32, tag="mm", name=f"tb{k + 3}")
                for k in range(2)
            ]
            nc.vector.tensor_copy(
                out=ot[0:D_HEAD, 1536:2048], in_=pv[3][0:D_HEAD, :]
            )
            nc.vector.reciprocal(
                rden[:, 1536:2048], pv[3][D_HEAD : D_HEAD + 1, :]
            )
            for t in range(4):
                nc.tensor.matmul(
                    pv[3][:, t : t + 1],
                    lhsT=rden[:, 1536 + t * P : 1536 + (t + 1) * P],
                    rhs=one_sb,
                    start=True,
                    stop=True,
                )
            nc.vector.tensor_copy(out=rdent[:, 12:16], in_=pv[3][:, 0:4])
            emit_out_op(3, 0, bank=tail_banks[0])
            emit_out_op(3, 1, bank=tail_banks[1])
            emit_out_op(3, 2, bank=tail_banks[2])
            emit_out_op(3, 3, bank=tail_banks[3])
            emit_out_op(3, 4, bank=tail_banks[4])
            emit_out_op(3, 5, bank=pv[1])
            emit_out_op(3, 6, bank=pv[2])
            # pv[3] frees as soon as the consolidated rdent copy has read
            # cols 0:4 (~right after the last PV), earlier than any scale
            emit_out_op(3, 7, bank=pv[3])

    nc.finalize()
    return nc


def _get_program():
    global _prog
    if _prog is None:
        _prog = _build_program()
    return _prog


def kernel(x, W_q, W_k, W_v, W_o):
    import ml_dtypes
    from concourse.bass_utils import run_bass_kernel_spmd

    bf = ml_dtypes.bfloat16
    nc = _get_program()

    x = np.asarray(x, dtype=np.float32)
    scale = np.float32(1.0 / np.sqrt(D_HEAD))
    wq_s = np.asarray(W_q, dtype=np.float32) * scale
    wkv = np.concatenate(
        [np.asarray(W_k, dtype=np.float32), np.asarray(W_v, dtype=np.float32)],
        axis=1,
    )  # [1024, 128]
    wq_part = wq_s.reshape(DCH, P, 64).transpose(1, 0, 2).reshape(P, 512)
    wkv_part = wkv.reshape(DCH, P, 128).transpose(1, 0, 2).reshape(P, 1024)
    # per-role exp bias for partner-band chunks: -30 kills the whole chunk
    # for role 0 (exp(s - 30) ~ 1e-10), 0 keeps it whole for role 1
    w_hosts = []
    for r in range(2):
        rb = np.full((P, 4), 0.0 if r == 1 else -30.0, dtype=np.float32)
        w_hosts.append(
            np.ascontiguousarray(
                np.concatenate([wq_part, wkv_part, rb], axis=1)
            ).astype(bf)
        )  # [128, 1540]
    wo_host = np.ascontiguousarray(np.asarray(W_o, dtype=np.float32)).astype(bf)

    in_maps = []
    for c in range(NCORES):
        b, r = c // 2, c % 2
        xt_b = x[b].T  # [1024, 4096]
        # permuted key order: position 2k holds this core's query block
        # 2k+r, position 2k+1 holds the partner block 2k+(1-r)
        cols = np.concatenate(
            [
                np.arange(512 * g, 512 * g + 512)
                for k in range(NSLOT)
                for g in (2 * k + r, 2 * k + 1 - r)
            ]
        )
        xt_host = np.ascontiguousarray(
            xt_b[:, cols].reshape(DCH, P, SEQ).transpose(1, 2, 0)
        ).astype(bf)  # [128, 4096, 8]
        in_maps.append(
            {
                "xt": xt_host,
                "w": w_hosts[r],
                "wo": wo_host,
            }
        )

    res = run_bass_kernel_spmd(nc, in_maps, core_ids=list(range(NCORES)))
    out = np.empty((BATCH, SEQ, D_MODEL), dtype=np.float32)
    for c in range(NCORES):
        b, r = c // 2, c % 2
        yv = np.asarray(res.results[c]["y"]).astype(np.float32)
        # y[j, p, t, :] -> query 512*(2j+r) + 128t + p
        yv = yv.transpose(0, 2, 1, 3)  # [j, t, p, m]
        for j in range(NSLOT):
            q0 = 512 * (2 * j + r)
            out[b, q0 : q0 + 512, :] = yv[j].reshape(512, D_MODEL)
    return out

